# revision 1
# baseline (speedup 1.0000x reference)
# Trainium2 Bass kernel for nn_Cross_Transformer (dense_transformer).
#
# Sharding: 8 cores = 2 towers x 2 batches x 2 sequence-halves.
# Each core computes block0 (self-attention) in full (its inputs are permuted
# so its own half leads, keeping the program SPMD-uniform), then its half of
# block1 (cross-attention), pool, and final projection. No collectives.
#
# Layout: activations are feature-major [D on partitions, S on free] so every
# projection is lhsT=W^T tiles (stationary) x X' (moving). Matmuls run as
# float32r (1 cyc/row at N>=256). Attention probabilities/V run in fp16.
# LayerNorm over D (= partitions) uses ones-column matmuls for sums and a
# K=1 ones-row matmul to broadcast per-column stats across partitions.

import math

import numpy as np

S = 1024
B = 2
D = 768
H = 12
DH = 64
EPS = 1e-6
SH = S // 2  # 512, per-core block1 rows
P = 128
ND = D // P  # 6 d-tiles
NS = S // P  # 8 s-tiles
VW = H * (DH + 1)  # 780: v row-major padded with a ones column per head

F32 = None  # filled lazily (mybir.dt.float32)
_BUILT = {}


def _dt():
    from concourse import mybir

    return mybir.dt


def _r(ap):
    """View an fp32 AP as float32r for full-rate PE matmuls; fp16 passes through."""
    dt = _dt()
    return ap.bitcast(dt.float32r) if ap.dtype == dt.float32 else ap


def _build_program():
    import concourse.bacc as bacc
    import concourse.tile as tile
    from concourse import mybir
    from concourse.masks import make_identity

    dt = mybir.dt
    f32 = dt.float32
    f16 = dt.float16
    AF = mybir.ActivationFunctionType
    OP = mybir.AluOpType

    nc = bacc.Bacc("TRN2", target_bir_lowering=False, debug=False, num_devices=8)

    # ---- DRAM I/O ----
    din = {}

    def dram_in(name, shape, dty=None):
        din[name] = nc.dram_tensor(
            name, list(shape), dty or f16, kind="ExternalInput"
        )
        return din[name]

    dram_in("srcT", (D, S))
    dram_in("s1T", (D, SH))
    for li in (0, 1):
        dram_in(f"l{li}_qT", (D, D))
        dram_in(f"l{li}_kT", (D, D))
        dram_in(f"l{li}_vTp", (D + 1, VW))
        dram_in(f"l{li}_oT", (D, D))
        dram_in(f"l{li}_w1T", (D, D))
        dram_in(f"l{li}_w2T", (D, D))
        for bn in ("bq", "bk", "bo", "b1", "b2", "ag", "ab", "fg", "fb"):
            dram_in(f"l{li}_{bn}", (P, ND), f32)
    dram_in("pwT", (S + 1, S))
    dram_in("finT", (2 * D, D))
    dram_in("finb", (P, ND), f32)

    outT = nc.dram_tensor("outT", [D, SH], f32, kind="ExternalOutput")
    scr1 = nc.dram_tensor("scr1", [SH * D], f16, kind="Internal")
    scr2 = nc.dram_tensor("scr2", [SH * D], f16, kind="Internal")

    with tile.TileContext(nc) as tc:
        _emit(nc, tc, tile, dt, AF, OP, din, outT, scr1, scr2, make_identity)

    nc.compile()
    return nc


def _emit(nc, tc, tile, dt, AF, OP, din, outT, scr1, scr2, make_identity):
    f32 = dt.float32
    f16 = dt.float16
    import contextlib

    es = contextlib.ExitStack()
    with es:
        persist = es.enter_context(tc.tile_pool(name="persist", bufs=1))
        wp = es.enter_context(tc.tile_pool(name="wp", bufs=13))
        psA = es.enter_context(tc.tile_pool(name="psA", bufs=2, space="PSUM"))
        psC = es.enter_context(tc.tile_pool(name="psC", bufs=1, space="PSUM"))
        psS = es.enter_context(tc.tile_pool(name="psS", bufs=2, space="PSUM"))
        expool = es.enter_context(tc.tile_pool(name="expool", bufs=6))
        ctxp = es.enter_context(tc.tile_pool(name="ctxp", bufs=3))
        sqp = es.enter_context(tc.tile_pool(name="sqp", bufs=4))
        brp = es.enter_context(tc.tile_pool(name="brp", bufs=4))
        smp = es.enter_context(tc.tile_pool(name="smp", bufs=4))

        # --- constants ---
        ident = persist.tile([P, P], f16, name="ident")
        make_identity(nc, ident)
        ones_col = persist.tile([P, 1], f16, name="ones_col")
        nc.vector.memset(ones_col, 1.0)
        ones_r64 = persist.tile([DH + 1, DH], f16, name="ones_r64")
        nc.vector.memset(ones_r64[DH : DH + 1, :], 1.0)
        ones_r128 = persist.tile([1, P], f16, name="ones_r128")
        nc.vector.memset(ones_r128, 1.0)

        # --- small params (biases, LN) ---
        par = {}
        for li in (0, 1):
            for bn in ("bq", "bk", "bo", "b1", "b2", "ag", "ab", "fg", "fb"):
                t = persist.tile([P, ND], f32, name=f"p_l{li}_{bn}")
                nc.gpsimd.dma_start(out=t, in_=din[f"l{li}_{bn}"].ap())
                par[f"l{li}_{bn}"] = t
        finb = persist.tile([P, ND], f32, name="p_finb")
        nc.gpsimd.dma_start(out=finb, in_=din["finb"].ap())

        # persistent activations
        FEATS = persist.tile([P, ND + 1, S], f16, name="FEATS")
        nc.vector.memset(FEATS[0:1, ND, :], 1.0)
        S1T = persist.tile([P, ND, SH], f16, name="S1T")
        nc.sync.dma_start(
            out=S1T, in_=din["s1T"].ap().rearrange("(j p) s -> p j s", p=P)
        )
        CTX1p = persist.tile([P, ND, SH], f16, name="CTX1p")
        Q1 = persist.tile([P, ND, SH], f16, name="Q1")
        rb_t = persist.tile([P, S], f16, name="rb_t")
        mrb_t = persist.tile([P, S], f16, name="mrb_t")
        # LN small stats rows
        lnm = persist.tile([1, S], f32, name="lnm")
        lns2 = persist.tile([1, S], f32, name="lns2")
        lnt = persist.tile([1, S], f32, name="lnt")
        lnr16 = persist.tile([1, S], f16, name="lnr16")
        lnmr16 = persist.tile([1, S], f16, name="lnmr16")

        def load_w(dram_h, width, nk, tagsuf=""):
            """DMA weight k-tiles [P, width] (+ optional trailing [1, width])."""
            ap = dram_h.ap()
            tiles = []
            for t in range(nk):
                wt = wp.tile([P, width], f16, tag="w", name=f"w_{dram_h.name}_{t}")
                nc.sync.dma_start(out=wt, in_=ap[t * P : (t + 1) * P, :])
                tiles.append(wt)
            return tiles

        def fm_proj(x_ktiles, w_tiles, Sx, evac, extra_k=None):
            """Feature-major projection: out[m] = sum_k w[k][:,m].T @ x[k].
            x_ktiles: list of APs [kp, Sx]; w_tiles: list of APs [kp, D].
            evac(m, ps): consume psum [P, Sx]."""
            nch = Sx // 512
            ks = list(zip(x_ktiles, w_tiles))
            if extra_k is not None:
                ks.append(extra_k)
            for m in range(ND):
                ps = psA.tile([P, Sx], f32, tag="psA", name=f"ps_m{m}")
                for c in range(nch):
                    sl = slice(512 * c, 512 * (c + 1))
                    for ki, (xk, wk) in enumerate(ks):
                        nc.tensor.matmul(
                            ps[:, sl],
                            _r(wk[:, m * P : (m + 1) * P]),
                            _r(xk[:, sl]),
                            start=(ki == 0),
                            stop=(ki == len(ks) - 1),
                        )
                evac(m, ps)

        def layernorm(Zt, Sx, g_t, b_t, out_fn):
            """LN over partitions(d) of Zt [P, ND, Sx] (Bessel std + eps).
            out_fn(k, c, src_ap, sl): writes result tile."""
            nch = Sx // 512
            for c in range(nch):
                sl = slice(512 * c, 512 * (c + 1))
                sum_ps = psS.tile([1, 512], f32, tag="psS", name=f"lnsum{c}")
                for k in range(ND):
                    nc.tensor.matmul(
                        sum_ps,
                        _r(ones_col),
                        _r(Zt[:, k, sl]),
                        start=(k == 0),
                        stop=(k == ND - 1),
                    )
                nc.scalar.activation(
                    lnm[:, sl], sum_ps, AF.Identity, scale=1.0 / D
                )
                sq_ps = psS.tile([1, 512], f32, tag="psS", name=f"lnsq{c}")
                for k in range(ND):
                    sq = sqp.tile([P, 512], f16, tag="sq", name=f"sq{k}{c}")
                    nc.vector.tensor_mul(sq, Zt[:, k, sl], Zt[:, k, sl])
                    nc.tensor.matmul(
                        sq_ps,
                        _r(ones_col),
                        _r(sq),
                        start=(k == 0),
                        stop=(k == ND - 1),
                    )
                nc.scalar.activation(
                    lns2[:, sl], sq_ps, AF.Identity, scale=1.0 / (D - 1)
                )
                # per-chunk stats chain so chunk 0 applies while chunk 1 sums
                nc.scalar.activation(
                    lnt[:, sl], lnm[:, sl], AF.Square,
                    scale=math.sqrt(D / (D - 1.0)),
                )
                nc.vector.tensor_sub(lns2[:, sl], lns2[:, sl], lnt[:, sl])
                nc.scalar.sqrt(lns2[:, sl], lns2[:, sl])
                nc.vector.tensor_scalar_add(lns2[:, sl], lns2[:, sl], EPS)
                nc.vector.reciprocal(lnt[:, sl], lns2[:, sl])  # r
                nc.vector.tensor_mul(lnm[:, sl], lnm[:, sl], lnt[:, sl])  # m*r
                nc.scalar.activation(lnr16[:, sl], lnt[:, sl], AF.Identity)
                nc.scalar.activation(lnmr16[:, sl], lnm[:, sl], AF.Identity)
            for c in range(nch):
                sl = slice(512 * c, 512 * (c + 1))
                rb_ps = psS.tile([P, 512], f32, tag="psS", name=f"rbps{c}")
                nc.tensor.matmul(
                    rb_ps, ones_r128, lnr16[0:1, sl], start=True, stop=True
                )
                nc.vector.tensor_copy(rb_t[:, sl], rb_ps)
                mrb_ps = psS.tile([P, 512], f32, tag="psS", name=f"mrbps{c}")
                nc.tensor.matmul(
                    mrb_ps, ones_r128, lnmr16[0:1, sl], start=True, stop=True
                )
                nc.vector.tensor_copy(mrb_t[:, sl], mrb_ps)
                for k in range(ND):
                    t1 = sqp.tile([P, 512], f16, tag="sq", name=f"ap{k}{c}")
                    nc.vector.tensor_mul(t1, Zt[:, k, sl], rb_t[:, sl])
                    nc.vector.tensor_sub(t1, t1, mrb_t[:, sl])
                    out_fn(k, c, t1, g_t, b_t, sl)

        def attention(Sq, Qt, Kt, Vt, ctx_sink):
            """ctx_sink(h, ap[64, Sq]) receives normalized per-head context."""
            nq = Sq // 512
            for h in range(H):
                hb = DH * (h % 2)
                j = h // 2
                ctx_ps = psC.tile([DH + 1, Sq], f32, tag="psC", name=f"ctx{h}")
                for st in range(NS):
                    sc_ps = psA.tile([P, Sq], f32, tag="psA", name=f"sc{h}_{st}")
                    for c in range(nq):
                        sl = slice(512 * c, 512 * (c + 1))
                        nc.tensor.matmul(
                            sc_ps[:, sl],
                            _r(Kt[hb : hb + DH, j, st * P : (st + 1) * P]),
                            _r(Qt[hb : hb + DH, j, sl]),
                            start=True,
                            stop=True,
                        )
                    ex = expool.tile([P, Sq], f16, tag="ex", name=f"ex{h}_{st}")
                    nc.scalar.activation(ex, sc_ps, AF.Exp)
                    for c in range(nq):
                        sl = slice(512 * c, 512 * (c + 1))
                        nc.tensor.matmul(
                            ctx_ps[:, sl],
                            Vt[:, st, (DH + 1) * h : (DH + 1) * (h + 1)],
                            ex[:, sl],
                            start=(st == 0),
                            stop=(st == NS - 1),
                        )
                rs = smp.tile([DH + 1, Sq], f16, tag="rs", name=f"rs{h}")
                with nc.allow_low_precision(reason="softmax recip fp16"):
                    nc.vector.reciprocal(
                        rs[DH : DH + 1, :], ctx_ps[DH : DH + 1, :]
                    )
                ctxn = ctxp.tile([DH, Sq], f16, tag="ctxh", name=f"ctxn{h}")
                for c in range(nq):
                    sl = slice(512 * c, 512 * (c + 1))
                    br_ps = psS.tile([DH, 512], f32, tag="psS", name=f"br{h}{c}")
                    nc.tensor.matmul(
                        br_ps,
                        ones_r64[DH : DH + 1, :],
                        rs[DH : DH + 1, sl],
                        start=True,
                        stop=True,
                    )
                    brc = brp.tile([DH, 512], f32, tag="brc", name=f"brc{h}{c}")
                    nc.vector.tensor_copy(brc, br_ps)
                    nc.vector.tensor_mul(ctxn[:, sl], ctx_ps[0:DH, sl], brc)
                ctx_sink(h, ctxn)

        # ================= BLOCK 0 (full S, self-attention on src) =========
        with tc.tile_pool(name="b0a", bufs=5) as act6, tc.tile_pool(
            name="b0x", bufs=1
        ) as act7, tc.tile_pool(name="b0v", bufs=1) as vp0:
            X0 = act7.tile([P, ND + 1, S], f16, tag="a7", name="X0")
            nc.sync.dma_start(
                out=X0[:, 0:ND, :],
                in_=din["srcT"].ap().rearrange("(j p) s -> p j s", p=P),
            )
            nc.vector.memset(X0[0:1, ND, :], 1.0)

            x_k = [X0[:, k, :] for k in range(ND)]
            V0 = vp0.tile([P, NS, VW], f16, name="V0")

            # block1 q-projection depends only on inputs: emit first to fill
            # the startup bubble while block0 weights stream in.
            q1w = load_w(din["l1_qT"], D, ND)
            bq1 = par["l1_bq"]

            def ev_q1(m, ps):
                nc.vector.tensor_scalar_add(Q1[:, m, :], ps, bq1[:, m : m + 1])

            s1_k0 = [S1T[:, k, :] for k in range(ND)]
            fm_proj(s1_k0, q1w, SH, ev_q1)

            def blk(li, Sq, Qsrc_k, KVsrc_k, kv_ones, resid_k, Vt, CTXa, Za, Zb,
                    ATT, OUTLN_fn):
                """One transformer block in feature-major layout."""
                pre = f"l{li}_"
                # --- q/k (feature-major) ---
                qw = load_w(din[pre + "qT"], D, ND)
                Qt = CTXa["Q"]
                bq = par[pre + "bq"]

                def ev_q(m, ps):
                    nc.vector.tensor_scalar_add(Qt[:, m, 0:Sq], ps, bq[:, m : m + 1])

                fm_proj(Qsrc_k, qw, Sq, ev_q)

                kw = load_w(din[pre + "kT"], D, ND)
                Kt = CTXa["K"]
                bk = par[pre + "bk"]

                def ev_k(m, ps):
                    nc.vector.tensor_scalar_add(Kt[:, m, :], ps, bk[:, m : m + 1])

                fm_proj(KVsrc_k, kw, S, ev_k)

                # --- v (row-major, bias+ones via augmented row) ---
                vw_t = load_w(din[pre + "vTp"], VW, ND)
                vb = wp.tile([1, VW], f16, tag="w", name=f"vb{li}")
                nc.sync.dma_start(out=vb, in_=din[pre + "vTp"].ap()[D : D + 1, :])
                for st in range(NS):
                    ps = psA.tile([P, VW], f32, tag="psA", name=f"vps{li}_{st}")
                    ssl = slice(st * P, (st + 1) * P)
                    for c, (c0, c1) in enumerate(((0, 512), (512, VW))):
                        for ki in range(ND + 1):
                            if ki < ND:
                                lhs = _r(KVsrc_k[ki][:, ssl])
                                rhs = _r(vw_t[ki][:, c0:c1])
                            else:
                                lhs = _r(kv_ones[:, ssl])
                                rhs = _r(vb[:, c0:c1])
                            nc.tensor.matmul(
                                ps[:, c0:c1], lhs, rhs,
                                start=(ki == 0), stop=(ki == ND),
                            )
                    nc.vector.tensor_copy(Vt[:, st, :], ps)

                # --- attention ---
                ow = load_w(din[pre + "oT"], D, ND)
                CTXh = CTXa["CTX"]

                def sink(h, ctxn):
                    hb = DH * (h % 2)
                    j = h // 2
                    nc.sync.dma_start(out=CTXh[hb : hb + DH, j, 0:Sq], in_=ctxn)

                attention(Sq, Qt, Kt, Vt, sink)

                # --- o-proj + bias + residual -> Za ---
                bo = par[pre + "bo"]

                def ev_o(m, ps):
                    for c in range(Sq // 512):
                        sl = slice(512 * c, 512 * (c + 1))
                        t = sqp.tile([P, 512], f16, tag="sq", name=f"oe{m}_{c}")
                        nc.scalar.activation(
                            t, ps[:, sl], AF.Identity, bias=bo[:, m : m + 1]
                        )
                        nc.vector.tensor_add(Za[:, m, sl], t, resid_k[m][:, sl])

                fm_proj([CTXh[:, k, 0:Sq] for k in range(ND)], ow, Sq, ev_o)

                # --- LN (attn) -> ATT ---
                ag, ab = par[pre + "ag"], par[pre + "ab"]

                def out_att(k, c, t1, g_t, b_t, sl):
                    nc.vector.tensor_scalar(
                        ATT[:, k, sl], t1, g_t[:, k : k + 1], b_t[:, k : k + 1],
                        OP.mult, OP.add,
                    )

                layernorm(Za, Sq, ag, ab, out_att)

                # --- ffn w1 + gelu ---
                w1 = load_w(din[pre + "w1T"], D, ND)
                H1 = CTXa["H1"]
                b1 = par[pre + "b1"]

                def ev_w1(m, ps):
                    nc.scalar.activation(
                        H1[:, m, 0:Sq], ps, AF.Gelu, bias=b1[:, m : m + 1]
                    )

                fm_proj([ATT[:, k, 0:Sq] for k in range(ND)], w1, Sq, ev_w1)

                # --- ffn w2 + bias + residual -> Zb, LN -> OUTLN_fn ---
                w2 = load_w(din[pre + "w2T"], D, ND)
                b2 = par[pre + "b2"]

                def ev_w2(m, ps):
                    for c in range(Sq // 512):
                        sl = slice(512 * c, 512 * (c + 1))
                        t = sqp.tile([P, 512], f16, tag="sq", name=f"w2e{m}_{c}")
                        nc.scalar.activation(
                            t, ps[:, sl], AF.Identity, bias=b2[:, m : m + 1]
                        )
                        nc.vector.tensor_add(Zb[:, m, sl], t, ATT[:, m, sl])

                fm_proj([H1[:, k, 0:Sq] for k in range(ND)], w2, Sq, ev_w2)

                fg, fb = par[pre + "fg"], par[pre + "fb"]
                layernorm(Zb, Sq, fg, fb, OUTLN_fn)

            # block0 tensor buffers (rotating in act6)
            Q0 = act6.tile([P, ND, S], f16, tag="a6", name="Q0")
            K0 = act6.tile([P, ND, S], f16, tag="a6", name="K0")
            CTX0 = act6.tile([P, ND, S], f16, tag="a6", name="CTX0")
            Z0a = act6.tile([P, ND, S], f16, tag="a6", name="Z0a")
            ATT0 = act6.tile([P, ND, S], f16, tag="a6", name="ATT0")
            H10 = act6.tile([P, ND, S], f16, tag="a6", name="H10")
            Z0b = act6.tile([P, ND, S], f16, tag="a6", name="Z0b")

            def out_feats(k, c, t1, g_t, b_t, sl):
                nc.vector.tensor_scalar(
                    FEATS[:, k, sl], t1, g_t[:, k : k + 1], b_t[:, k : k + 1],
                    OP.mult, OP.add,
                )

            blk(
                0, S, x_k, x_k, X0[0:1, ND, :], x_k, V0,
                {"Q": Q0, "K": K0, "CTX": CTX0, "H1": H10},
                Z0a, Z0b, ATT0, out_feats,
            )

        # ================= BLOCK 1 (half S on q-side, cross-attention) ======
        with tc.tile_pool(name="b1a", bufs=4) as a6h, tc.tile_pool(
            name="b1b", bufs=1
        ) as a6f, tc.tile_pool(name="b1v", bufs=1) as vp1:
            feats_k = [FEATS[:, k, :] for k in range(ND)]
            s1_k = [S1T[:, k, :] for k in range(ND)]
            K1 = a6f.tile([P, ND, S], f16, tag="af", name="K1")
            CTX1 = a6h.tile([P, ND, SH], f16, tag="ah", name="CTX1")
            Z1a = a6h.tile([P, ND, SH], f16, tag="ah", name="Z1a")
            ATT1 = a6h.tile([P, ND, SH], f16, tag="ah", name="ATT1")
            H11 = a6h.tile([P, ND, SH], f16, tag="ah", name="H11")
            Z1b = a6h.tile([P, ND, SH], f16, tag="ah", name="Z1b")
            V1 = vp1.tile([P, NS, VW], f16, name="V1")

            def out_ctx1(k, c, t1, g_t, b_t, sl):
                nc.vector.tensor_scalar(
                    CTX1p[:, k, sl], t1, g_t[:, k : k + 1], b_t[:, k : k + 1],
                    OP.mult, OP.add,
                )

            # reuse blk via closure over act6-> but buffers differ; inline call:
            # q from s1 (Sq=SH), k/v from feats (full S), residual = s1
            def blk1():
                pre = "l1_"
                kw = load_w(din[pre + "kT"], D, ND)
                bk = par[pre + "bk"]

                def ev_k(m, ps):
                    nc.vector.tensor_scalar_add(K1[:, m, :], ps, bk[:, m : m + 1])

                fm_proj(feats_k, kw, S, ev_k)

                vw_t = load_w(din[pre + "vTp"], VW, ND)
                vb = wp.tile([1, VW], f16, tag="w", name="vb1")
                nc.sync.dma_start(out=vb, in_=din[pre + "vTp"].ap()[D : D + 1, :])
                for st in range(NS):
                    ps = psA.tile([P, VW], f32, tag="psA", name=f"v1ps{st}")
                    ssl = slice(st * P, (st + 1) * P)
                    for c0, c1 in ((0, 512), (512, VW)):
                        for ki in range(ND + 1):
                            if ki < ND:
                                lhs = _r(feats_k[ki][:, ssl])
                                rhs = _r(vw_t[ki][:, c0:c1])
                            else:
                                lhs = _r(FEATS[0:1, ND, ssl])
                                rhs = _r(vb[:, c0:c1])
                            nc.tensor.matmul(
                                ps[:, c0:c1], lhs, rhs,
                                start=(ki == 0), stop=(ki == ND),
                            )
                    nc.vector.tensor_copy(V1[:, st, :], ps)

                ow = load_w(din[pre + "oT"], D, ND)

                # head-pair attention: two heads share one [128,1024] psum
                for jp in range(ND):
                    ctx_ps = psC.tile(
                        [DH + 1, 2 * SH], f32, tag="psC", name=f"c1ps{jp}"
                    )
                    for st in range(NS):
                        sc_ps = psA.tile(
                            [P, 2 * SH], f32, tag="psA", name=f"s1ps{jp}_{st}"
                        )
                        for half in (0, 1):
                            hb = DH * half
                            sl = slice(SH * half, SH * (half + 1))
                            nc.tensor.matmul(
                                sc_ps[:, sl],
                                K1[hb : hb + DH, jp, st * P : (st + 1) * P],
                                Q1[hb : hb + DH, jp, :],
                                start=True,
                                stop=True,
                            )
                        ex = expool.tile(
                            [P, 2 * SH], f16, tag="ex", name=f"e1x{jp}_{st}"
                        )
                        nc.scalar.activation(ex, sc_ps, AF.Exp)
                        for half in (0, 1):
                            h = 2 * jp + half
                            sl = slice(SH * half, SH * (half + 1))
                            nc.tensor.matmul(
                                ctx_ps[:, sl],
                                V1[:, st, (DH + 1) * h : (DH + 1) * (h + 1)],
                                ex[:, sl],
                                start=(st == 0),
                                stop=(st == NS - 1),
                            )
                    rs = smp.tile([DH + 1, 2 * SH], f16, tag="rs", name=f"r1s{jp}")
                    with nc.allow_low_precision(reason="softmax recip fp16"):
                        nc.vector.reciprocal(
                            rs[DH : DH + 1, :], ctx_ps[DH : DH + 1, :]
                        )
                    for half in (0, 1):
                        hb = DH * half
                        sl = slice(SH * half, SH * (half + 1))
                        br_ps = psS.tile(
                            [DH, 512], f32, tag="psS", name=f"b1r{jp}{half}"
                        )
                        nc.tensor.matmul(
                            br_ps,
                            ones_r64[DH : DH + 1, :],
                            rs[DH : DH + 1, sl],
                            start=True,
                            stop=True,
                        )
                        brc = brp.tile([DH, 512], f32, tag="brc", name=f"b1c{jp}{half}")
                        nc.vector.tensor_copy(brc, br_ps)
                        ctxn = ctxp.tile([DH, SH], f16, tag="ctxh", name=f"c1n{jp}{half}")
                        nc.vector.tensor_mul(ctxn, ctx_ps[0:DH, sl], brc)
                        nc.sync.dma_start(
                            out=CTX1[hb : hb + DH, jp, :], in_=ctxn
                        )

                bo = par[pre + "bo"]

                def ev_o(m, ps):
                    t = sqp.tile([P, 512], f16, tag="sq", name=f"o1e{m}")
                    nc.scalar.activation(
                        t, ps, AF.Identity, bias=bo[:, m : m + 1]
                    )
                    nc.vector.tensor_add(Z1a[:, m, :], t, S1T[:, m, :])

                fm_proj([CTX1[:, k, :] for k in range(ND)], ow, SH, ev_o)

                ag, ab = par[pre + "ag"], par[pre + "ab"]

                def out_att(k, c, t1, g_t, b_t, sl):
                    nc.vector.tensor_scalar(
                        ATT1[:, k, sl], t1, g_t[:, k : k + 1], b_t[:, k : k + 1],
                        OP.mult, OP.add,
                    )

                layernorm(Z1a, SH, ag, ab, out_att)

                w1 = load_w(din[pre + "w1T"], D, ND)
                b1 = par[pre + "b1"]

                def ev_w1(m, ps):
                    nc.scalar.activation(
                        H11[:, m, :], ps, AF.Gelu, bias=b1[:, m : m + 1]
                    )

                fm_proj([ATT1[:, k, :] for k in range(ND)], w1, SH, ev_w1)

                w2 = load_w(din[pre + "w2T"], D, ND)
                b2 = par[pre + "b2"]

                def ev_w2(m, ps):
                    t = sqp.tile([P, 512], f16, tag="sq", name=f"w21e{m}")
                    nc.scalar.activation(
                        t, ps, AF.Identity, bias=b2[:, m : m + 1]
                    )
                    nc.vector.tensor_add(Z1b[:, m, :], t, ATT1[:, m, :])

                fm_proj([H11[:, k, :] for k in range(ND)], w2, SH, ev_w2)

                fg, fb = par[pre + "fg"], par[pre + "fb"]
                layernorm(Z1b, SH, fg, fb, out_ctx1)

            blk1()

        # ================= POOL + FINAL =====================================
        with tc.tile_pool(name="late", bufs=2) as lp, tc.tile_pool(
            name="wbig", bufs=14
        ) as wb:
            # weight prefetch first: overlaps the whole pool chain
            pw_t = []
            for k in range(NS):
                t = wb.tile([P, S], f16, tag="wb", name=f"pw{k}")
                nc.sync.dma_start(out=t, in_=din["pwT"].ap()[k * P : (k + 1) * P, :])
                pw_t.append(t)
            pwb = wb.tile([1, S], f16, tag="wb", name="pwb")
            nc.sync.dma_start(out=pwb, in_=din["pwT"].ap()[S : S + 1, :])
            fin_t = []
            for k in range(2 * ND):
                t = wb.tile([P, D], f16, tag="wb", name=f"fin{k}")
                nc.sync.dma_start(
                    out=t, in_=din["finT"].ap()[k * P : (k + 1) * P, :]
                )
                fin_t.append(t)

            # 1) transpose CTX1p [768, 512] -> row-major [512, 768]
            C1RM = lp.tile([P, SH // P, D], f16, tag="lt", name="C1RM")
            for st in range(SH // P):
                tp = psA.tile([P, D], f16, tag="psA", name=f"t1ps{st}")
                for j in range(ND):
                    nc.tensor.transpose(
                        tp[:, j * P : (j + 1) * P],
                        CTX1p[:, j, st * P : (st + 1) * P],
                        ident,
                    )
                nc.vector.tensor_copy(C1RM[:, st, :], tp)
                nc.sync.dma_start(
                    out=scr1.ap().rearrange("(s d) -> s d", d=D)[
                        st * P : (st + 1) * P, :
                    ],
                    in_=C1RM[:, st, :],
                )
            # 2) read back as M_view rows [384, 1024], transpose to [1024, 384]
            MV = lp.tile([P, 3, S], f16, tag="lt", name="MV")
            v2 = scr1.ap().rearrange("(r c) -> r c", c=S)
            for rt in range(3):
                nc.sync.dma_start(out=MV[:, rt, :], in_=v2[rt * P : (rt + 1) * P, :])
            MVT = lp.tile([P, NS + 1, 3 * P], f16, tag="lt", name="MVT")
            nc.vector.memset(MVT[0:1, NS, :], 1.0)
            for ct in range(NS):
                tp = psS.tile([P, 3 * P], f16, tag="psS", name=f"t2ps{ct}")
                for rt in range(3):
                    nc.tensor.transpose(
                        tp[:, rt * P : (rt + 1) * P],
                        MV[:, rt, ct * P : (ct + 1) * P],
                        ident,
                    )
                nc.vector.tensor_copy(MVT[:, ct, :], tp)
            # 3) pool matmul: out_rm [384, 1024] = M_view @ pw.T + pb
            PRM = lp.tile([P, 3, S], f16, tag="lt", name="PRM")
            for rt in range(3):
                ps = psA.tile([P, S], f32, tag="psA", name=f"plps{rt}")
                for c in range(2):
                    sl = slice(512 * c, 512 * (c + 1))
                    for ki in range(NS + 1):
                        if ki < NS:
                            lhs = _r(MVT[:, ki, rt * P : (rt + 1) * P])
                            rhs = _r(pw_t[ki][:, sl])
                        else:
                            lhs = _r(MVT[0:1, NS, rt * P : (rt + 1) * P])
                            rhs = _r(pwb[:, sl])
                        nc.tensor.matmul(
                            ps[:, sl], lhs, rhs, start=(ki == 0), stop=(ki == NS)
                        )
                nc.vector.tensor_copy(PRM[:, rt, :], ps)
                nc.sync.dma_start(
                    out=scr2.ap().rearrange("(r c) -> r c", c=S)[
                        rt * P : (rt + 1) * P, :
                    ],
                    in_=PRM[:, rt, :],
                )
            # 4) read back as app row-major [512, 768], transpose -> APPT'
            APPRM = lp.tile([P, SH // P, D], f16, tag="lt", name="APPRM")
            v3 = scr2.ap().rearrange("(s d) -> s d", d=D)
            for st in range(SH // P):
                nc.sync.dma_start(
                    out=APPRM[:, st, :], in_=v3[st * P : (st + 1) * P, :]
                )
            APPT = lp.tile([P, ND, SH], f16, tag="lt", name="APPT")
            for j in range(ND):
                tp = psS.tile([P, SH], f16, tag="psS", name=f"t3ps{j}")
                for st in range(SH // P):
                    nc.tensor.transpose(
                        tp[:, st * P : (st + 1) * P],
                        APPRM[:, st, j * P : (j + 1) * P],
                        ident,
                    )
                nc.vector.tensor_copy(APPT[:, j, :], tp)
            # 5) final: out' = finT.T @ [feats_half ; app]
            OUTT = lp.tile([P, ND, SH], f32, tag="lt", name="OUTT")
            for m in range(ND):
                ps = psS.tile([P, SH], f32, tag="psS", name=f"fps{m}")
                for ki in range(2 * ND):
                    rhs = (
                        FEATS[:, ki, 0:SH]
                        if ki < ND
                        else APPT[:, ki - ND, :]
                    )
                    nc.tensor.matmul(
                        ps,
                        _r(fin_t[ki][:, m * P : (m + 1) * P]),
                        _r(rhs),
                        start=(ki == 0),
                        stop=(ki == 2 * ND - 1),
                    )
                nc.scalar.activation(
                    OUTT[:, m, :], ps, AF.Identity, bias=finb[:, m : m + 1]
                )
            nc.sync.dma_start(
                out=outT.ap().rearrange("(j p) s -> p j s", p=P), in_=OUTT
            )


def _prep_inputs(inputs):
    e = np.ascontiguousarray(np.asarray(inputs["e"], dtype=np.float32))
    f = np.ascontiguousarray(np.asarray(inputs["f"], dtype=np.float32))
    wq = np.asarray(inputs["wq"], np.float32)
    wk = np.asarray(inputs["wk"], np.float32)
    wv = np.asarray(inputs["wv"], np.float32)
    wo = np.asarray(inputs["wo"], np.float32)
    bq = np.asarray(inputs["bq"], np.float32)
    bk = np.asarray(inputs["bk"], np.float32)
    bv = np.asarray(inputs["bv"], np.float32)
    bo = np.asarray(inputs["bo"], np.float32)
    ag = np.asarray(inputs["attn_ln_g"], np.float32)
    ab = np.asarray(inputs["attn_ln_b"], np.float32)
    w1 = np.asarray(inputs["ffn_w1"], np.float32)
    b1 = np.asarray(inputs["ffn_b1"], np.float32)
    w2 = np.asarray(inputs["ffn_w2"], np.float32)
    b2 = np.asarray(inputs["ffn_b2"], np.float32)
    fg = np.asarray(inputs["ffn_ln_g"], np.float32)
    fb = np.asarray(inputs["ffn_ln_b"], np.float32)
    pw = np.asarray(inputs["pool_w"], np.float32)
    pb = np.asarray(inputs["pool_b"], np.float32)
    fw = np.asarray(inputs["final_w"], np.float32)
    fnb = np.asarray(inputs["final_b"], np.float32)

    def vec6(v):
        return np.ascontiguousarray(v.reshape(ND, P).T)

    scale = 1.0 / math.sqrt(DH)
    in_maps = []
    for c in range(8):
        ti, b, h = c // 4, (c // 2) % 2, c % 2
        src = e if ti == 0 else f
        s1 = f if ti == 0 else e
        own = slice(SH * h, SH * (h + 1))
        oth = slice(SH * (1 - h), SH * (2 - h))
        src_b = src[:, b, :]
        src_perm = np.concatenate([src_b[own], src_b[oth]], axis=0)
        m = {
            "srcT": np.ascontiguousarray(src_perm.T).astype(np.float16),
            "s1T": np.ascontiguousarray(s1[own, b, :].T).astype(np.float16),
            "pwT": np.ascontiguousarray(
                np.concatenate([pw[ti].T, pb[ti][None, :]], axis=0)
            ).astype(np.float16),
            "finT": np.ascontiguousarray(fw[ti].T).astype(np.float16),
            "finb": vec6(fnb[ti]),
        }
        for li in (0, 1):
            vTp = np.zeros((D + 1, VW), np.float16)
            wvT = wv[ti, li].T
            for hh in range(H):
                vTp[0:D, (DH + 1) * hh : (DH + 1) * hh + DH] = wvT[
                    :, DH * hh : DH * (hh + 1)
                ]
                vTp[D, (DH + 1) * hh : (DH + 1) * hh + DH] = bv[
                    ti, li, DH * hh : DH * (hh + 1)
                ]
                vTp[D, (DH + 1) * hh + DH] = 1.0
            m.update(
                {
                    f"l{li}_qT": np.ascontiguousarray(wq[ti, li].T * scale).astype(np.float16),
                    f"l{li}_kT": np.ascontiguousarray(wk[ti, li].T).astype(np.float16),
                    f"l{li}_vTp": vTp,
                    f"l{li}_oT": np.ascontiguousarray(wo[ti, li].T).astype(np.float16),
                    f"l{li}_w1T": np.ascontiguousarray(w1[ti, li].T).astype(np.float16),
                    f"l{li}_w2T": np.ascontiguousarray(w2[ti, li].T).astype(np.float16),
                    f"l{li}_bq": vec6(bq[ti, li] * scale),
                    f"l{li}_bk": vec6(bk[ti, li]),
                    f"l{li}_bo": vec6(bo[ti, li]),
                    f"l{li}_b1": vec6(b1[ti, li]),
                    f"l{li}_b2": vec6(b2[ti, li]),
                    f"l{li}_ag": vec6(ag[ti, li]),
                    f"l{li}_ab": vec6(ab[ti, li]),
                    f"l{li}_fg": vec6(fg[ti, li]),
                    f"l{li}_fb": vec6(fb[ti, li]),
                }
            )
        in_maps.append(m)
    return in_maps


def get_program():
    if "nc" not in _BUILT:
        _BUILT["nc"] = _build_program()
    return _BUILT["nc"]


def kernel(**inputs):
    from concourse.bass_utils import run_bass_kernel_spmd

    nc = get_program()
    in_maps = _prep_inputs(inputs)
    res = run_bass_kernel_spmd(nc, in_maps, core_ids=list(range(8)))
    c_e_f = np.empty((S, B, D), np.float32)
    c_f_e = np.empty((S, B, D), np.float32)
    for c in range(8):
        ti, b, h = c // 4, (c // 2) % 2, c % 2
        dst = c_e_f if ti == 0 else c_f_e
        dst[SH * h : SH * (h + 1), b, :] = res.results[c]["outT"].T
    return c_e_f, c_f_e



# revision 7
# speedup vs baseline: 1.0396x; 1.0396x over previous
# Trainium2 Bass kernel for nn_Cross_Transformer (dense_transformer).
#
# Sharding: 8 cores = 2 towers x 2 batches x 2 sequence-halves.
# Each core computes block0 (self-attention) in full (its inputs are permuted
# so its own half leads, keeping the program SPMD-uniform), then its half of
# block1 (cross-attention), pool, and final projection. No collectives.
#
# Layout: activations are feature-major [D on partitions, S on free] so every
# projection is lhsT=W^T tiles (stationary) x X' (moving). Matmuls run as
# float32r (1 cyc/row at N>=256). Attention probabilities/V run in fp16.
# LayerNorm over D (= partitions) uses ones-column matmuls for sums and a
# K=1 ones-row matmul to broadcast per-column stats across partitions.

import math

import numpy as np

S = 1024
B = 2
D = 768
H = 12
DH = 64
EPS = 1e-6
SH = S // 2  # 512, per-core block1 rows
P = 128
ND = D // P  # 6 d-tiles
NS = S // P  # 8 s-tiles
VW = H * (DH + 1)  # 780: v row-major padded with a ones column per head

F32 = None  # filled lazily (mybir.dt.float32)
_BUILT = {}


def _dt():
    from concourse import mybir

    return mybir.dt


def _r(ap):
    """View an fp32 AP as float32r for full-rate PE matmuls; fp16 passes through."""
    dt = _dt()
    return ap.bitcast(dt.float32r) if ap.dtype == dt.float32 else ap


def _build_program():
    import concourse.bacc as bacc
    import concourse.tile as tile
    from concourse import mybir
    from concourse.masks import make_identity

    dt = mybir.dt
    f32 = dt.float32
    f16 = dt.float16
    AF = mybir.ActivationFunctionType
    OP = mybir.AluOpType

    nc = bacc.Bacc("TRN2", target_bir_lowering=False, debug=False, num_devices=8)

    # ---- DRAM I/O ----
    din = {}

    def dram_in(name, shape, dty=None):
        din[name] = nc.dram_tensor(
            name, list(shape), dty or f16, kind="ExternalInput"
        )
        return din[name]

    dram_in("srcT", (D, S))
    dram_in("s1T", (D, SH))
    for li in (0, 1):
        dram_in(f"l{li}_qT", (D, D))
        dram_in(f"l{li}_kT", (D, D))
        dram_in(f"l{li}_vTp", (D + 1, VW))
        dram_in(f"l{li}_oT", (D, D))
        dram_in(f"l{li}_w1T", (D, D))
        dram_in(f"l{li}_w2T", (D, D))
        for bn in ("bq", "bk", "bo", "b1", "b2", "ag", "ab", "fg", "fb"):
            dram_in(f"l{li}_{bn}", (P, ND), f32)
    dram_in("pwT", (S + 1, S))
    dram_in("pbT", (P, NS), f32)
    dram_in("finT", (2 * D, D))
    dram_in("finb", (P, ND), f32)

    outT = nc.dram_tensor("outT", [D, SH], f32, kind="ExternalOutput")

    with tile.TileContext(nc) as tc:
        _emit(nc, tc, tile, dt, AF, OP, din, outT, make_identity)

    nc.compile()
    return nc


def _emit(nc, tc, tile, dt, AF, OP, din, outT, make_identity):
    f32 = dt.float32
    f16 = dt.float16
    import contextlib

    es = contextlib.ExitStack()
    with es:
        persist = es.enter_context(tc.tile_pool(name="persist", bufs=1))
        wp = es.enter_context(tc.tile_pool(name="wp", bufs=13))
        psA = es.enter_context(tc.tile_pool(name="psA", bufs=2, space="PSUM"))
        psC = es.enter_context(tc.tile_pool(name="psC", bufs=1, space="PSUM"))
        psS = es.enter_context(tc.tile_pool(name="psS", bufs=2, space="PSUM"))
        expool = es.enter_context(tc.tile_pool(name="expool", bufs=6))
        ctxp = es.enter_context(tc.tile_pool(name="ctxp", bufs=3))
        sqp = es.enter_context(tc.tile_pool(name="sqp", bufs=4))
        brp = es.enter_context(tc.tile_pool(name="brp", bufs=4))
        smp = es.enter_context(tc.tile_pool(name="smp", bufs=4))

        # --- constants ---
        ident = persist.tile([P, P], f16, name="ident")
        make_identity(nc, ident)
        ones_col = persist.tile([P, 1], f16, name="ones_col")
        nc.vector.memset(ones_col, 1.0)
        ones_r64 = persist.tile([DH + 1, DH], f16, name="ones_r64")
        nc.vector.memset(ones_r64[DH : DH + 1, :], 1.0)
        ones_r128 = persist.tile([1, P], f16, name="ones_r128")
        nc.vector.memset(ones_r128, 1.0)

        # --- small params (biases, LN) ---
        par = {}
        for li in (0, 1):
            for bn in ("bq", "bk", "bo", "b1", "b2", "ag", "ab", "fg", "fb"):
                t = persist.tile([P, ND], f32, name=f"p_l{li}_{bn}")
                nc.gpsimd.dma_start(out=t, in_=din[f"l{li}_{bn}"].ap())
                par[f"l{li}_{bn}"] = t
        finb = persist.tile([P, ND], f32, name="p_finb")
        nc.gpsimd.dma_start(out=finb, in_=din["finb"].ap())

        # persistent activations
        FEATS = persist.tile([P, ND + 1, S], f16, name="FEATS")
        nc.vector.memset(FEATS[0:1, ND, :], 1.0)
        S1T = persist.tile([P, ND, SH], f16, name="S1T")
        nc.sync.dma_start(
            out=S1T, in_=din["s1T"].ap().rearrange("(j p) s -> p j s", p=P)
        )
        # M_viewT [c-tile partitions, c-tile idx, r]: M_view[r, 128t+p] with
        # r = 3k+rho -> ctx1_ln[4k + u//6, d=128*(u%6)+p], u = 8*rho + t.
        # Written directly (strided) by block1's final LN.
        MVT = persist.tile([P, NS, 3 * P], f16, name="MVT")
        Q1 = persist.tile([P, ND, SH], f16, name="Q1")
        rb_t = persist.tile([P, S], f16, name="rb_t")
        mrb_t = persist.tile([P, S], f16, name="mrb_t")
        # LN small stats rows
        lnm = persist.tile([1, S], f32, name="lnm")
        lns2 = persist.tile([1, S], f32, name="lns2")
        lnt = persist.tile([1, S], f32, name="lnt")
        lnr16 = persist.tile([1, S], f16, name="lnr16")
        lnmr16 = persist.tile([1, S], f16, name="lnmr16")

        def load_w(dram_h, width, nk, tagsuf=""):
            """DMA weight k-tiles [P, width] (+ optional trailing [1, width])."""
            ap = dram_h.ap()
            tiles = []
            for t in range(nk):
                wt = wp.tile([P, width], f16, tag="w", name=f"w_{dram_h.name}_{t}")
                nc.sync.dma_start(out=wt, in_=ap[t * P : (t + 1) * P, :])
                tiles.append(wt)
            return tiles

        def fm_proj(x_ktiles, w_tiles, Sx, evac, extra_k=None):
            """Feature-major projection: out[m] = sum_k w[k][:,m].T @ x[k].
            x_ktiles: list of APs [kp, Sx]; w_tiles: list of APs [kp, D].
            evac(m, ps): consume psum [P, Sx]."""
            nch = Sx // 512
            ks = list(zip(x_ktiles, w_tiles))
            if extra_k is not None:
                ks.append(extra_k)
            for m in range(ND):
                ps = psA.tile([P, Sx], f32, tag="psA", name=f"ps_m{m}")
                for c in range(nch):
                    sl = slice(512 * c, 512 * (c + 1))
                    for ki, (xk, wk) in enumerate(ks):
                        nc.tensor.matmul(
                            ps[:, sl],
                            _r(wk[:, m * P : (m + 1) * P]),
                            _r(xk[:, sl]),
                            start=(ki == 0),
                            stop=(ki == len(ks) - 1),
                        )
                evac(m, ps)

        def layernorm(Zt, Sx, g_t, b_t, out_fn):
            """LN over partitions(d) of Zt [P, ND, Sx] (Bessel std + eps).
            out_fn(k, c, src_ap, sl): writes result tile."""
            nch = Sx // 512
            for c in range(nch):
                sl = slice(512 * c, 512 * (c + 1))
                sum_ps = psS.tile([1, 512], f32, tag="psS", name=f"lnsum{c}")
                for k in range(ND):
                    nc.tensor.matmul(
                        sum_ps,
                        _r(ones_col),
                        _r(Zt[:, k, sl]),
                        start=(k == 0),
                        stop=(k == ND - 1),
                    )
                nc.scalar.activation(
                    lnm[:, sl], sum_ps, AF.Identity, scale=1.0 / D
                )
                sq_ps = psS.tile([1, 512], f32, tag="psS", name=f"lnsq{c}")
                for k in range(ND):
                    sq = sqp.tile([P, 512], f16, tag="sq", name=f"sq{k}{c}")
                    nc.vector.tensor_mul(sq, Zt[:, k, sl], Zt[:, k, sl])
                    nc.tensor.matmul(
                        sq_ps,
                        _r(ones_col),
                        _r(sq),
                        start=(k == 0),
                        stop=(k == ND - 1),
                    )
                nc.scalar.activation(
                    lns2[:, sl], sq_ps, AF.Identity, scale=1.0 / (D - 1)
                )
                # per-chunk stats chain so chunk 0 applies while chunk 1 sums
                nc.scalar.activation(
                    lnt[:, sl], lnm[:, sl], AF.Square,
                    scale=math.sqrt(D / (D - 1.0)),
                )
                nc.vector.tensor_sub(lns2[:, sl], lns2[:, sl], lnt[:, sl])
                nc.scalar.sqrt(lns2[:, sl], lns2[:, sl])
                nc.vector.tensor_scalar_add(lns2[:, sl], lns2[:, sl], EPS)
                nc.vector.reciprocal(lnt[:, sl], lns2[:, sl])  # r
                nc.vector.tensor_mul(lnm[:, sl], lnm[:, sl], lnt[:, sl])  # m*r
                nc.scalar.activation(lnr16[:, sl], lnt[:, sl], AF.Identity)
                nc.scalar.activation(lnmr16[:, sl], lnm[:, sl], AF.Identity)
            for c in range(nch):
                sl = slice(512 * c, 512 * (c + 1))
                rb_ps = psS.tile([P, 512], f32, tag="psS", name=f"rbps{c}")
                nc.tensor.matmul(
                    rb_ps, ones_r128, lnr16[0:1, sl], start=True, stop=True
                )
                nc.vector.tensor_copy(rb_t[:, sl], rb_ps)
                mrb_ps = psS.tile([P, 512], f32, tag="psS", name=f"mrbps{c}")
                nc.tensor.matmul(
                    mrb_ps, ones_r128, lnmr16[0:1, sl], start=True, stop=True
                )
                nc.vector.tensor_copy(mrb_t[:, sl], mrb_ps)
                for k in range(ND):
                    t1 = sqp.tile([P, 512], f16, tag="sq", name=f"ap{k}{c}")
                    nc.vector.tensor_mul(t1, Zt[:, k, sl], rb_t[:, sl])
                    nc.vector.tensor_sub(t1, t1, mrb_t[:, sl])
                    out_fn(k, c, t1, g_t, b_t, sl)

        def attention(Sq, Qt, Kt, Vt, ctx_sink):
            """ctx_sink(h, ap[64, Sq]) receives normalized per-head context."""
            nq = Sq // 512
            for h in range(H):
                hb = DH * (h % 2)
                j = h // 2
                ctx_ps = psC.tile([DH + 1, Sq], f32, tag="psC", name=f"ctx{h}")
                for st in range(NS):
                    sc_ps = psA.tile([P, Sq], f32, tag="psA", name=f"sc{h}_{st}")
                    for c in range(nq):
                        sl = slice(512 * c, 512 * (c + 1))
                        nc.tensor.matmul(
                            sc_ps[:, sl],
                            _r(Kt[hb : hb + DH, j, st * P : (st + 1) * P]),
                            _r(Qt[hb : hb + DH, j, sl]),
                            start=True,
                            stop=True,
                        )
                    ex = expool.tile([P, Sq], f16, tag="ex", name=f"ex{h}_{st}")
                    nc.scalar.activation(ex, sc_ps, AF.Exp)
                    for c in range(nq):
                        sl = slice(512 * c, 512 * (c + 1))
                        nc.tensor.matmul(
                            ctx_ps[:, sl],
                            Vt[:, st, (DH + 1) * h : (DH + 1) * (h + 1)],
                            ex[:, sl],
                            start=(st == 0),
                            stop=(st == NS - 1),
                        )
                rs = smp.tile([DH + 1, Sq], f16, tag="rs", name=f"rs{h}")
                with nc.allow_low_precision(reason="softmax recip fp16"):
                    nc.vector.reciprocal(
                        rs[DH : DH + 1, :], ctx_ps[DH : DH + 1, :]
                    )
                ctxn = ctxp.tile([DH, Sq], f16, tag="ctxh", name=f"ctxn{h}")
                for c in range(nq):
                    sl = slice(512 * c, 512 * (c + 1))
                    br_ps = psS.tile([DH, 512], f32, tag="psS", name=f"br{h}{c}")
                    nc.tensor.matmul(
                        br_ps,
                        ones_r64[DH : DH + 1, :],
                        rs[DH : DH + 1, sl],
                        start=True,
                        stop=True,
                    )
                    brc = brp.tile([DH, 512], f32, tag="brc", name=f"brc{h}{c}")
                    nc.vector.tensor_copy(brc, br_ps)
                    nc.vector.tensor_mul(ctxn[:, sl], ctx_ps[0:DH, sl], brc)
                ctx_sink(h, ctxn)

        # ================= BLOCK 0 (full S, self-attention on src) =========
        with tc.tile_pool(name="b0a", bufs=5) as act6, tc.tile_pool(
            name="b0x", bufs=1
        ) as act7, tc.tile_pool(name="b0v", bufs=1) as vp0:
            X0 = act7.tile([P, ND + 1, S], f16, tag="a7", name="X0")
            nc.sync.dma_start(
                out=X0[:, 0:ND, :],
                in_=din["srcT"].ap().rearrange("(j p) s -> p j s", p=P),
            )
            nc.vector.memset(X0[0:1, ND, :], 1.0)

            x_k = [X0[:, k, :] for k in range(ND)]
            V0 = vp0.tile([P, NS, VW], f16, name="V0")

            # block1 q-projection depends only on inputs: emit first to fill
            # the startup bubble while block0 weights stream in.
            q1w = load_w(din["l1_qT"], D, ND)
            bq1 = par["l1_bq"]

            def ev_q1(m, ps):
                nc.vector.tensor_scalar_add(Q1[:, m, :], ps, bq1[:, m : m + 1])

            s1_k0 = [S1T[:, k, :] for k in range(ND)]
            fm_proj(s1_k0, q1w, SH, ev_q1)

            def blk(li, Sq, Qsrc_k, KVsrc_k, kv_ones, resid_k, Vt, CTXa, Za, Zb,
                    ATT, OUTLN_fn):
                """One transformer block in feature-major layout."""
                pre = f"l{li}_"
                # --- q/k (feature-major) ---
                qw = load_w(din[pre + "qT"], D, ND)
                Qt = CTXa["Q"]
                bq = par[pre + "bq"]

                def ev_q(m, ps):
                    nc.vector.tensor_scalar_add(Qt[:, m, 0:Sq], ps, bq[:, m : m + 1])

                fm_proj(Qsrc_k, qw, Sq, ev_q)

                kw = load_w(din[pre + "kT"], D, ND)
                Kt = CTXa["K"]
                bk = par[pre + "bk"]

                def ev_k(m, ps):
                    nc.vector.tensor_scalar_add(Kt[:, m, :], ps, bk[:, m : m + 1])

                fm_proj(KVsrc_k, kw, S, ev_k)

                # --- v (row-major, bias+ones via augmented row) ---
                vw_t = load_w(din[pre + "vTp"], VW, ND)
                vb = wp.tile([1, VW], f16, tag="w", name=f"vb{li}")
                nc.sync.dma_start(out=vb, in_=din[pre + "vTp"].ap()[D : D + 1, :])
                for st in range(NS):
                    ps = psA.tile([P, VW], f32, tag="psA", name=f"vps{li}_{st}")
                    ssl = slice(st * P, (st + 1) * P)
                    for c, (c0, c1) in enumerate(((0, 512), (512, VW))):
                        for ki in range(ND + 1):
                            if ki < ND:
                                lhs = _r(KVsrc_k[ki][:, ssl])
                                rhs = _r(vw_t[ki][:, c0:c1])
                            else:
                                lhs = _r(kv_ones[:, ssl])
                                rhs = _r(vb[:, c0:c1])
                            nc.tensor.matmul(
                                ps[:, c0:c1], lhs, rhs,
                                start=(ki == 0), stop=(ki == ND),
                            )
                    nc.vector.tensor_copy(Vt[:, st, :], ps)

                # --- attention ---
                ow = load_w(din[pre + "oT"], D, ND)
                CTXh = CTXa["CTX"]

                def sink(h, ctxn):
                    hb = DH * (h % 2)
                    j = h // 2
                    nc.sync.dma_start(out=CTXh[hb : hb + DH, j, 0:Sq], in_=ctxn)

                attention(Sq, Qt, Kt, Vt, sink)

                # --- o-proj + bias + residual -> Za ---
                bo = par[pre + "bo"]

                def ev_o(m, ps):
                    for c in range(Sq // 512):
                        sl = slice(512 * c, 512 * (c + 1))
                        t = sqp.tile([P, 512], f16, tag="sq", name=f"oe{m}_{c}")
                        nc.scalar.activation(
                            t, ps[:, sl], AF.Identity, bias=bo[:, m : m + 1]
                        )
                        nc.vector.tensor_add(Za[:, m, sl], t, resid_k[m][:, sl])

                fm_proj([CTXh[:, k, 0:Sq] for k in range(ND)], ow, Sq, ev_o)

                # --- LN (attn) -> ATT ---
                ag, ab = par[pre + "ag"], par[pre + "ab"]

                def out_att(k, c, t1, g_t, b_t, sl):
                    nc.vector.tensor_scalar(
                        ATT[:, k, sl], t1, g_t[:, k : k + 1], b_t[:, k : k + 1],
                        OP.mult, OP.add,
                    )

                layernorm(Za, Sq, ag, ab, out_att)

                # --- ffn w1 + gelu ---
                w1 = load_w(din[pre + "w1T"], D, ND)
                H1 = CTXa["H1"]
                b1 = par[pre + "b1"]

                def ev_w1(m, ps):
                    nc.scalar.activation(
                        H1[:, m, 0:Sq], ps, AF.Gelu, bias=b1[:, m : m + 1]
                    )

                fm_proj([ATT[:, k, 0:Sq] for k in range(ND)], w1, Sq, ev_w1)

                # --- ffn w2 + bias + residual -> Zb, LN -> OUTLN_fn ---
                w2 = load_w(din[pre + "w2T"], D, ND)
                b2 = par[pre + "b2"]

                def ev_w2(m, ps):
                    for c in range(Sq // 512):
                        sl = slice(512 * c, 512 * (c + 1))
                        t = sqp.tile([P, 512], f16, tag="sq", name=f"w2e{m}_{c}")
                        nc.scalar.activation(
                            t, ps[:, sl], AF.Identity, bias=b2[:, m : m + 1]
                        )
                        nc.vector.tensor_add(Zb[:, m, sl], t, ATT[:, m, sl])

                fm_proj([H1[:, k, 0:Sq] for k in range(ND)], w2, Sq, ev_w2)

                fg, fb = par[pre + "fg"], par[pre + "fb"]
                layernorm(Zb, Sq, fg, fb, OUTLN_fn)

            # block0 tensor buffers (rotating in act6)
            Q0 = act6.tile([P, ND, S], f16, tag="a6", name="Q0")
            K0 = act6.tile([P, ND, S], f16, tag="a6", name="K0")
            CTX0 = act6.tile([P, ND, S], f16, tag="a6", name="CTX0")
            Z0a = act6.tile([P, ND, S], f16, tag="a6", name="Z0a")
            ATT0 = act6.tile([P, ND, S], f16, tag="a6", name="ATT0")
            H10 = act6.tile([P, ND, S], f16, tag="a6", name="H10")
            Z0b = act6.tile([P, ND, S], f16, tag="a6", name="Z0b")

            def out_feats(k, c, t1, g_t, b_t, sl):
                nc.vector.tensor_scalar(
                    FEATS[:, k, sl], t1, g_t[:, k : k + 1], b_t[:, k : k + 1],
                    OP.mult, OP.add,
                )

            blk(
                0, S, x_k, x_k, X0[0:1, ND, :], x_k, V0,
                {"Q": Q0, "K": K0, "CTX": CTX0, "H1": H10},
                Z0a, Z0b, ATT0, out_feats,
            )

        # ================= BLOCK 1 (half S on q-side, cross-attention) ======
        with tc.tile_pool(name="b1a", bufs=4) as a6h, tc.tile_pool(
            name="b1b", bufs=1
        ) as a6f, tc.tile_pool(name="b1v", bufs=1) as vp1:
            feats_k = [FEATS[:, k, :] for k in range(ND)]
            s1_k = [S1T[:, k, :] for k in range(ND)]
            K1 = a6f.tile([P, ND, S], f16, tag="af", name="K1")
            CTX1 = a6h.tile([P, ND, SH], f16, tag="ah", name="CTX1")
            Z1a = a6h.tile([P, ND, SH], f16, tag="ah", name="Z1a")
            ATT1 = a6h.tile([P, ND, SH], f16, tag="ah", name="ATT1")
            H11 = a6h.tile([P, ND, SH], f16, tag="ah", name="H11")
            Z1b = a6h.tile([P, ND, SH], f16, tag="ah", name="Z1b")
            V1 = vp1.tile([P, NS, VW], f16, name="V1")

            def out_ctx1(k, c, t1, g_t, b_t, sl):
                # write straight into M_viewT layout (4 strided slices)
                for s0 in range(4):
                    u = 6 * s0 + k
                    t_, rho = u % 8, u // 8
                    nc.vector.tensor_scalar(
                        MVT[:, t_, rho : 3 * P : 3], t1[:, s0 : SH : 4],
                        g_t[:, k : k + 1], b_t[:, k : k + 1],
                        OP.mult, OP.add,
                    )

            # reuse blk via closure over act6-> but buffers differ; inline call:
            # q from s1 (Sq=SH), k/v from feats (full S), residual = s1
            def blk1():
                pre = "l1_"
                kw = load_w(din[pre + "kT"], D, ND)
                bk = par[pre + "bk"]

                def ev_k(m, ps):
                    nc.vector.tensor_scalar_add(K1[:, m, :], ps, bk[:, m : m + 1])

                fm_proj(feats_k, kw, S, ev_k)

                vw_t = load_w(din[pre + "vTp"], VW, ND)
                vb = wp.tile([1, VW], f16, tag="w", name="vb1")
                nc.sync.dma_start(out=vb, in_=din[pre + "vTp"].ap()[D : D + 1, :])
                for st in range(NS):
                    ps = psA.tile([P, VW], f32, tag="psA", name=f"v1ps{st}")
                    ssl = slice(st * P, (st + 1) * P)
                    for c0, c1 in ((0, 512), (512, VW)):
                        for ki in range(ND + 1):
                            if ki < ND:
                                lhs = _r(feats_k[ki][:, ssl])
                                rhs = _r(vw_t[ki][:, c0:c1])
                            else:
                                lhs = _r(FEATS[0:1, ND, ssl])
                                rhs = _r(vb[:, c0:c1])
                            nc.tensor.matmul(
                                ps[:, c0:c1], lhs, rhs,
                                start=(ki == 0), stop=(ki == ND),
                            )
                    nc.vector.tensor_copy(V1[:, st, :], ps)

                ow = load_w(din[pre + "oT"], D, ND)

                # head-pair attention: two heads share one [128,1024] psum
                for jp in range(ND):
                    ctx_ps = psC.tile(
                        [DH + 1, 2 * SH], f32, tag="psC", name=f"c1ps{jp}"
                    )
                    for st in range(NS):
                        sc_ps = psA.tile(
                            [P, 2 * SH], f32, tag="psA", name=f"s1ps{jp}_{st}"
                        )
                        for half in (0, 1):
                            hb = DH * half
                            sl = slice(SH * half, SH * (half + 1))
                            nc.tensor.matmul(
                                sc_ps[:, sl],
                                K1[hb : hb + DH, jp, st * P : (st + 1) * P],
                                Q1[hb : hb + DH, jp, :],
                                start=True,
                                stop=True,
                            )
                        ex = expool.tile(
                            [P, 2 * SH], f16, tag="ex", name=f"e1x{jp}_{st}"
                        )
                        nc.scalar.activation(ex, sc_ps, AF.Exp)
                        for half in (0, 1):
                            h = 2 * jp + half
                            sl = slice(SH * half, SH * (half + 1))
                            nc.tensor.matmul(
                                ctx_ps[:, sl],
                                V1[:, st, (DH + 1) * h : (DH + 1) * (h + 1)],
                                ex[:, sl],
                                start=(st == 0),
                                stop=(st == NS - 1),
                            )
                    rs = smp.tile([DH + 1, 2 * SH], f16, tag="rs", name=f"r1s{jp}")
                    with nc.allow_low_precision(reason="softmax recip fp16"):
                        nc.vector.reciprocal(
                            rs[DH : DH + 1, :], ctx_ps[DH : DH + 1, :]
                        )
                    for half in (0, 1):
                        hb = DH * half
                        sl = slice(SH * half, SH * (half + 1))
                        br_ps = psS.tile(
                            [DH, 512], f32, tag="psS", name=f"b1r{jp}{half}"
                        )
                        nc.tensor.matmul(
                            br_ps,
                            ones_r64[DH : DH + 1, :],
                            rs[DH : DH + 1, sl],
                            start=True,
                            stop=True,
                        )
                        brc = brp.tile([DH, 512], f32, tag="brc", name=f"b1c{jp}{half}")
                        nc.vector.tensor_copy(brc, br_ps)
                        ctxn = ctxp.tile([DH, SH], f16, tag="ctxh", name=f"c1n{jp}{half}")
                        nc.vector.tensor_mul(ctxn, ctx_ps[0:DH, sl], brc)
                        nc.sync.dma_start(
                            out=CTX1[hb : hb + DH, jp, :], in_=ctxn
                        )

                bo = par[pre + "bo"]

                def ev_o(m, ps):
                    t = sqp.tile([P, 512], f16, tag="sq", name=f"o1e{m}")
                    nc.scalar.activation(
                        t, ps, AF.Identity, bias=bo[:, m : m + 1]
                    )
                    nc.vector.tensor_add(Z1a[:, m, :], t, S1T[:, m, :])

                fm_proj([CTX1[:, k, :] for k in range(ND)], ow, SH, ev_o)

                ag, ab = par[pre + "ag"], par[pre + "ab"]

                def out_att(k, c, t1, g_t, b_t, sl):
                    nc.vector.tensor_scalar(
                        ATT1[:, k, sl], t1, g_t[:, k : k + 1], b_t[:, k : k + 1],
                        OP.mult, OP.add,
                    )

                layernorm(Z1a, SH, ag, ab, out_att)

                w1 = load_w(din[pre + "w1T"], D, ND)
                b1 = par[pre + "b1"]

                def ev_w1(m, ps):
                    nc.scalar.activation(
                        H11[:, m, :], ps, AF.Gelu, bias=b1[:, m : m + 1]
                    )

                fm_proj([ATT1[:, k, :] for k in range(ND)], w1, SH, ev_w1)

                w2 = load_w(din[pre + "w2T"], D, ND)
                b2 = par[pre + "b2"]

                def ev_w2(m, ps):
                    t = sqp.tile([P, 512], f16, tag="sq", name=f"w21e{m}")
                    nc.scalar.activation(
                        t, ps, AF.Identity, bias=b2[:, m : m + 1]
                    )
                    nc.vector.tensor_add(Z1b[:, m, :], t, ATT1[:, m, :])

                fm_proj([H11[:, k, :] for k in range(ND)], w2, SH, ev_w2)

                fg, fb = par[pre + "fg"], par[pre + "fb"]
                layernorm(Z1b, SH, fg, fb, out_ctx1)

            blk1()

        # ================= POOL + FINAL =====================================
        with tc.tile_pool(name="late", bufs=2) as lp, tc.tile_pool(
            name="wbig", bufs=14
        ) as wb:
            # weight prefetch first: overlaps the whole pool chain
            pw_t = []
            for k in range(NS):
                t = wb.tile([P, S], f16, tag="wb", name=f"pw{k}")
                nc.sync.dma_start(out=t, in_=din["pwT"].ap()[k * P : (k + 1) * P, :])
                pw_t.append(t)
            pbT = wb.tile([P, NS], f32, tag="wb", name="pbT")
            nc.gpsimd.dma_start(out=pbT, in_=din["pbT"].ap())
            fin_t = []
            for k in range(2 * ND):
                t = wb.tile([P, D], f16, tag="wb", name=f"fin{k}")
                nc.sync.dma_start(
                    out=t, in_=din["finT"].ap()[k * P : (k + 1) * P, :]
                )
                fin_t.append(t)

            # pool matmul on M_viewT: poolT[128*jt+p, r] accumulated over
            # c-tiles t; evacuate straight into app^T layout via 3 strided
            # activations per jt (APPT[p, jd, sg+4k] = poolT[.., i+3k]).
            APPT = lp.tile([P, ND, SH], f16, tag="lt", name="APPT")
            for jt in range(NS):
                ps = psA.tile([P, 3 * P], f32, tag="psA", name=f"plps{jt}")
                for t in range(NS):
                    nc.tensor.matmul(
                        ps,
                        _r(pw_t[t][:, jt * P : (jt + 1) * P]),
                        _r(MVT[:, t, :]),
                        start=(t == 0),
                        stop=(t == NS - 1),
                    )
                for i in range(3):
                    u2 = 8 * i + jt
                    sg, jd = divmod(u2, 6)
                    nc.scalar.activation(
                        APPT[:, jd, sg : SH : 4],
                        ps[:, i : 3 * P : 3],
                        AF.Identity,
                        bias=pbT[:, jt : jt + 1],
                    )
            # final: out' = finT.T @ [feats_half ; app]
            OUTT = lp.tile([P, ND, SH], f32, tag="lt", name="OUTT")
            for m in range(ND):
                ps = psS.tile([P, SH], f32, tag="psS", name=f"fps{m}")
                for ki in range(2 * ND):
                    rhs = (
                        FEATS[:, ki, 0:SH]
                        if ki < ND
                        else APPT[:, ki - ND, :]
                    )
                    nc.tensor.matmul(
                        ps,
                        _r(fin_t[ki][:, m * P : (m + 1) * P]),
                        _r(rhs),
                        start=(ki == 0),
                        stop=(ki == 2 * ND - 1),
                    )
                nc.scalar.activation(
                    OUTT[:, m, :], ps, AF.Identity, bias=finb[:, m : m + 1]
                )
            nc.sync.dma_start(
                out=outT.ap().rearrange("(j p) s -> p j s", p=P), in_=OUTT
            )


def _prep_inputs(inputs):
    e = np.ascontiguousarray(np.asarray(inputs["e"], dtype=np.float32))
    f = np.ascontiguousarray(np.asarray(inputs["f"], dtype=np.float32))
    wq = np.asarray(inputs["wq"], np.float32)
    wk = np.asarray(inputs["wk"], np.float32)
    wv = np.asarray(inputs["wv"], np.float32)
    wo = np.asarray(inputs["wo"], np.float32)
    bq = np.asarray(inputs["bq"], np.float32)
    bk = np.asarray(inputs["bk"], np.float32)
    bv = np.asarray(inputs["bv"], np.float32)
    bo = np.asarray(inputs["bo"], np.float32)
    ag = np.asarray(inputs["attn_ln_g"], np.float32)
    ab = np.asarray(inputs["attn_ln_b"], np.float32)
    w1 = np.asarray(inputs["ffn_w1"], np.float32)
    b1 = np.asarray(inputs["ffn_b1"], np.float32)
    w2 = np.asarray(inputs["ffn_w2"], np.float32)
    b2 = np.asarray(inputs["ffn_b2"], np.float32)
    fg = np.asarray(inputs["ffn_ln_g"], np.float32)
    fb = np.asarray(inputs["ffn_ln_b"], np.float32)
    pw = np.asarray(inputs["pool_w"], np.float32)
    pb = np.asarray(inputs["pool_b"], np.float32)
    fw = np.asarray(inputs["final_w"], np.float32)
    fnb = np.asarray(inputs["final_b"], np.float32)

    def vec6(v):
        return np.ascontiguousarray(v.reshape(ND, P).T)

    scale = 1.0 / math.sqrt(DH)
    in_maps = []
    for c in range(8):
        ti, b, h = c // 4, (c // 2) % 2, c % 2
        src = e if ti == 0 else f
        s1 = f if ti == 0 else e
        own = slice(SH * h, SH * (h + 1))
        oth = slice(SH * (1 - h), SH * (2 - h))
        src_b = src[:, b, :]
        src_perm = np.concatenate([src_b[own], src_b[oth]], axis=0)
        m = {
            "srcT": np.ascontiguousarray(src_perm.T).astype(np.float16),
            "s1T": np.ascontiguousarray(s1[own, b, :].T).astype(np.float16),
            "pwT": np.ascontiguousarray(
                np.concatenate([pw[ti].T, pb[ti][None, :]], axis=0)
            ).astype(np.float16),
            "pbT": np.ascontiguousarray(pb[ti].reshape(NS, P).T),
            "finT": np.ascontiguousarray(fw[ti].T).astype(np.float16),
            "finb": vec6(fnb[ti]),
        }
        for li in (0, 1):
            vTp = np.zeros((D + 1, VW), np.float16)
            wvT = wv[ti, li].T
            for hh in range(H):
                vTp[0:D, (DH + 1) * hh : (DH + 1) * hh + DH] = wvT[
                    :, DH * hh : DH * (hh + 1)
                ]
                vTp[D, (DH + 1) * hh : (DH + 1) * hh + DH] = bv[
                    ti, li, DH * hh : DH * (hh + 1)
                ]
                vTp[D, (DH + 1) * hh + DH] = 1.0
            m.update(
                {
                    f"l{li}_qT": np.ascontiguousarray(wq[ti, li].T * scale).astype(np.float16),
                    f"l{li}_kT": np.ascontiguousarray(wk[ti, li].T).astype(np.float16),
                    f"l{li}_vTp": vTp,
                    f"l{li}_oT": np.ascontiguousarray(wo[ti, li].T).astype(np.float16),
                    f"l{li}_w1T": np.ascontiguousarray(w1[ti, li].T).astype(np.float16),
                    f"l{li}_w2T": np.ascontiguousarray(w2[ti, li].T).astype(np.float16),
                    f"l{li}_bq": vec6(bq[ti, li] * scale),
                    f"l{li}_bk": vec6(bk[ti, li]),
                    f"l{li}_bo": vec6(bo[ti, li]),
                    f"l{li}_b1": vec6(b1[ti, li]),
                    f"l{li}_b2": vec6(b2[ti, li]),
                    f"l{li}_ag": vec6(ag[ti, li]),
                    f"l{li}_ab": vec6(ab[ti, li]),
                    f"l{li}_fg": vec6(fg[ti, li]),
                    f"l{li}_fb": vec6(fb[ti, li]),
                }
            )
        in_maps.append(m)
    return in_maps


def get_program():
    if "nc" not in _BUILT:
        _BUILT["nc"] = _build_program()
    return _BUILT["nc"]


def kernel(**inputs):
    from concourse.bass_utils import run_bass_kernel_spmd

    nc = get_program()
    in_maps = _prep_inputs(inputs)
    res = run_bass_kernel_spmd(nc, in_maps, core_ids=list(range(8)))
    c_e_f = np.empty((S, B, D), np.float32)
    c_f_e = np.empty((S, B, D), np.float32)
    for c in range(8):
        ti, b, h = c // 4, (c // 2) % 2, c % 2
        dst = c_e_f if ti == 0 else c_f_e
        dst[SH * h : SH * (h + 1), b, :] = res.results[c]["outT"].T
    return c_e_f, c_f_e



# revision 12
# speedup vs baseline: 1.1222x; 1.0795x over previous
# Trainium2 Bass kernel for nn_Cross_Transformer (dense_transformer).
#
# Sharding: 8 cores = 2 towers x 2 batches x 2 sequence-halves.
# Each core computes block0 (self-attention) in full (its inputs are permuted
# so its own half leads, keeping the program SPMD-uniform), then its half of
# block1 (cross-attention), pool, and final projection. No collectives.
#
# Layout: activations are feature-major [D on partitions, S on free] so every
# projection is lhsT=W^T tiles (stationary) x X' (moving). Matmuls run as
# float32r (1 cyc/row at N>=256). Attention probabilities/V run in fp16.
# LayerNorm over D (= partitions) uses ones-column matmuls for sums and a
# K=1 ones-row matmul to broadcast per-column stats across partitions.

import math

import numpy as np

S = 1024
B = 2
D = 768
H = 12
DH = 64
EPS = 1e-6
SH = S // 2  # 512, per-core block1 rows
P = 128
ND = D // P  # 6 d-tiles
NS = S // P  # 8 s-tiles
VW = H * (DH + 1)  # 780: v row-major padded with a ones column per head
VWP = 784  # fp8 DoubleRow needs the st-pair stride 16B-aligned

F32 = None  # filled lazily (mybir.dt.float32)
_BUILT = {}


def _dt():
    from concourse import mybir

    return mybir.dt


def _r(ap):
    """View an fp32 AP as float32r for full-rate PE matmuls; fp16 passes through."""
    dt = _dt()
    return ap.bitcast(dt.float32r) if ap.dtype == dt.float32 else ap


def _build_program():
    import concourse.bacc as bacc
    import concourse.tile as tile
    from concourse import mybir
    from concourse.masks import make_identity

    dt = mybir.dt
    f32 = dt.float32
    f16 = dt.float16
    AF = mybir.ActivationFunctionType
    OP = mybir.AluOpType

    nc = bacc.Bacc("TRN2", target_bir_lowering=False, debug=False, num_devices=8)

    # ---- DRAM I/O ----
    din = {}

    def dram_in(name, shape, dty=None):
        din[name] = nc.dram_tensor(
            name, list(shape), dty or f16, kind="ExternalInput"
        )
        return din[name]

    dram_in("srcT", (D, S))
    dram_in("s1T", (D, SH))
    for li in (0, 1):
        dram_in(f"l{li}_qT", (D, D))
        dram_in(f"l{li}_kT", (D, D))
        dram_in(f"l{li}_vTp", (D + 1, VW))
        dram_in(f"l{li}_oT", (D, D))
        dram_in(f"l{li}_w1T", (D, D))
        dram_in(f"l{li}_w2T", (D, D))
        for bn in ("bq", "bk", "bo", "b1", "b2", "ag", "ab", "fg", "fb"):
            dram_in(f"l{li}_{bn}", (P, ND), f32)
    dram_in("pwT", (S + 1, S))
    dram_in("pbT", (P, NS), f32)
    dram_in("finT", (2 * D, D))
    dram_in("finb", (P, ND), f32)

    outT = nc.dram_tensor("outT", [D, SH], f32, kind="ExternalOutput")

    with tile.TileContext(nc) as tc:
        _emit(nc, tc, tile, dt, AF, OP, din, outT, make_identity)

    nc.compile()
    return nc


def _emit(nc, tc, tile, dt, AF, OP, din, outT, make_identity):
    f32 = dt.float32
    f16 = dt.float16
    f8 = dt.float8e4
    from concourse import mybir as _mb

    PM = _mb.MatmulPerfMode
    import contextlib

    es = contextlib.ExitStack()
    with es:
        persist = es.enter_context(tc.tile_pool(name="persist", bufs=1))
        wp = es.enter_context(tc.tile_pool(name="wp", bufs=13))
        psA = es.enter_context(tc.tile_pool(name="psA", bufs=2, space="PSUM"))
        psC = es.enter_context(tc.tile_pool(name="psC", bufs=1, space="PSUM"))
        psS = es.enter_context(tc.tile_pool(name="psS", bufs=2, space="PSUM"))
        expool = es.enter_context(tc.tile_pool(name="expool", bufs=6))
        ctxp = es.enter_context(tc.tile_pool(name="ctxp", bufs=3))
        sqp = es.enter_context(tc.tile_pool(name="sqp", bufs=4))
        brp = es.enter_context(tc.tile_pool(name="brp", bufs=4))
        smp = es.enter_context(tc.tile_pool(name="smp", bufs=4))

        # --- constants ---
        ident = persist.tile([P, P], f16, name="ident")
        make_identity(nc, ident)
        ones_col = persist.tile([P, 1], f16, name="ones_col")
        nc.vector.memset(ones_col, 1.0)
        ones_r64 = persist.tile([DH + 1, DH], f16, name="ones_r64")
        nc.vector.memset(ones_r64[DH : DH + 1, :], 1.0)
        ones_r128 = persist.tile([1, P], f16, name="ones_r128")
        nc.vector.memset(ones_r128, 1.0)

        # --- small params (biases, LN) ---
        par = {}
        for li in (0, 1):
            for bn in ("bq", "bk", "bo", "b1", "b2", "ag", "ab", "fg", "fb"):
                t = persist.tile([P, ND], f32, name=f"p_l{li}_{bn}")
                nc.gpsimd.dma_start(out=t, in_=din[f"l{li}_{bn}"].ap())
                par[f"l{li}_{bn}"] = t
        finb = persist.tile([P, ND], f32, name="p_finb")
        nc.gpsimd.dma_start(out=finb, in_=din["finb"].ap())

        # persistent activations
        FEATS = persist.tile([P, ND + 1, S], f16, name="FEATS")
        nc.vector.memset(FEATS[0:1, ND, :], 1.0)
        S1T = persist.tile([P, ND, SH], f16, name="S1T")
        nc.sync.dma_start(
            out=S1T, in_=din["s1T"].ap().rearrange("(j p) s -> p j s", p=P)
        )
        # M_viewT [c-tile partitions, c-tile idx, r]: M_view[r, 128t+p] with
        # r = 3k+rho -> ctx1_ln[4k + u//6, d=128*(u%6)+p], u = 8*rho + t.
        # Written directly (strided) by block1's final LN.
        MVT = persist.tile([P, NS, 3 * P], f16, name="MVT")
        Q1 = persist.tile([P, ND, SH], f16, name="Q1")
        rb_t = persist.tile([P, S], f16, name="rb_t")
        mrb_t = persist.tile([P, S], f16, name="mrb_t")
        # LN small stats rows
        lnm = persist.tile([1, S], f32, name="lnm")
        lns2 = persist.tile([1, S], f32, name="lns2")
        lnt = persist.tile([1, S], f32, name="lnt")
        lnr16 = persist.tile([1, S], f16, name="lnr16")
        lnmr16 = persist.tile([1, S], f16, name="lnmr16")

        def load_w(dram_h, width, nk, tagsuf=""):
            """DMA weight k-tiles [P, width] (+ optional trailing [1, width])."""
            ap = dram_h.ap()
            tiles = []
            for t in range(nk):
                wt = wp.tile([P, width], f16, tag="w", name=f"w_{dram_h.name}_{t}")
                nc.sync.dma_start(out=wt, in_=ap[t * P : (t + 1) * P, :])
                tiles.append(wt)
            return tiles

        def fm_proj(x_ktiles, w_tiles, Sx, evac, extra_k=None):
            """Feature-major projection: out[m] = sum_k w[k][:,m].T @ x[k].
            x_ktiles: list of APs [kp, Sx]; w_tiles: list of APs [kp, D].
            evac(m, ps): consume psum [P, Sx]."""
            nch = Sx // 512
            ks = list(zip(x_ktiles, w_tiles))
            if extra_k is not None:
                ks.append(extra_k)
            for m in range(ND):
                ps = psA.tile([P, Sx], f32, tag="psA", name=f"ps_m{m}")
                for c in range(nch):
                    sl = slice(512 * c, 512 * (c + 1))
                    for ki, (xk, wk) in enumerate(ks):
                        nc.tensor.matmul(
                            ps[:, sl],
                            _r(wk[:, m * P : (m + 1) * P]),
                            _r(xk[:, sl]),
                            start=(ki == 0),
                            stop=(ki == len(ks) - 1),
                        )
                evac(m, ps)

        def layernorm(Zt, Sx, g_t, b_t, out_fn):
            """LN over partitions(d) of Zt [P, ND, Sx] (Bessel std + eps).
            out_fn(k, c, src_ap, sl): writes result tile."""
            nch = Sx // 512
            for c in range(nch):
                sl = slice(512 * c, 512 * (c + 1))
                sum_ps = psS.tile([1, 512], f32, tag="psS", name=f"lnsum{c}")
                for k in range(ND):
                    nc.tensor.matmul(
                        sum_ps,
                        _r(ones_col),
                        _r(Zt[:, k, sl]),
                        start=(k == 0),
                        stop=(k == ND - 1),
                    )
                nc.scalar.activation(
                    lnm[:, sl], sum_ps, AF.Identity, scale=1.0 / D
                )
                sq_ps = psS.tile([1, 512], f32, tag="psS", name=f"lnsq{c}")
                for k in range(ND):
                    sq = sqp.tile([P, 512], f16, tag="sq", name=f"sq{k}{c}")
                    nc.vector.tensor_mul(sq, Zt[:, k, sl], Zt[:, k, sl])
                    nc.tensor.matmul(
                        sq_ps,
                        _r(ones_col),
                        _r(sq),
                        start=(k == 0),
                        stop=(k == ND - 1),
                    )
                nc.scalar.activation(
                    lns2[:, sl], sq_ps, AF.Identity, scale=1.0 / (D - 1)
                )
                # per-chunk stats chain so chunk 0 applies while chunk 1 sums
                nc.scalar.activation(
                    lnt[:, sl], lnm[:, sl], AF.Square,
                    scale=math.sqrt(D / (D - 1.0)),
                )
                nc.vector.tensor_sub(lns2[:, sl], lns2[:, sl], lnt[:, sl])
                nc.scalar.sqrt(lns2[:, sl], lns2[:, sl])
                nc.vector.tensor_scalar_add(lns2[:, sl], lns2[:, sl], EPS)
                nc.vector.reciprocal(lnt[:, sl], lns2[:, sl])  # r
                nc.vector.tensor_mul(lnm[:, sl], lnm[:, sl], lnt[:, sl])  # m*r
                nc.scalar.activation(lnr16[:, sl], lnt[:, sl], AF.Identity)
                nc.scalar.activation(lnmr16[:, sl], lnm[:, sl], AF.Identity)
            for c in range(nch):
                sl = slice(512 * c, 512 * (c + 1))
                rb_ps = psS.tile([P, 512], f32, tag="psS", name=f"rbps{c}")
                nc.tensor.matmul(
                    rb_ps, ones_r128, lnr16[0:1, sl], start=True, stop=True
                )
                nc.vector.tensor_copy(rb_t[:, sl], rb_ps)
                mrb_ps = psS.tile([P, 512], f32, tag="psS", name=f"mrbps{c}")
                nc.tensor.matmul(
                    mrb_ps, ones_r128, lnmr16[0:1, sl], start=True, stop=True
                )
                nc.vector.tensor_copy(mrb_t[:, sl], mrb_ps)
                for k in range(ND):
                    t1 = sqp.tile([P, 512], f16, tag="sq", name=f"ap{k}{c}")
                    nc.vector.tensor_mul(t1, Zt[:, k, sl], rb_t[:, sl])
                    nc.vector.tensor_sub(t1, t1, mrb_t[:, sl])
                    out_fn(k, c, t1, g_t, b_t, sl)

        def attention(Sq, Qt, Kt, Vt, ctx_sink):
            """ctx_sink(h, ap[64, Sq]) receives normalized per-head context.
            Probabilities and V run in fp8e4; P@V uses DoubleRow over
            st-pairs (256-row contraction at 0.5 cyc/row)."""
            nq = Sq // 512
            for h in range(H):
                hb = DH * (h % 2)
                j = h // 2
                ctx_ps = psC.tile([DH + 1, Sq], f32, tag="psC", name=f"ctx{h}")
                ex2 = None
                for st in range(NS):
                    sc_ps = psA.tile([P, Sq], f32, tag="psA", name=f"sc{h}_{st}")
                    for c in range(nq):
                        sl = slice(512 * c, 512 * (c + 1))
                        nc.tensor.matmul(
                            sc_ps[:, sl],
                            _r(Kt[hb : hb + DH, j, st * P : (st + 1) * P]),
                            _r(Qt[hb : hb + DH, j, sl]),
                            start=True,
                            stop=True,
                        )
                    if st % 2 == 0:
                        ex2 = expool.tile(
                            [P, 2, Sq], f8, tag="ex", name=f"ex{h}_{st // 2}"
                        )
                    with nc.allow_low_precision(reason="fp8 probs"):
                        nc.scalar.activation(ex2[:, st % 2, :], sc_ps, AF.Exp)
                    if st % 2 == 1:
                        sp = st // 2
                        for c in range(nq):
                            sl = slice(512 * c, 512 * (c + 1))
                            nc.tensor.matmul(
                                ctx_ps[:, sl],
                                Vt[:, 2 * sp : 2 * sp + 2,
                                   (DH + 1) * h : (DH + 1) * h + DH + 1],
                                ex2[:, :, sl],
                                start=(sp == 0),
                                stop=(sp == NS // 2 - 1),
                                perf_mode=PM.DoubleRow,
                            )
                rs = smp.tile([DH + 1, Sq], f16, tag="rs", name=f"rs{h}")
                with nc.allow_low_precision(reason="softmax recip fp16"):
                    nc.vector.reciprocal(
                        rs[DH : DH + 1, :], ctx_ps[DH : DH + 1, :]
                    )
                ctxn = ctxp.tile([DH, Sq], f16, tag="ctxh", name=f"ctxn{h}")
                for c in range(nq):
                    sl = slice(512 * c, 512 * (c + 1))
                    br_ps = psS.tile([DH, 512], f32, tag="psS", name=f"br{h}{c}")
                    nc.tensor.matmul(
                        br_ps,
                        ones_r64[DH : DH + 1, :],
                        rs[DH : DH + 1, sl],
                        start=True,
                        stop=True,
                    )
                    brc = brp.tile([DH, 512], f32, tag="brc", name=f"brc{h}{c}")
                    nc.vector.tensor_copy(brc, br_ps)
                    nc.vector.tensor_mul(ctxn[:, sl], ctx_ps[0:DH, sl], brc)
                ctx_sink(h, ctxn)

        # ================= BLOCK 0 (full S, self-attention on src) =========
        with tc.tile_pool(name="b0a", bufs=5) as act6, tc.tile_pool(
            name="b0x", bufs=1
        ) as act7, tc.tile_pool(name="b0v", bufs=1) as vp0:
            X0 = act7.tile([P, ND + 1, S], f16, tag="a7", name="X0")
            nc.sync.dma_start(
                out=X0[:, 0:ND, :],
                in_=din["srcT"].ap().rearrange("(j p) s -> p j s", p=P),
            )
            nc.vector.memset(X0[0:1, ND, :], 1.0)

            x_k = [X0[:, k, :] for k in range(ND)]
            V0 = vp0.tile([P, NS, VWP], f8, name="V0")

            # block1 q-projection depends only on inputs: emit first to fill
            # the startup bubble while block0 weights stream in.
            q1w = load_w(din["l1_qT"], D, ND)
            bq1 = par["l1_bq"]

            def ev_q1(m, ps):
                nc.vector.tensor_scalar_add(Q1[:, m, :], ps, bq1[:, m : m + 1])

            s1_k0 = [S1T[:, k, :] for k in range(ND)]
            fm_proj(s1_k0, q1w, SH, ev_q1)

            def blk(li, Sq, Qsrc_k, KVsrc_k, kv_ones, resid_k, Vt, CTXa, Za, Zb,
                    ATT, OUTLN_fn):
                """One transformer block in feature-major layout."""
                pre = f"l{li}_"
                # --- q/k (feature-major) ---
                qw = load_w(din[pre + "qT"], D, ND)
                Qt = CTXa["Q"]
                bq = par[pre + "bq"]

                def ev_q(m, ps):
                    nc.vector.tensor_scalar_add(Qt[:, m, 0:Sq], ps, bq[:, m : m + 1])

                fm_proj(Qsrc_k, qw, Sq, ev_q)

                kw = load_w(din[pre + "kT"], D, ND)
                Kt = CTXa["K"]
                bk = par[pre + "bk"]

                def ev_k(m, ps):
                    nc.vector.tensor_scalar_add(Kt[:, m, :], ps, bk[:, m : m + 1])

                fm_proj(KVsrc_k, kw, S, ev_k)

                # --- v (row-major, bias+ones via augmented row) ---
                vw_t = load_w(din[pre + "vTp"], VW, ND)
                vb = wp.tile([1, VW], f16, tag="w", name=f"vb{li}")
                nc.sync.dma_start(out=vb, in_=din[pre + "vTp"].ap()[D : D + 1, :])
                for st in range(NS):
                    ps = psA.tile([P, VW], f32, tag="psA", name=f"vps{li}_{st}")
                    ssl = slice(st * P, (st + 1) * P)
                    for c, (c0, c1) in enumerate(((0, 512), (512, VW))):
                        for ki in range(ND + 1):
                            if ki < ND:
                                lhs = _r(KVsrc_k[ki][:, ssl])
                                rhs = _r(vw_t[ki][:, c0:c1])
                            else:
                                lhs = _r(kv_ones[:, ssl])
                                rhs = _r(vb[:, c0:c1])
                            nc.tensor.matmul(
                                ps[:, c0:c1], lhs, rhs,
                                start=(ki == 0), stop=(ki == ND),
                            )
                    with nc.allow_low_precision(reason="fp8 V"):
                        nc.vector.tensor_copy(Vt[:, st, 0:VW], ps)

                # --- attention ---
                ow = load_w(din[pre + "oT"], D, ND)
                CTXh = CTXa["CTX"]

                def sink(h, ctxn):
                    hb = DH * (h % 2)
                    j = h // 2
                    nc.sync.dma_start(out=CTXh[hb : hb + DH, j, 0:Sq], in_=ctxn)

                attention(Sq, Qt, Kt, Vt, sink)

                # --- o-proj + bias + residual -> Za ---
                bo = par[pre + "bo"]

                def ev_o(m, ps):
                    for c in range(Sq // 512):
                        sl = slice(512 * c, 512 * (c + 1))
                        t = sqp.tile([P, 512], f16, tag="sq", name=f"oe{m}_{c}")
                        nc.scalar.activation(
                            t, ps[:, sl], AF.Identity, bias=bo[:, m : m + 1]
                        )
                        nc.vector.tensor_add(Za[:, m, sl], t, resid_k[m][:, sl])

                fm_proj([CTXh[:, k, 0:Sq] for k in range(ND)], ow, Sq, ev_o)

                # --- LN (attn) -> ATT ---
                ag, ab = par[pre + "ag"], par[pre + "ab"]

                def out_att(k, c, t1, g_t, b_t, sl):
                    nc.vector.tensor_scalar(
                        ATT[:, k, sl], t1, g_t[:, k : k + 1], b_t[:, k : k + 1],
                        OP.mult, OP.add,
                    )

                layernorm(Za, Sq, ag, ab, out_att)

                # --- ffn w1 + gelu ---
                w1 = load_w(din[pre + "w1T"], D, ND)
                H1 = CTXa["H1"]
                b1 = par[pre + "b1"]

                def ev_w1(m, ps):
                    nc.scalar.activation(
                        H1[:, m, 0:Sq], ps, AF.Gelu, bias=b1[:, m : m + 1]
                    )

                fm_proj([ATT[:, k, 0:Sq] for k in range(ND)], w1, Sq, ev_w1)

                # --- ffn w2 + bias + residual -> Zb, LN -> OUTLN_fn ---
                w2 = load_w(din[pre + "w2T"], D, ND)
                b2 = par[pre + "b2"]

                def ev_w2(m, ps):
                    for c in range(Sq // 512):
                        sl = slice(512 * c, 512 * (c + 1))
                        t = sqp.tile([P, 512], f16, tag="sq", name=f"w2e{m}_{c}")
                        nc.scalar.activation(
                            t, ps[:, sl], AF.Identity, bias=b2[:, m : m + 1]
                        )
                        nc.vector.tensor_add(Zb[:, m, sl], t, ATT[:, m, sl])

                fm_proj([H1[:, k, 0:Sq] for k in range(ND)], w2, Sq, ev_w2)

                fg, fb = par[pre + "fg"], par[pre + "fb"]
                layernorm(Zb, Sq, fg, fb, OUTLN_fn)

            # block0 tensor buffers (rotating in act6)
            Q0 = act6.tile([P, ND, S], f16, tag="a6", name="Q0")
            K0 = act6.tile([P, ND, S], f16, tag="a6", name="K0")
            CTX0 = act6.tile([P, ND, S], f16, tag="a6", name="CTX0")
            Z0a = act6.tile([P, ND, S], f16, tag="a6", name="Z0a")
            ATT0 = act6.tile([P, ND, S], f16, tag="a6", name="ATT0")
            H10 = act6.tile([P, ND, S], f16, tag="a6", name="H10")
            Z0b = act6.tile([P, ND, S], f16, tag="a6", name="Z0b")

            def out_feats(k, c, t1, g_t, b_t, sl):
                nc.vector.tensor_scalar(
                    FEATS[:, k, sl], t1, g_t[:, k : k + 1], b_t[:, k : k + 1],
                    OP.mult, OP.add,
                )

            blk(
                0, S, x_k, x_k, X0[0:1, ND, :], x_k, V0,
                {"Q": Q0, "K": K0, "CTX": CTX0, "H1": H10},
                Z0a, Z0b, ATT0, out_feats,
            )

        # ================= BLOCK 1 (half S on q-side, cross-attention) ======
        with tc.tile_pool(name="b1a", bufs=4) as a6h, tc.tile_pool(
            name="b1b", bufs=1
        ) as a6f, tc.tile_pool(name="b1v", bufs=1) as vp1:
            feats_k = [FEATS[:, k, :] for k in range(ND)]
            s1_k = [S1T[:, k, :] for k in range(ND)]
            K1 = a6f.tile([P, ND, S], f16, tag="af", name="K1")
            CTX1 = a6h.tile([P, ND, SH], f16, tag="ah", name="CTX1")
            Z1a = a6h.tile([P, ND, SH], f16, tag="ah", name="Z1a")
            ATT1 = a6h.tile([P, ND, SH], f16, tag="ah", name="ATT1")
            H11 = a6h.tile([P, ND, SH], f16, tag="ah", name="H11")
            Z1b = a6h.tile([P, ND, SH], f16, tag="ah", name="Z1b")
            V1 = vp1.tile([P, NS, VWP], f8, name="V1")

            def out_ctx1(k, c, t1, g_t, b_t, sl):
                # write straight into M_viewT layout (4 strided slices)
                for s0 in range(4):
                    u = 6 * s0 + k
                    t_, rho = u % 8, u // 8
                    nc.vector.tensor_scalar(
                        MVT[:, t_, rho : 3 * P : 3], t1[:, s0 : SH : 4],
                        g_t[:, k : k + 1], b_t[:, k : k + 1],
                        OP.mult, OP.add,
                    )

            # reuse blk via closure over act6-> but buffers differ; inline call:
            # q from s1 (Sq=SH), k/v from feats (full S), residual = s1
            def blk1():
                pre = "l1_"
                kw = load_w(din[pre + "kT"], D, ND)
                bk = par[pre + "bk"]

                def ev_k(m, ps):
                    nc.vector.tensor_scalar_add(K1[:, m, :], ps, bk[:, m : m + 1])

                fm_proj(feats_k, kw, S, ev_k)

                vw_t = load_w(din[pre + "vTp"], VW, ND)
                vb = wp.tile([1, VW], f16, tag="w", name="vb1")
                nc.sync.dma_start(out=vb, in_=din[pre + "vTp"].ap()[D : D + 1, :])
                for st in range(NS):
                    ps = psA.tile([P, VW], f32, tag="psA", name=f"v1ps{st}")
                    ssl = slice(st * P, (st + 1) * P)
                    for c0, c1 in ((0, 512), (512, VW)):
                        for ki in range(ND + 1):
                            if ki < ND:
                                lhs = _r(feats_k[ki][:, ssl])
                                rhs = _r(vw_t[ki][:, c0:c1])
                            else:
                                lhs = _r(FEATS[0:1, ND, ssl])
                                rhs = _r(vb[:, c0:c1])
                            nc.tensor.matmul(
                                ps[:, c0:c1], lhs, rhs,
                                start=(ki == 0), stop=(ki == ND),
                            )
                    with nc.allow_low_precision(reason="fp8 V"):
                        nc.vector.tensor_copy(V1[:, st, 0:VW], ps)

                ow = load_w(din[pre + "oT"], D, ND)

                # head-pair attention: two heads share one [128,1024] psum
                for jp in range(ND):
                    ctx_ps = psC.tile(
                        [DH + 1, 2 * SH], f32, tag="psC", name=f"c1ps{jp}"
                    )
                    for st in range(NS):
                        sc_ps = psA.tile(
                            [P, 2 * SH], f32, tag="psA", name=f"s1ps{jp}_{st}"
                        )
                        for half in (0, 1):
                            hb = DH * half
                            sl = slice(SH * half, SH * (half + 1))
                            nc.tensor.matmul(
                                sc_ps[:, sl],
                                K1[hb : hb + DH, jp, st * P : (st + 1) * P],
                                Q1[hb : hb + DH, jp, :],
                                start=True,
                                stop=True,
                            )
                        if st % 2 == 0:
                            ex2 = expool.tile(
                                [P, 2, 2 * SH], f8, tag="ex",
                                name=f"e1x{jp}_{st // 2}",
                            )
                        with nc.allow_low_precision(reason="fp8 probs"):
                            nc.scalar.activation(ex2[:, st % 2, :], sc_ps, AF.Exp)
                        if st % 2 == 1:
                            sp = st // 2
                            for half in (0, 1):
                                h = 2 * jp + half
                                sl = slice(SH * half, SH * (half + 1))
                                nc.tensor.matmul(
                                    ctx_ps[:, sl],
                                    V1[:, 2 * sp : 2 * sp + 2,
                                       (DH + 1) * h : (DH + 1) * h + DH + 1],
                                    ex2[:, :, sl],
                                    start=(sp == 0),
                                    stop=(sp == NS // 2 - 1),
                                    perf_mode=PM.DoubleRow,
                                )
                    rs = smp.tile([DH + 1, 2 * SH], f16, tag="rs", name=f"r1s{jp}")
                    with nc.allow_low_precision(reason="softmax recip fp16"):
                        nc.vector.reciprocal(
                            rs[DH : DH + 1, :], ctx_ps[DH : DH + 1, :]
                        )
                    for half in (0, 1):
                        hb = DH * half
                        sl = slice(SH * half, SH * (half + 1))
                        br_ps = psS.tile(
                            [DH, 512], f32, tag="psS", name=f"b1r{jp}{half}"
                        )
                        nc.tensor.matmul(
                            br_ps,
                            ones_r64[DH : DH + 1, :],
                            rs[DH : DH + 1, sl],
                            start=True,
                            stop=True,
                        )
                        brc = brp.tile([DH, 512], f32, tag="brc", name=f"b1c{jp}{half}")
                        nc.vector.tensor_copy(brc, br_ps)
                        ctxn = ctxp.tile([DH, SH], f16, tag="ctxh", name=f"c1n{jp}{half}")
                        nc.vector.tensor_mul(ctxn, ctx_ps[0:DH, sl], brc)
                        nc.sync.dma_start(
                            out=CTX1[hb : hb + DH, jp, :], in_=ctxn
                        )

                bo = par[pre + "bo"]

                def ev_o(m, ps):
                    t = sqp.tile([P, 512], f16, tag="sq", name=f"o1e{m}")
                    nc.scalar.activation(
                        t, ps, AF.Identity, bias=bo[:, m : m + 1]
                    )
                    nc.vector.tensor_add(Z1a[:, m, :], t, S1T[:, m, :])

                fm_proj([CTX1[:, k, :] for k in range(ND)], ow, SH, ev_o)

                ag, ab = par[pre + "ag"], par[pre + "ab"]

                def out_att(k, c, t1, g_t, b_t, sl):
                    nc.vector.tensor_scalar(
                        ATT1[:, k, sl], t1, g_t[:, k : k + 1], b_t[:, k : k + 1],
                        OP.mult, OP.add,
                    )

                layernorm(Z1a, SH, ag, ab, out_att)

                w1 = load_w(din[pre + "w1T"], D, ND)
                b1 = par[pre + "b1"]

                def ev_w1(m, ps):
                    nc.scalar.activation(
                        H11[:, m, :], ps, AF.Gelu, bias=b1[:, m : m + 1]
                    )

                fm_proj([ATT1[:, k, :] for k in range(ND)], w1, SH, ev_w1)

                w2 = load_w(din[pre + "w2T"], D, ND)
                b2 = par[pre + "b2"]

                def ev_w2(m, ps):
                    t = sqp.tile([P, 512], f16, tag="sq", name=f"w21e{m}")
                    nc.scalar.activation(
                        t, ps, AF.Identity, bias=b2[:, m : m + 1]
                    )
                    nc.vector.tensor_add(Z1b[:, m, :], t, ATT1[:, m, :])

                fm_proj([H11[:, k, :] for k in range(ND)], w2, SH, ev_w2)

                fg, fb = par[pre + "fg"], par[pre + "fb"]
                layernorm(Z1b, SH, fg, fb, out_ctx1)

            blk1()

        # ================= POOL + FINAL =====================================
        with tc.tile_pool(name="late", bufs=2) as lp, tc.tile_pool(
            name="wbig", bufs=14
        ) as wb:
            # weight prefetch first: overlaps the whole pool chain
            pw_t = []
            for k in range(NS):
                t = wb.tile([P, S], f16, tag="wb", name=f"pw{k}")
                nc.sync.dma_start(out=t, in_=din["pwT"].ap()[k * P : (k + 1) * P, :])
                pw_t.append(t)
            pbT = wb.tile([P, NS], f32, tag="wb", name="pbT")
            nc.gpsimd.dma_start(out=pbT, in_=din["pbT"].ap())
            fin_t = []
            for k in range(2 * ND):
                t = wb.tile([P, D], f16, tag="wb", name=f"fin{k}")
                nc.sync.dma_start(
                    out=t, in_=din["finT"].ap()[k * P : (k + 1) * P, :]
                )
                fin_t.append(t)

            # pool matmul on M_viewT: poolT[128*jt+p, r] accumulated over
            # c-tiles t; evacuate straight into app^T layout via 3 strided
            # activations per jt (APPT[p, jd, sg+4k] = poolT[.., i+3k]).
            APPT = lp.tile([P, ND, SH], f16, tag="lt", name="APPT")
            for jt in range(NS):
                ps = psA.tile([P, 3 * P], f32, tag="psA", name=f"plps{jt}")
                for t in range(NS):
                    nc.tensor.matmul(
                        ps,
                        _r(pw_t[t][:, jt * P : (jt + 1) * P]),
                        _r(MVT[:, t, :]),
                        start=(t == 0),
                        stop=(t == NS - 1),
                    )
                for i in range(3):
                    u2 = 8 * i + jt
                    sg, jd = divmod(u2, 6)
                    nc.scalar.activation(
                        APPT[:, jd, sg : SH : 4],
                        ps[:, i : 3 * P : 3],
                        AF.Identity,
                        bias=pbT[:, jt : jt + 1],
                    )
            # final: out' = finT.T @ [feats_half ; app]
            OUTT = lp.tile([P, ND, SH], f32, tag="lt", name="OUTT")
            for m in range(ND):
                ps = psS.tile([P, SH], f32, tag="psS", name=f"fps{m}")
                for ki in range(2 * ND):
                    rhs = (
                        FEATS[:, ki, 0:SH]
                        if ki < ND
                        else APPT[:, ki - ND, :]
                    )
                    nc.tensor.matmul(
                        ps,
                        _r(fin_t[ki][:, m * P : (m + 1) * P]),
                        _r(rhs),
                        start=(ki == 0),
                        stop=(ki == 2 * ND - 1),
                    )
                nc.scalar.activation(
                    OUTT[:, m, :], ps, AF.Identity, bias=finb[:, m : m + 1]
                )
            nc.sync.dma_start(
                out=outT.ap().rearrange("(j p) s -> p j s", p=P), in_=OUTT
            )


def _prep_inputs(inputs):
    e = np.ascontiguousarray(np.asarray(inputs["e"], dtype=np.float32))
    f = np.ascontiguousarray(np.asarray(inputs["f"], dtype=np.float32))
    wq = np.asarray(inputs["wq"], np.float32)
    wk = np.asarray(inputs["wk"], np.float32)
    wv = np.asarray(inputs["wv"], np.float32)
    wo = np.asarray(inputs["wo"], np.float32)
    bq = np.asarray(inputs["bq"], np.float32)
    bk = np.asarray(inputs["bk"], np.float32)
    bv = np.asarray(inputs["bv"], np.float32)
    bo = np.asarray(inputs["bo"], np.float32)
    ag = np.asarray(inputs["attn_ln_g"], np.float32)
    ab = np.asarray(inputs["attn_ln_b"], np.float32)
    w1 = np.asarray(inputs["ffn_w1"], np.float32)
    b1 = np.asarray(inputs["ffn_b1"], np.float32)
    w2 = np.asarray(inputs["ffn_w2"], np.float32)
    b2 = np.asarray(inputs["ffn_b2"], np.float32)
    fg = np.asarray(inputs["ffn_ln_g"], np.float32)
    fb = np.asarray(inputs["ffn_ln_b"], np.float32)
    pw = np.asarray(inputs["pool_w"], np.float32)
    pb = np.asarray(inputs["pool_b"], np.float32)
    fw = np.asarray(inputs["final_w"], np.float32)
    fnb = np.asarray(inputs["final_b"], np.float32)

    def vec6(v):
        return np.ascontiguousarray(v.reshape(ND, P).T)

    scale = 1.0 / math.sqrt(DH)
    in_maps = []
    for c in range(8):
        ti, b, h = c // 4, (c // 2) % 2, c % 2
        src = e if ti == 0 else f
        s1 = f if ti == 0 else e
        own = slice(SH * h, SH * (h + 1))
        oth = slice(SH * (1 - h), SH * (2 - h))
        src_b = src[:, b, :]
        src_perm = np.concatenate([src_b[own], src_b[oth]], axis=0)
        m = {
            "srcT": np.ascontiguousarray(src_perm.T).astype(np.float16),
            "s1T": np.ascontiguousarray(s1[own, b, :].T).astype(np.float16),
            "pwT": np.ascontiguousarray(
                np.concatenate([pw[ti].T, pb[ti][None, :]], axis=0)
            ).astype(np.float16),
            "pbT": np.ascontiguousarray(pb[ti].reshape(NS, P).T),
            "finT": np.ascontiguousarray(fw[ti].T).astype(np.float16),
            "finb": vec6(fnb[ti]),
        }
        for li in (0, 1):
            vTp = np.zeros((D + 1, VW), np.float16)
            wvT = wv[ti, li].T
            for hh in range(H):
                vTp[0:D, (DH + 1) * hh : (DH + 1) * hh + DH] = wvT[
                    :, DH * hh : DH * (hh + 1)
                ]
                vTp[D, (DH + 1) * hh : (DH + 1) * hh + DH] = bv[
                    ti, li, DH * hh : DH * (hh + 1)
                ]
                vTp[D, (DH + 1) * hh + DH] = 1.0
            m.update(
                {
                    f"l{li}_qT": np.ascontiguousarray(wq[ti, li].T * scale).astype(np.float16),
                    f"l{li}_kT": np.ascontiguousarray(wk[ti, li].T).astype(np.float16),
                    f"l{li}_vTp": vTp,
                    f"l{li}_oT": np.ascontiguousarray(wo[ti, li].T).astype(np.float16),
                    f"l{li}_w1T": np.ascontiguousarray(w1[ti, li].T).astype(np.float16),
                    f"l{li}_w2T": np.ascontiguousarray(w2[ti, li].T).astype(np.float16),
                    f"l{li}_bq": vec6(bq[ti, li] * scale),
                    f"l{li}_bk": vec6(bk[ti, li]),
                    f"l{li}_bo": vec6(bo[ti, li]),
                    f"l{li}_b1": vec6(b1[ti, li]),
                    f"l{li}_b2": vec6(b2[ti, li]),
                    f"l{li}_ag": vec6(ag[ti, li]),
                    f"l{li}_ab": vec6(ab[ti, li]),
                    f"l{li}_fg": vec6(fg[ti, li]),
                    f"l{li}_fb": vec6(fb[ti, li]),
                }
            )
        in_maps.append(m)
    return in_maps


def get_program():
    if "nc" not in _BUILT:
        _BUILT["nc"] = _build_program()
    return _BUILT["nc"]


def kernel(**inputs):
    from concourse.bass_utils import run_bass_kernel_spmd

    nc = get_program()
    in_maps = _prep_inputs(inputs)
    res = run_bass_kernel_spmd(nc, in_maps, core_ids=list(range(8)))
    c_e_f = np.empty((S, B, D), np.float32)
    c_f_e = np.empty((S, B, D), np.float32)
    for c in range(8):
        ti, b, h = c // 4, (c // 2) % 2, c % 2
        dst = c_e_f if ti == 0 else c_f_e
        dst[SH * h : SH * (h + 1), b, :] = res.results[c]["outT"].T
    return c_e_f, c_f_e



# revision 24
# speedup vs baseline: 1.2178x; 1.0851x over previous
# Trainium2 Bass kernel for nn_Cross_Transformer (dense_transformer).
#
# Sharding: 8 cores = 2 towers x 2 batches x 2 sequence-halves.
# Each core computes block0 (self-attention) in full (its inputs are permuted
# so its own half leads, keeping the program SPMD-uniform), then its half of
# block1 (cross-attention), pool, and final projection. No collectives.
#
# Layout: activations are feature-major [D on partitions, S on free] so every
# projection is lhsT=W^T tiles (stationary) x X' (moving). Matmuls run as
# float32r (1 cyc/row at N>=256). Attention probabilities/V run in fp16.
# LayerNorm over D (= partitions) uses ones-column matmuls for sums and a
# K=1 ones-row matmul to broadcast per-column stats across partitions.

import math

import numpy as np

S = 1024
B = 2
D = 768
H = 12
DH = 64
EPS = 1e-6
SH = S // 2  # 512, per-core block1 rows
P = 128
ND = D // P  # 6 d-tiles
NS = S // P  # 8 s-tiles
VW = H * (DH + 1)  # 780: v row-major padded with a ones column per head
VWP = 784  # fp8 DoubleRow needs the st-pair stride 16B-aligned

F32 = None  # filled lazily (mybir.dt.float32)
_BUILT = {}


def _dt():
    from concourse import mybir

    return mybir.dt


def _r(ap):
    """View an fp32 AP as float32r for full-rate PE matmuls; fp16 passes through."""
    dt = _dt()
    return ap.bitcast(dt.float32r) if ap.dtype == dt.float32 else ap


def _build_program():
    import concourse.bacc as bacc
    import concourse.tile as tile
    from concourse import mybir
    from concourse.masks import make_identity

    dt = mybir.dt
    f32 = dt.float32
    f16 = dt.float16
    AF = mybir.ActivationFunctionType
    OP = mybir.AluOpType

    nc = bacc.Bacc("TRN2", target_bir_lowering=False, debug=False, num_devices=8)

    # ---- DRAM I/O ----
    din = {}

    def dram_in(name, shape, dty=None):
        din[name] = nc.dram_tensor(
            name, list(shape), dty or f16, kind="ExternalInput"
        )
        return din[name]

    f8 = mybir.dt.float8e4
    dram_in("srcT", (D, S))
    dram_in("srcT8", (D, S), f8)
    dram_in("s1T", (D, SH))
    dram_in("s1T8", (D, SH), f8)
    for li in (0, 1):
        dram_in(f"l{li}_qT8", (D, D), f8)
        dram_in(f"l{li}_kT8", (D, D), f8)
        dram_in(f"l{li}_vT8", (NS * P, VWP), f8)
        dram_in(f"l{li}_oT", (D, D))
        dram_in(f"l{li}_w1T", (D, D))
        dram_in(f"l{li}_w2T", (D, D))
        for bn in ("bq", "bk", "bo", "b1", "b2", "ag", "ab", "fg", "fb"):
            dram_in(f"l{li}_{bn}", (P, ND), f32)
    dram_in("pwT", (S + 1, S))
    dram_in("pbT", (P, NS), f32)
    dram_in("finT", (2 * D, D))
    dram_in("finb", (P, ND), f32)

    outT = nc.dram_tensor("outT", [D, SH], f32, kind="ExternalOutput")

    with tile.TileContext(nc) as tc:
        _emit(nc, tc, tile, dt, AF, OP, din, outT, make_identity)

    nc.compile()
    return nc


def _emit(nc, tc, tile, dt, AF, OP, din, outT, make_identity):
    f32 = dt.float32
    f16 = dt.float16
    f8 = dt.float8e4
    from concourse import mybir as _mb

    PM = _mb.MatmulPerfMode
    import contextlib

    es = contextlib.ExitStack()
    with es:
        persist = es.enter_context(tc.tile_pool(name="persist", bufs=1))
        wp = es.enter_context(tc.tile_pool(name="wp", bufs=9))
        w8p = es.enter_context(tc.tile_pool(name="w8p", bufs=3))
        psA = es.enter_context(tc.tile_pool(name="psA", bufs=2, space="PSUM"))
        psC = es.enter_context(tc.tile_pool(name="psC", bufs=1, space="PSUM"))
        psS = es.enter_context(tc.tile_pool(name="psS", bufs=2, space="PSUM"))
        expool = es.enter_context(tc.tile_pool(name="expool", bufs=4))
        ctxp = es.enter_context(tc.tile_pool(name="ctxp", bufs=2))
        sqp = es.enter_context(tc.tile_pool(name="sqp", bufs=4))
        brp = es.enter_context(tc.tile_pool(name="brp", bufs=2))
        smp = es.enter_context(tc.tile_pool(name="smp", bufs=2))

        # --- constants ---
        ident = persist.tile([P, P], f16, name="ident")
        make_identity(nc, ident)
        ones_col = persist.tile([P, 1], f16, name="ones_col")
        nc.vector.memset(ones_col, 1.0)
        ones_r64 = persist.tile([DH + 1, DH], f16, name="ones_r64")
        nc.vector.memset(ones_r64[DH : DH + 1, :], 1.0)
        ones_r128 = persist.tile([1, P], f16, name="ones_r128")
        nc.vector.memset(ones_r128, 1.0)

        # --- small params (biases, LN) ---
        par = {}
        for li in (0, 1):
            for bn in ("bq", "bk", "bo", "b1", "b2", "ag", "ab", "fg", "fb"):
                t = persist.tile([P, ND], f32, name=f"p_l{li}_{bn}")
                nc.gpsimd.dma_start(out=t, in_=din[f"l{li}_{bn}"].ap())
                par[f"l{li}_{bn}"] = t
        finb = persist.tile([P, ND], f32, name="p_finb")
        nc.gpsimd.dma_start(out=finb, in_=din["finb"].ap())

        # persistent activations
        FEATS = persist.tile([P, ND + 1, S], f16, name="FEATS")
        nc.vector.memset(FEATS[0:1, ND, :], 1.0)
        FEATSq = persist.tile([P, NS, S], f8, name="FEATSq")
        nc.vector.memset(FEATSq[:, ND : ND + 2, :], 0.0)
        nc.vector.memset(FEATSq[0:1, ND, :], 1.0)
        S1T = persist.tile([P, ND, SH], f16, name="S1T")
        nc.sync.dma_start(
            out=S1T, in_=din["s1T"].ap().rearrange("(j p) s -> p j s", p=P)
        )
        S1Tq = persist.tile([P, ND, SH], f8, name="S1Tq")
        nc.sync.dma_start(
            out=S1Tq, in_=din["s1T8"].ap().rearrange("(j p) s -> p j s", p=P)
        )
        # M_viewT [c-tile partitions, c-tile idx, r]: M_view[r, 128t+p] with
        # r = 3k+rho -> ctx1_ln[4k + u//6, d=128*(u%6)+p], u = 8*rho + t.
        # Written directly (strided) by block1's final LN.
        MVT = persist.tile([P, NS, 3 * P], f16, name="MVT")
        Q1 = persist.tile([P, ND, SH], f16, name="Q1")
        rb_t = persist.tile([P, S], f16, name="rb_t")
        mrb_t = persist.tile([P, S], f16, name="mrb_t")
        # LN small stats rows
        lnm = persist.tile([1, S], f32, name="lnm")
        lns2 = persist.tile([1, S], f32, name="lns2")
        lnt = persist.tile([1, S], f32, name="lnt")
        lnr16 = persist.tile([1, S], f16, name="lnr16")
        lnmr16 = persist.tile([1, S], f16, name="lnmr16")

        def load_w(dram_h, width, nk, tagsuf=""):
            """DMA weight k-tiles [P, width] (+ optional trailing [1, width])."""
            ap = dram_h.ap()
            tiles = []
            for t in range(nk):
                wt = wp.tile([P, width], f16, tag="w", name=f"w_{dram_h.name}_{t}")
                nc.sync.dma_start(out=wt, in_=ap[t * P : (t + 1) * P, :])
                tiles.append(wt)
            return tiles

        def fm_proj(x_ktiles, w_tiles, Sx, evac, extra_k=None):
            """Feature-major projection: out[m] = sum_k w[k][:,m].T @ x[k].
            x_ktiles: list of APs [kp, Sx]; w_tiles: list of APs [kp, D].
            evac(m, ps): consume psum [P, Sx]."""
            nch = Sx // 512
            ks = list(zip(x_ktiles, w_tiles))
            if extra_k is not None:
                ks.append(extra_k)
            for m in range(ND):
                ps = psA.tile([P, Sx], f32, tag="psA", name=f"ps_m{m}")
                for c in range(nch):
                    sl = slice(512 * c, 512 * (c + 1))
                    for ki, (xk, wk) in enumerate(ks):
                        nc.tensor.matmul(
                            ps[:, sl],
                            _r(wk[:, m * P : (m + 1) * P]),
                            _r(xk[:, sl]),
                            start=(ki == 0),
                            stop=(ki == len(ks) - 1),
                        )
                evac(m, ps)

        def load_w8(dram_h, nk, width):
            """One-DMA fp8 weight load: [P, nk, width] (k-tile planes)."""
            wt = w8p.tile([P, nk, width], f8, tag="w8", name=f"w8_{dram_h.name}")
            nc.sync.dma_start(
                out=wt, in_=dram_h.ap().rearrange("(t p) m -> p t m", p=P)
            )
            return wt

        def fm_proj8(Xq, w8, Sx, evac, mlist=None):
            """fp8 DoubleRow projection: contraction over 3 k-tile pairs.
            Xq [P, >=6, Sfull] fp8 planes; w8 [P, 6, D] fp8."""
            nch = Sx // 512
            for m in mlist if mlist is not None else range(ND):
                ps = psA.tile([P, Sx], f32, tag="psA", name=f"ps_m{m}")
                for c in range(nch):
                    sl = slice(512 * c, 512 * (c + 1))
                    for kp in range(3):
                        nc.tensor.matmul(
                            ps[:, sl],
                            w8[:, 2 * kp : 2 * kp + 2, m * P : (m + 1) * P],
                            Xq[:, 2 * kp : 2 * kp + 2, sl],
                            start=(kp == 0),
                            stop=(kp == 2),
                            perf_mode=PM.DoubleRow,
                        )
                evac(m, ps)

        def v_proj8(Xq8, vw8, Vt, li, stlist=None):
            """fp8 DoubleRow v-projection (keys-major, 4 plane-pairs: 6 data
            + ones/bias plane + zero plane)."""
            for st in stlist if stlist is not None else range(NS):
                ps = psA.tile([P, VWP], f32, tag="psA", name=f"vps{li}_{st}")
                ssl = slice(st * P, (st + 1) * P)
                for c0, c1 in ((0, 512), (512, VWP)):
                    for kp in range(4):
                        nc.tensor.matmul(
                            ps[:, c0:c1],
                            Xq8[:, 2 * kp : 2 * kp + 2, ssl],
                            vw8[:, 2 * kp : 2 * kp + 2, c0:c1],
                            start=(kp == 0),
                            stop=(kp == 3),
                            perf_mode=PM.DoubleRow,
                        )
                with nc.allow_low_precision(reason="fp8 V"):
                    nc.vector.tensor_copy(Vt[:, st, 0:VW], ps[:, 0:VW])

        def layernorm(Zt, Sx, g_t, b_t, out_fn):
            """LN over partitions(d) of Zt [P, ND, Sx] (Bessel std + eps).
            out_fn(k, c, src_ap, sl): writes result tile."""
            nch = Sx // 512
            for c in range(nch):
                sl = slice(512 * c, 512 * (c + 1))
                sum_ps = psS.tile([1, 512], f32, tag="psS", name=f"lnsum{c}")
                for k in range(ND):
                    nc.tensor.matmul(
                        sum_ps,
                        _r(ones_col),
                        _r(Zt[:, k, sl]),
                        start=(k == 0),
                        stop=(k == ND - 1),
                    )
                nc.scalar.activation(
                    lnm[:, sl], sum_ps, AF.Identity, scale=1.0 / D
                )
                sq_ps = psS.tile([1, 512], f32, tag="psS", name=f"lnsq{c}")
                for k in range(ND):
                    sq = sqp.tile([P, 512], f16, tag="sq", name=f"sq{k}{c}")
                    nc.vector.tensor_mul(sq, Zt[:, k, sl], Zt[:, k, sl])
                    nc.tensor.matmul(
                        sq_ps,
                        _r(ones_col),
                        _r(sq),
                        start=(k == 0),
                        stop=(k == ND - 1),
                    )
                nc.scalar.activation(
                    lns2[:, sl], sq_ps, AF.Identity, scale=1.0 / (D - 1)
                )
                # per-chunk stats chain so chunk 0 applies while chunk 1 sums
                nc.scalar.activation(
                    lnt[:, sl], lnm[:, sl], AF.Square,
                    scale=math.sqrt(D / (D - 1.0)),
                )
                nc.vector.tensor_sub(lns2[:, sl], lns2[:, sl], lnt[:, sl])
                nc.scalar.sqrt(lns2[:, sl], lns2[:, sl])
                nc.vector.tensor_scalar_add(lns2[:, sl], lns2[:, sl], EPS)
                nc.vector.reciprocal(lnt[:, sl], lns2[:, sl])  # r
                nc.vector.tensor_mul(lnm[:, sl], lnm[:, sl], lnt[:, sl])  # m*r
                nc.scalar.activation(lnr16[:, sl], lnt[:, sl], AF.Identity)
                nc.scalar.activation(lnmr16[:, sl], lnm[:, sl], AF.Identity)
            for c in range(nch):
                sl = slice(512 * c, 512 * (c + 1))
                rb_ps = psS.tile([P, 512], f32, tag="psS", name=f"rbps{c}")
                nc.tensor.matmul(
                    rb_ps, ones_r128, lnr16[0:1, sl], start=True, stop=True
                )
                nc.vector.tensor_copy(rb_t[:, sl], rb_ps)
                mrb_ps = psS.tile([P, 512], f32, tag="psS", name=f"mrbps{c}")
                nc.tensor.matmul(
                    mrb_ps, ones_r128, lnmr16[0:1, sl], start=True, stop=True
                )
                nc.vector.tensor_copy(mrb_t[:, sl], mrb_ps)
                for k in range(ND):
                    t1 = sqp.tile([P, 512], f16, tag="sq", name=f"ap{k}{c}")
                    nc.vector.tensor_mul(t1, Zt[:, k, sl], rb_t[:, sl])
                    nc.vector.tensor_sub(t1, t1, mrb_t[:, sl])
                    out_fn(k, c, t1, g_t, b_t, sl)

        def attention(Sq, Qt, Kt, Vt, ctx_sink):
            """ctx_sink(h, ap[64, Sq]) receives normalized per-head context.
            Probabilities and V run in fp8e4; P@V uses DoubleRow over
            st-pairs (256-row contraction at 0.5 cyc/row)."""
            nq = Sq // 512
            for h in range(H):
                hb = DH * (h % 2)
                j = h // 2
                ctx_ps = psC.tile([DH + 1, Sq], f32, tag="psC", name=f"ctx{h}")
                ex2 = None
                for st in range(NS):
                    sc_ps = psA.tile([P, Sq], f32, tag="psA", name=f"sc{h}_{st}")
                    for c in range(nq):
                        sl = slice(512 * c, 512 * (c + 1))
                        nc.tensor.matmul(
                            sc_ps[:, sl],
                            _r(Kt[hb : hb + DH, j, st * P : (st + 1) * P]),
                            _r(Qt[hb : hb + DH, j, sl]),
                            start=True,
                            stop=True,
                        )
                    if st % 2 == 0:
                        ex2 = expool.tile(
                            [P, 2, Sq], f8, tag="ex", name=f"ex{h}_{st // 2}"
                        )
                    with nc.allow_low_precision(reason="fp8 probs"):
                        nc.scalar.activation(ex2[:, st % 2, :], sc_ps, AF.Exp)
                    if st % 2 == 1:
                        sp = st // 2
                        for c in range(nq):
                            sl = slice(512 * c, 512 * (c + 1))
                            nc.tensor.matmul(
                                ctx_ps[:, sl],
                                Vt[:, 2 * sp : 2 * sp + 2,
                                   (DH + 1) * h : (DH + 1) * h + DH + 1],
                                ex2[:, :, sl],
                                start=(sp == 0),
                                stop=(sp == NS // 2 - 1),
                                perf_mode=PM.DoubleRow,
                            )
                rs = smp.tile([DH + 1, Sq], f16, tag="rs", name=f"rs{h}")
                with nc.allow_low_precision(reason="softmax recip fp16"):
                    nc.vector.reciprocal(
                        rs[DH : DH + 1, :], ctx_ps[DH : DH + 1, :]
                    )
                ctxn = ctxp.tile([DH, Sq], f16, tag="ctxh", name=f"ctxn{h}")
                for c in range(nq):
                    sl = slice(512 * c, 512 * (c + 1))
                    br_ps = psS.tile([DH, 512], f32, tag="psS", name=f"br{h}{c}")
                    nc.tensor.matmul(
                        br_ps,
                        ones_r64[DH : DH + 1, :],
                        rs[DH : DH + 1, sl],
                        start=True,
                        stop=True,
                    )
                    brc = brp.tile([DH, 512], f32, tag="brc", name=f"brc{h}{c}")
                    nc.vector.tensor_copy(brc, br_ps)
                    nc.vector.tensor_mul(ctxn[:, sl], ctx_ps[0:DH, sl], brc)
                ctx_sink(h, ctxn)

        # ================= BLOCK 0 (full S, self-attention on src) =========
        with tc.tile_pool(name="b0a", bufs=4) as act6, tc.tile_pool(
            name="b0x", bufs=1
        ) as act7, tc.tile_pool(name="b0v", bufs=1) as vp0:
            X0 = act7.tile([P, ND + 1, S], f16, tag="a7", name="X0")
            nc.sync.dma_start(
                out=X0[:, 0:ND, :],
                in_=din["srcT"].ap().rearrange("(j p) s -> p j s", p=P),
            )
            nc.vector.memset(X0[0:1, ND, :], 1.0)
            X0q = act7.tile([P, NS, S], f8, tag="x8", name="X0q")
            nc.sync.dma_start(
                out=X0q[:, 0:ND, :],
                in_=din["srcT8"].ap().rearrange("(j p) s -> p j s", p=P),
            )
            nc.vector.memset(X0q[:, ND : ND + 2, :], 0.0)
            nc.vector.memset(X0q[0:1, ND, :], 1.0)

            x_k = [X0[:, k, :] for k in range(ND)]
            V0 = vp0.tile([P, NS, VWP], f8, name="V0")

            # block1 q-projection depends only on inputs: emit first to fill
            # the startup bubble while block0 weights stream in.
            q1w8 = load_w8(din["l1_qT8"], ND, D)
            bq1 = par["l1_bq"]

            def ev_q1(m, ps):
                nc.vector.tensor_scalar_add(Q1[:, m, :], ps, bq1[:, m : m + 1])

            fm_proj8(S1Tq, q1w8, SH, ev_q1)

            def blk(li, Sq, Qsrc8, KVsrc8, resid_k, Vt, CTXa, Za, Zb,
                    ATT, OUTLN_fn):
                """One transformer block in feature-major layout."""
                pre = f"l{li}_"
                # --- q/k (feature-major, fp8 DoubleRow) ---
                qw8 = load_w8(din[pre + "qT8"], ND, D)
                Qt = CTXa["Q"]
                bq = par[pre + "bq"]

                def ev_q(m, ps):
                    nc.vector.tensor_scalar_add(Qt[:, m, 0:Sq], ps, bq[:, m : m + 1])

                fm_proj8(Qsrc8, qw8, Sq, ev_q)

                kw8 = load_w8(din[pre + "kT8"], ND, D)
                Kt = CTXa["K"]
                bk = par[pre + "bk"]

                def ev_k(m, ps):
                    nc.vector.tensor_scalar_add(Kt[:, m, :], ps, bk[:, m : m + 1])

                fm_proj8(KVsrc8, kw8, S, ev_k)

                # --- v (keys-major, fp8 DoubleRow, bias via plane 6/7) ---
                vw8 = load_w8(din[pre + "vT8"], NS, VWP)
                v_proj8(KVsrc8, vw8, Vt, li)

                # --- attention ---
                ow = load_w(din[pre + "oT"], D, ND)
                CTXh = CTXa["CTX"]

                def sink(h, ctxn):
                    hb = DH * (h % 2)
                    j = h // 2
                    nc.sync.dma_start(out=CTXh[hb : hb + DH, j, 0:Sq], in_=ctxn)

                attention(Sq, Qt, Kt, Vt, sink)

                # --- o-proj + bias + residual -> Za ---
                bo = par[pre + "bo"]

                def ev_o(m, ps):
                    for c in range(Sq // 512):
                        sl = slice(512 * c, 512 * (c + 1))
                        t = sqp.tile([P, 512], f16, tag="sq", name=f"oe{m}_{c}")
                        nc.scalar.activation(
                            t, ps[:, sl], AF.Identity, bias=bo[:, m : m + 1]
                        )
                        nc.vector.tensor_add(Za[:, m, sl], t, resid_k[m][:, sl])

                fm_proj([CTXh[:, k, 0:Sq] for k in range(ND)], ow, Sq, ev_o)

                # --- LN (attn) -> ATT ---
                ag, ab = par[pre + "ag"], par[pre + "ab"]

                def out_att(k, c, t1, g_t, b_t, sl):
                    nc.vector.tensor_scalar(
                        ATT[:, k, sl], t1, g_t[:, k : k + 1], b_t[:, k : k + 1],
                        OP.mult, OP.add,
                    )

                layernorm(Za, Sq, ag, ab, out_att)

                # --- ffn w1 + gelu ---
                w1 = load_w(din[pre + "w1T"], D, ND)
                H1 = CTXa["H1"]
                b1 = par[pre + "b1"]

                def ev_w1(m, ps):
                    nc.scalar.activation(
                        H1[:, m, 0:Sq], ps, AF.Gelu, bias=b1[:, m : m + 1]
                    )

                fm_proj([ATT[:, k, 0:Sq] for k in range(ND)], w1, Sq, ev_w1)

                # --- ffn w2 + bias + residual -> Zb, LN -> OUTLN_fn ---
                w2 = load_w(din[pre + "w2T"], D, ND)
                b2 = par[pre + "b2"]

                def ev_w2(m, ps):
                    for c in range(Sq // 512):
                        sl = slice(512 * c, 512 * (c + 1))
                        t = sqp.tile([P, 512], f16, tag="sq", name=f"w2e{m}_{c}")
                        nc.scalar.activation(
                            t, ps[:, sl], AF.Identity, bias=b2[:, m : m + 1]
                        )
                        nc.vector.tensor_add(Zb[:, m, sl], t, ATT[:, m, sl])

                fm_proj([H1[:, k, 0:Sq] for k in range(ND)], w2, Sq, ev_w2)

                fg, fb = par[pre + "fg"], par[pre + "fb"]
                layernorm(Zb, Sq, fg, fb, OUTLN_fn)

            # block0 tensor buffers (rotating in act6)
            Q0 = act6.tile([P, ND, S], f16, tag="a6", name="Q0")
            K0 = act6.tile([P, ND, S], f16, tag="a6", name="K0")
            CTX0 = act6.tile([P, ND, S], f16, tag="a6", name="CTX0")
            Z0a = act6.tile([P, ND, S], f16, tag="a6", name="Z0a")
            ATT0 = act6.tile([P, ND, S], f16, tag="a6", name="ATT0")
            H10 = act6.tile([P, ND, S], f16, tag="a6", name="H10")
            Z0b = act6.tile([P, ND, S], f16, tag="a6", name="Z0b")

            def out_feats(k, c, t1, g_t, b_t, sl):
                nc.vector.tensor_scalar(
                    FEATS[:, k, sl], t1, g_t[:, k : k + 1], b_t[:, k : k + 1],
                    OP.mult, OP.add,
                )
                with nc.allow_low_precision(reason="fp8 feats"):
                    nc.gpsimd.tensor_copy(FEATSq[:, k, sl], FEATS[:, k, sl])

            blk(
                0, S, X0q, X0q, x_k, V0,
                {"Q": Q0, "K": K0, "CTX": CTX0, "H1": H10},
                Z0a, Z0b, ATT0, out_feats,
            )

        # ================= BLOCK 1 (half S on q-side, cross-attention) ======
        with tc.tile_pool(name="b1a", bufs=4) as a6h, tc.tile_pool(
            name="b1b", bufs=1
        ) as a6f, tc.tile_pool(name="b1v", bufs=1) as vp1:
            feats_k = [FEATS[:, k, :] for k in range(ND)]
            s1_k = [S1T[:, k, :] for k in range(ND)]
            K1 = a6f.tile([P, ND, S], f16, tag="af", name="K1")
            CTX1 = a6h.tile([P, ND, SH], f16, tag="ah", name="CTX1")
            Z1a = a6h.tile([P, ND, SH], f16, tag="ah", name="Z1a")
            ATT1 = a6h.tile([P, ND, SH], f16, tag="ah", name="ATT1")
            H11 = a6h.tile([P, ND, SH], f16, tag="ah", name="H11")
            Z1b = a6h.tile([P, ND, SH], f16, tag="ah", name="Z1b")
            V1 = vp1.tile([P, NS, VWP], f8, name="V1")

            def out_ctx1(k, c, t1, g_t, b_t, sl):
                # write straight into M_viewT layout (4 strided slices)
                for s0 in range(4):
                    u = 6 * s0 + k
                    t_, rho = u % 8, u // 8
                    nc.vector.tensor_scalar(
                        MVT[:, t_, rho : 3 * P : 3], t1[:, s0 : SH : 4],
                        g_t[:, k : k + 1], b_t[:, k : k + 1],
                        OP.mult, OP.add,
                    )

            # reuse blk via closure over act6-> but buffers differ; inline call:
            # q from s1 (Sq=SH), k/v from feats (full S), residual = s1
            def blk1():
                pre = "l1_"
                kw8 = load_w8(din[pre + "kT8"], ND, D)
                bk = par[pre + "bk"]

                def ev_k(m, ps):
                    nc.vector.tensor_scalar_add(K1[:, m, :], ps, bk[:, m : m + 1])

                fm_proj8(FEATSq, kw8, S, ev_k)

                vw8 = load_w8(din[pre + "vT8"], NS, VWP)
                v_proj8(FEATSq, vw8, V1, 1)

                ow = load_w(din[pre + "oT"], D, ND)

                # head-pair attention: two heads share one [128,1024] psum
                for jp in range(ND):
                    ctx_ps = psC.tile(
                        [DH + 1, 2 * SH], f32, tag="psC", name=f"c1ps{jp}"
                    )
                    for st in range(NS):
                        sc_ps = psA.tile(
                            [P, 2 * SH], f32, tag="psA", name=f"s1ps{jp}_{st}"
                        )
                        for half in (0, 1):
                            hb = DH * half
                            sl = slice(SH * half, SH * (half + 1))
                            nc.tensor.matmul(
                                sc_ps[:, sl],
                                K1[hb : hb + DH, jp, st * P : (st + 1) * P],
                                Q1[hb : hb + DH, jp, :],
                                start=True,
                                stop=True,
                            )
                        if st % 2 == 0:
                            ex2 = expool.tile(
                                [P, 2, 2 * SH], f8, tag="ex",
                                name=f"e1x{jp}_{st // 2}",
                            )
                        with nc.allow_low_precision(reason="fp8 probs"):
                            nc.scalar.activation(ex2[:, st % 2, :], sc_ps, AF.Exp)
                        if st % 2 == 1:
                            sp = st // 2
                            for half in (0, 1):
                                h = 2 * jp + half
                                sl = slice(SH * half, SH * (half + 1))
                                nc.tensor.matmul(
                                    ctx_ps[:, sl],
                                    V1[:, 2 * sp : 2 * sp + 2,
                                       (DH + 1) * h : (DH + 1) * h + DH + 1],
                                    ex2[:, :, sl],
                                    start=(sp == 0),
                                    stop=(sp == NS // 2 - 1),
                                    perf_mode=PM.DoubleRow,
                                )
                    rs = smp.tile([DH + 1, 2 * SH], f16, tag="rs", name=f"r1s{jp}")
                    with nc.allow_low_precision(reason="softmax recip fp16"):
                        nc.vector.reciprocal(
                            rs[DH : DH + 1, :], ctx_ps[DH : DH + 1, :]
                        )
                    for half in (0, 1):
                        hb = DH * half
                        sl = slice(SH * half, SH * (half + 1))
                        br_ps = psS.tile(
                            [DH, 512], f32, tag="psS", name=f"b1r{jp}{half}"
                        )
                        nc.tensor.matmul(
                            br_ps,
                            ones_r64[DH : DH + 1, :],
                            rs[DH : DH + 1, sl],
                            start=True,
                            stop=True,
                        )
                        brc = brp.tile([DH, 512], f32, tag="brc", name=f"b1c{jp}{half}")
                        nc.vector.tensor_copy(brc, br_ps)
                        ctxn = ctxp.tile([DH, SH], f16, tag="ctxh", name=f"c1n{jp}{half}")
                        nc.vector.tensor_mul(ctxn, ctx_ps[0:DH, sl], brc)
                        nc.sync.dma_start(
                            out=CTX1[hb : hb + DH, jp, :], in_=ctxn
                        )

                bo = par[pre + "bo"]

                def ev_o(m, ps):
                    t = sqp.tile([P, 512], f16, tag="sq", name=f"o1e{m}")
                    nc.scalar.activation(
                        t, ps, AF.Identity, bias=bo[:, m : m + 1]
                    )
                    nc.vector.tensor_add(Z1a[:, m, :], t, S1T[:, m, :])

                fm_proj([CTX1[:, k, :] for k in range(ND)], ow, SH, ev_o)

                ag, ab = par[pre + "ag"], par[pre + "ab"]

                def out_att(k, c, t1, g_t, b_t, sl):
                    nc.vector.tensor_scalar(
                        ATT1[:, k, sl], t1, g_t[:, k : k + 1], b_t[:, k : k + 1],
                        OP.mult, OP.add,
                    )

                layernorm(Z1a, SH, ag, ab, out_att)

                w1 = load_w(din[pre + "w1T"], D, ND)
                b1 = par[pre + "b1"]

                def ev_w1(m, ps):
                    nc.scalar.activation(
                        H11[:, m, :], ps, AF.Gelu, bias=b1[:, m : m + 1]
                    )

                fm_proj([ATT1[:, k, :] for k in range(ND)], w1, SH, ev_w1)

                w2 = load_w(din[pre + "w2T"], D, ND)
                b2 = par[pre + "b2"]

                def ev_w2(m, ps):
                    t = sqp.tile([P, 512], f16, tag="sq", name=f"w21e{m}")
                    nc.scalar.activation(
                        t, ps, AF.Identity, bias=b2[:, m : m + 1]
                    )
                    nc.vector.tensor_add(Z1b[:, m, :], t, ATT1[:, m, :])

                fm_proj([H11[:, k, :] for k in range(ND)], w2, SH, ev_w2)

                fg, fb = par[pre + "fg"], par[pre + "fb"]
                layernorm(Z1b, SH, fg, fb, out_ctx1)

            blk1()

        # ================= POOL + FINAL =====================================
        with tc.tile_pool(name="late", bufs=2) as lp, tc.tile_pool(
            name="wbig", bufs=14
        ) as wb:
            # weight prefetch first: overlaps the whole pool chain
            pw_t = []
            for k in range(NS):
                t = wb.tile([P, S], f16, tag="wb", name=f"pw{k}")
                nc.sync.dma_start(out=t, in_=din["pwT"].ap()[k * P : (k + 1) * P, :])
                pw_t.append(t)
            pbT = wb.tile([P, NS], f32, tag="wb", name="pbT")
            nc.gpsimd.dma_start(out=pbT, in_=din["pbT"].ap())
            fin_t = []
            for k in range(2 * ND):
                t = wb.tile([P, D], f16, tag="wb", name=f"fin{k}")
                nc.sync.dma_start(
                    out=t, in_=din["finT"].ap()[k * P : (k + 1) * P, :]
                )
                fin_t.append(t)

            # pool matmul on M_viewT: poolT[128*jt+p, r] accumulated over
            # c-tiles t; evacuate straight into app^T layout via 3 strided
            # activations per jt (APPT[p, jd, sg+4k] = poolT[.., i+3k]).
            APPT = lp.tile([P, ND, SH], f16, tag="lt", name="APPT")
            for jt in range(NS):
                ps = psA.tile([P, 3 * P], f32, tag="psA", name=f"plps{jt}")
                for t in range(NS):
                    nc.tensor.matmul(
                        ps,
                        _r(pw_t[t][:, jt * P : (jt + 1) * P]),
                        _r(MVT[:, t, :]),
                        start=(t == 0),
                        stop=(t == NS - 1),
                    )
                for i in range(3):
                    u2 = 8 * i + jt
                    sg, jd = divmod(u2, 6)
                    nc.scalar.activation(
                        APPT[:, jd, sg : SH : 4],
                        ps[:, i : 3 * P : 3],
                        AF.Identity,
                        bias=pbT[:, jt : jt + 1],
                    )
            # final: out' = finT.T @ [feats_half ; app]
            OUTT = lp.tile([P, ND, SH], f32, tag="lt", name="OUTT")
            for m in range(ND):
                ps = psS.tile([P, SH], f32, tag="psS", name=f"fps{m}")
                for ki in range(2 * ND):
                    rhs = (
                        FEATS[:, ki, 0:SH]
                        if ki < ND
                        else APPT[:, ki - ND, :]
                    )
                    nc.tensor.matmul(
                        ps,
                        _r(fin_t[ki][:, m * P : (m + 1) * P]),
                        _r(rhs),
                        start=(ki == 0),
                        stop=(ki == 2 * ND - 1),
                    )
                nc.scalar.activation(
                    OUTT[:, m, :], ps, AF.Identity, bias=finb[:, m : m + 1]
                )
            nc.sync.dma_start(
                out=outT.ap().rearrange("(j p) s -> p j s", p=P), in_=OUTT
            )


def _q8(x):
    import ml_dtypes

    return np.ascontiguousarray(
        np.clip(np.asarray(x, np.float32), -240.0, 240.0)
    ).astype(ml_dtypes.float8_e4m3)


def _prep_inputs(inputs):
    e = np.ascontiguousarray(np.asarray(inputs["e"], dtype=np.float32))
    f = np.ascontiguousarray(np.asarray(inputs["f"], dtype=np.float32))
    wq = np.asarray(inputs["wq"], np.float32)
    wk = np.asarray(inputs["wk"], np.float32)
    wv = np.asarray(inputs["wv"], np.float32)
    wo = np.asarray(inputs["wo"], np.float32)
    bq = np.asarray(inputs["bq"], np.float32)
    bk = np.asarray(inputs["bk"], np.float32)
    bv = np.asarray(inputs["bv"], np.float32)
    bo = np.asarray(inputs["bo"], np.float32)
    ag = np.asarray(inputs["attn_ln_g"], np.float32)
    ab = np.asarray(inputs["attn_ln_b"], np.float32)
    w1 = np.asarray(inputs["ffn_w1"], np.float32)
    b1 = np.asarray(inputs["ffn_b1"], np.float32)
    w2 = np.asarray(inputs["ffn_w2"], np.float32)
    b2 = np.asarray(inputs["ffn_b2"], np.float32)
    fg = np.asarray(inputs["ffn_ln_g"], np.float32)
    fb = np.asarray(inputs["ffn_ln_b"], np.float32)
    pw = np.asarray(inputs["pool_w"], np.float32)
    pb = np.asarray(inputs["pool_b"], np.float32)
    fw = np.asarray(inputs["final_w"], np.float32)
    fnb = np.asarray(inputs["final_b"], np.float32)

    def vec6(v):
        return np.ascontiguousarray(v.reshape(ND, P).T)

    scale = 1.0 / math.sqrt(DH)
    in_maps = []
    for c in range(8):
        ti, b, h = c // 4, (c // 2) % 2, c % 2
        src = e if ti == 0 else f
        s1 = f if ti == 0 else e
        own = slice(SH * h, SH * (h + 1))
        oth = slice(SH * (1 - h), SH * (2 - h))
        src_b = src[:, b, :]
        src_perm = np.concatenate([src_b[own], src_b[oth]], axis=0)
        m = {
            "srcT": np.ascontiguousarray(src_perm.T).astype(np.float16),
            "srcT8": _q8(src_perm.T),
            "s1T": np.ascontiguousarray(s1[own, b, :].T).astype(np.float16),
            "s1T8": _q8(s1[own, b, :].T),
            "pwT": np.ascontiguousarray(
                np.concatenate([pw[ti].T, pb[ti][None, :]], axis=0)
            ).astype(np.float16),
            "pbT": np.ascontiguousarray(pb[ti].reshape(NS, P).T),
            "finT": np.ascontiguousarray(fw[ti].T).astype(np.float16),
            "finb": vec6(fnb[ti]),
        }
        for li in (0, 1):
            # vT8 planes: 0-5 = wv.T head-blocks, 6 = row0 bias/ones, 7 = 0
            vT8 = np.zeros((NS * P, VWP), np.float32)
            wvT = wv[ti, li].T
            for hh in range(H):
                vT8[0:D, (DH + 1) * hh : (DH + 1) * hh + DH] = wvT[
                    :, DH * hh : DH * (hh + 1)
                ]
                vT8[D, (DH + 1) * hh : (DH + 1) * hh + DH] = bv[
                    ti, li, DH * hh : DH * (hh + 1)
                ]
                vT8[D, (DH + 1) * hh + DH] = 1.0
            m.update(
                {
                    f"l{li}_qT8": _q8(wq[ti, li].T * scale),
                    f"l{li}_kT8": _q8(wk[ti, li].T),
                    f"l{li}_vT8": _q8(vT8),
                    f"l{li}_oT": np.ascontiguousarray(wo[ti, li].T).astype(np.float16),
                    f"l{li}_w1T": np.ascontiguousarray(w1[ti, li].T).astype(np.float16),
                    f"l{li}_w2T": np.ascontiguousarray(w2[ti, li].T).astype(np.float16),
                    f"l{li}_bq": vec6(bq[ti, li] * scale),
                    f"l{li}_bk": vec6(bk[ti, li]),
                    f"l{li}_bo": vec6(bo[ti, li]),
                    f"l{li}_b1": vec6(b1[ti, li]),
                    f"l{li}_b2": vec6(b2[ti, li]),
                    f"l{li}_ag": vec6(ag[ti, li]),
                    f"l{li}_ab": vec6(ab[ti, li]),
                    f"l{li}_fg": vec6(fg[ti, li]),
                    f"l{li}_fb": vec6(fb[ti, li]),
                }
            )
        in_maps.append(m)
    return in_maps


def get_program():
    if "nc" not in _BUILT:
        _BUILT["nc"] = _build_program()
    return _BUILT["nc"]


def kernel(**inputs):
    from concourse.bass_utils import run_bass_kernel_spmd

    nc = get_program()
    in_maps = _prep_inputs(inputs)
    res = run_bass_kernel_spmd(nc, in_maps, core_ids=list(range(8)))
    c_e_f = np.empty((S, B, D), np.float32)
    c_f_e = np.empty((S, B, D), np.float32)
    for c in range(8):
        ti, b, h = c // 4, (c // 2) % 2, c % 2
        dst = c_e_f if ti == 0 else c_f_e
        dst[SH * h : SH * (h + 1), b, :] = res.results[c]["outT"].T
    return c_e_f, c_f_e



# revision 26
# speedup vs baseline: 1.3175x; 1.0819x over previous
# Trainium2 Bass kernel for nn_Cross_Transformer (dense_transformer).
#
# Sharding: 8 cores = 2 towers x 2 batches x 2 sequence-halves.
# Each core computes block0 (self-attention) in full (its inputs are permuted
# so its own half leads, keeping the program SPMD-uniform), then its half of
# block1 (cross-attention), pool, and final projection. No collectives.
#
# Layout: activations are feature-major [D on partitions, S on free] so every
# projection is lhsT=W^T tiles (stationary) x X' (moving). Matmuls run as
# float32r (1 cyc/row at N>=256). Attention probabilities/V run in fp16.
# LayerNorm over D (= partitions) uses ones-column matmuls for sums and a
# K=1 ones-row matmul to broadcast per-column stats across partitions.

import math

import numpy as np

S = 1024
B = 2
D = 768
H = 12
DH = 64
EPS = 1e-6
SH = S // 2  # 512, per-core block1 rows
P = 128
ND = D // P  # 6 d-tiles
NS = S // P  # 8 s-tiles
VW = H * (DH + 1)  # 780: v row-major padded with a ones column per head
VWP = 784  # fp8 DoubleRow needs the st-pair stride 16B-aligned

F32 = None  # filled lazily (mybir.dt.float32)
_BUILT = {}


def _dt():
    from concourse import mybir

    return mybir.dt


def _r(ap):
    """View an fp32 AP as float32r for full-rate PE matmuls; fp16 passes through."""
    dt = _dt()
    return ap.bitcast(dt.float32r) if ap.dtype == dt.float32 else ap


def _build_program():
    import concourse.bacc as bacc
    import concourse.tile as tile
    from concourse import mybir
    from concourse.masks import make_identity

    dt = mybir.dt
    f32 = dt.float32
    f16 = dt.float16
    AF = mybir.ActivationFunctionType
    OP = mybir.AluOpType

    nc = bacc.Bacc("TRN2", target_bir_lowering=False, debug=False, num_devices=8)

    # ---- DRAM I/O ----
    din = {}

    def dram_in(name, shape, dty=None):
        din[name] = nc.dram_tensor(
            name, list(shape), dty or f16, kind="ExternalInput"
        )
        return din[name]

    f8 = mybir.dt.float8e4
    dram_in("srcT", (D, S))
    dram_in("srcT8", (D, S), f8)
    dram_in("s1T", (D, SH))
    dram_in("s1T8", (D, SH), f8)
    for li in (0, 1):
        dram_in(f"l{li}_qT8", (D, D), f8)
        dram_in(f"l{li}_kT8", (D, D), f8)
        dram_in(f"l{li}_vT8", (NS * P, VWP), f8)
        dram_in(f"l{li}_oT8", (D, D), f8)
        dram_in(f"l{li}_w1T", (D, D))
        dram_in(f"l{li}_w2T8", (D, D), f8)
        for bn in ("bq", "bk", "bo", "b1", "b2", "ag", "ab", "fg", "fb"):
            dram_in(f"l{li}_{bn}", (P, ND), f32)
    dram_in("pwT", (S + 1, S))
    dram_in("pbT", (P, NS), f32)
    dram_in("finT", (2 * D, D))
    dram_in("finb", (P, ND), f32)

    outT = nc.dram_tensor("outT", [D, SH], f32, kind="ExternalOutput")

    with tile.TileContext(nc) as tc:
        _emit(nc, tc, tile, dt, AF, OP, din, outT, make_identity)

    nc.compile()
    return nc


def _emit(nc, tc, tile, dt, AF, OP, din, outT, make_identity):
    f32 = dt.float32
    f16 = dt.float16
    f8 = dt.float8e4
    from concourse import mybir as _mb

    PM = _mb.MatmulPerfMode
    import contextlib

    es = contextlib.ExitStack()
    with es:
        persist = es.enter_context(tc.tile_pool(name="persist", bufs=1))
        wp = es.enter_context(tc.tile_pool(name="wp", bufs=9))
        w8p = es.enter_context(tc.tile_pool(name="w8p", bufs=3))
        psA = es.enter_context(tc.tile_pool(name="psA", bufs=2, space="PSUM"))
        psC = es.enter_context(tc.tile_pool(name="psC", bufs=1, space="PSUM"))
        psS = es.enter_context(tc.tile_pool(name="psS", bufs=2, space="PSUM"))
        expool = es.enter_context(tc.tile_pool(name="expool", bufs=4))
        ctxp = es.enter_context(tc.tile_pool(name="ctxp", bufs=2))
        sqp = es.enter_context(tc.tile_pool(name="sqp", bufs=4))
        brp = es.enter_context(tc.tile_pool(name="brp", bufs=2))
        smp = es.enter_context(tc.tile_pool(name="smp", bufs=2))

        # --- constants ---
        ident = persist.tile([P, P], f16, name="ident")
        make_identity(nc, ident)
        ones_col = persist.tile([P, 1], f16, name="ones_col")
        nc.vector.memset(ones_col, 1.0)
        ones_r64 = persist.tile([DH + 1, DH], f16, name="ones_r64")
        nc.vector.memset(ones_r64[DH : DH + 1, :], 1.0)
        ones_r128 = persist.tile([1, P], f16, name="ones_r128")
        nc.vector.memset(ones_r128, 1.0)

        # --- small params (biases, LN) ---
        par = {}
        for li in (0, 1):
            for bn in ("bq", "bk", "bo", "b1", "b2", "ag", "ab", "fg", "fb"):
                t = persist.tile([P, ND], f32, name=f"p_l{li}_{bn}")
                nc.gpsimd.dma_start(out=t, in_=din[f"l{li}_{bn}"].ap())
                par[f"l{li}_{bn}"] = t
        finb = persist.tile([P, ND], f32, name="p_finb")
        nc.gpsimd.dma_start(out=finb, in_=din["finb"].ap())

        # persistent activations
        FEATS = persist.tile([P, ND + 1, S], f16, name="FEATS")
        nc.vector.memset(FEATS[0:1, ND, :], 1.0)
        FEATSq = persist.tile([P, NS, S], f8, name="FEATSq")
        nc.vector.memset(FEATSq[:, ND : ND + 2, :], 0.0)
        nc.vector.memset(FEATSq[0:1, ND, :], 1.0)
        S1T = persist.tile([P, ND, SH], f16, name="S1T")
        nc.sync.dma_start(
            out=S1T, in_=din["s1T"].ap().rearrange("(j p) s -> p j s", p=P)
        )
        S1Tq = persist.tile([P, ND, SH], f8, name="S1Tq")
        nc.sync.dma_start(
            out=S1Tq, in_=din["s1T8"].ap().rearrange("(j p) s -> p j s", p=P)
        )
        # M_viewT [c-tile partitions, c-tile idx, r]: M_view[r, 128t+p] with
        # r = 3k+rho -> ctx1_ln[4k + u//6, d=128*(u%6)+p], u = 8*rho + t.
        # Written directly (strided) by block1's final LN.
        MVT = persist.tile([P, NS, 3 * P], f16, name="MVT")
        Q1 = persist.tile([P, ND, SH], f16, name="Q1")
        rb_t = persist.tile([P, S], f16, name="rb_t")
        mrb_t = persist.tile([P, S], f16, name="mrb_t")
        # LN small stats rows
        lnm = persist.tile([1, S], f32, name="lnm")
        lns2 = persist.tile([1, S], f32, name="lns2")
        lnt = persist.tile([1, S], f32, name="lnt")
        lnr16 = persist.tile([1, S], f16, name="lnr16")
        lnmr16 = persist.tile([1, S], f16, name="lnmr16")

        def load_w(dram_h, width, nk, tagsuf=""):
            """DMA weight k-tiles [P, width] (+ optional trailing [1, width])."""
            ap = dram_h.ap()
            tiles = []
            for t in range(nk):
                wt = wp.tile([P, width], f16, tag="w", name=f"w_{dram_h.name}_{t}")
                nc.sync.dma_start(out=wt, in_=ap[t * P : (t + 1) * P, :])
                tiles.append(wt)
            return tiles

        def fm_proj(x_ktiles, w_tiles, Sx, evac, extra_k=None):
            """Feature-major projection: out[m] = sum_k w[k][:,m].T @ x[k].
            x_ktiles: list of APs [kp, Sx]; w_tiles: list of APs [kp, D].
            evac(m, ps): consume psum [P, Sx]."""
            nch = Sx // 512
            ks = list(zip(x_ktiles, w_tiles))
            if extra_k is not None:
                ks.append(extra_k)
            for m in range(ND):
                ps = psA.tile([P, Sx], f32, tag="psA", name=f"ps_m{m}")
                for c in range(nch):
                    sl = slice(512 * c, 512 * (c + 1))
                    for ki, (xk, wk) in enumerate(ks):
                        nc.tensor.matmul(
                            ps[:, sl],
                            _r(wk[:, m * P : (m + 1) * P]),
                            _r(xk[:, sl]),
                            start=(ki == 0),
                            stop=(ki == len(ks) - 1),
                        )
                evac(m, ps)

        def load_w8(dram_h, nk, width):
            """One-DMA fp8 weight load: [P, nk, width] (k-tile planes)."""
            wt = w8p.tile([P, nk, width], f8, tag="w8", name=f"w8_{dram_h.name}")
            nc.sync.dma_start(
                out=wt, in_=dram_h.ap().rearrange("(t p) m -> p t m", p=P)
            )
            return wt

        def fm_proj8(Xq, w8, Sx, evac, mlist=None):
            """fp8 DoubleRow projection: contraction over 3 k-tile pairs.
            Xq [P, >=6, Sfull] fp8 planes; w8 [P, 6, D] fp8."""
            nch = Sx // 512
            for m in mlist if mlist is not None else range(ND):
                ps = psA.tile([P, Sx], f32, tag="psA", name=f"ps_m{m}")
                for c in range(nch):
                    sl = slice(512 * c, 512 * (c + 1))
                    for kp in range(3):
                        nc.tensor.matmul(
                            ps[:, sl],
                            w8[:, 2 * kp : 2 * kp + 2, m * P : (m + 1) * P],
                            Xq[:, 2 * kp : 2 * kp + 2, sl],
                            start=(kp == 0),
                            stop=(kp == 2),
                            perf_mode=PM.DoubleRow,
                        )
                evac(m, ps)

        def v_proj8(Xq8, vw8, Vt, li, stlist=None):
            """fp8 DoubleRow v-projection (keys-major, 4 plane-pairs: 6 data
            + ones/bias plane + zero plane)."""
            for st in stlist if stlist is not None else range(NS):
                ps = psA.tile([P, VWP], f32, tag="psA", name=f"vps{li}_{st}")
                ssl = slice(st * P, (st + 1) * P)
                for c0, c1 in ((0, 512), (512, VWP)):
                    for kp in range(4):
                        nc.tensor.matmul(
                            ps[:, c0:c1],
                            Xq8[:, 2 * kp : 2 * kp + 2, ssl],
                            vw8[:, 2 * kp : 2 * kp + 2, c0:c1],
                            start=(kp == 0),
                            stop=(kp == 3),
                            perf_mode=PM.DoubleRow,
                        )
                with nc.allow_low_precision(reason="fp8 V"):
                    nc.vector.tensor_copy(Vt[:, st, 0:VW], ps[:, 0:VW])

        def layernorm(Zt, Sx, g_t, b_t, out_fn):
            """LN over partitions(d) of Zt [P, ND, Sx] (Bessel std + eps).
            out_fn(k, c, src_ap, sl): writes result tile."""
            nch = Sx // 512
            for c in range(nch):
                sl = slice(512 * c, 512 * (c + 1))
                sum_ps = psS.tile([1, 512], f32, tag="psS", name=f"lnsum{c}")
                for k in range(ND):
                    nc.tensor.matmul(
                        sum_ps,
                        _r(ones_col),
                        _r(Zt[:, k, sl]),
                        start=(k == 0),
                        stop=(k == ND - 1),
                    )
                nc.scalar.activation(
                    lnm[:, sl], sum_ps, AF.Identity, scale=1.0 / D
                )
                sq_ps = psS.tile([1, 512], f32, tag="psS", name=f"lnsq{c}")
                for k in range(ND):
                    sq = sqp.tile([P, 512], f16, tag="sq", name=f"sq{k}{c}")
                    nc.vector.tensor_mul(sq, Zt[:, k, sl], Zt[:, k, sl])
                    nc.tensor.matmul(
                        sq_ps,
                        _r(ones_col),
                        _r(sq),
                        start=(k == 0),
                        stop=(k == ND - 1),
                    )
                nc.scalar.activation(
                    lns2[:, sl], sq_ps, AF.Identity, scale=1.0 / (D - 1)
                )
                # per-chunk stats chain so chunk 0 applies while chunk 1 sums
                nc.scalar.activation(
                    lnt[:, sl], lnm[:, sl], AF.Square,
                    scale=math.sqrt(D / (D - 1.0)),
                )
                nc.vector.tensor_sub(lns2[:, sl], lns2[:, sl], lnt[:, sl])
                nc.scalar.sqrt(lns2[:, sl], lns2[:, sl])
                nc.vector.tensor_scalar_add(lns2[:, sl], lns2[:, sl], EPS)
                nc.vector.reciprocal(lnt[:, sl], lns2[:, sl])  # r
                nc.vector.tensor_mul(lnm[:, sl], lnm[:, sl], lnt[:, sl])  # m*r
                nc.scalar.activation(lnr16[:, sl], lnt[:, sl], AF.Identity)
                nc.scalar.activation(lnmr16[:, sl], lnm[:, sl], AF.Identity)
            for c in range(nch):
                sl = slice(512 * c, 512 * (c + 1))
                rb_ps = psS.tile([P, 512], f32, tag="psS", name=f"rbps{c}")
                nc.tensor.matmul(
                    rb_ps, ones_r128, lnr16[0:1, sl], start=True, stop=True
                )
                nc.vector.tensor_copy(rb_t[:, sl], rb_ps)
                mrb_ps = psS.tile([P, 512], f32, tag="psS", name=f"mrbps{c}")
                nc.tensor.matmul(
                    mrb_ps, ones_r128, lnmr16[0:1, sl], start=True, stop=True
                )
                nc.vector.tensor_copy(mrb_t[:, sl], mrb_ps)
                for k in range(ND):
                    t1 = sqp.tile([P, 512], f16, tag="sq", name=f"ap{k}{c}")
                    nc.vector.tensor_mul(t1, Zt[:, k, sl], rb_t[:, sl])
                    nc.vector.tensor_sub(t1, t1, mrb_t[:, sl])
                    out_fn(k, c, t1, g_t, b_t, sl)

        def attention(Sq, Qt, Kt, Vt, ctx_sink):
            """ctx_sink(h, ap[64, Sq]) receives normalized per-head context.
            Probabilities and V run in fp8e4; P@V uses DoubleRow over
            st-pairs (256-row contraction at 0.5 cyc/row)."""
            nq = Sq // 512
            for h in range(H):
                hb = DH * (h % 2)
                j = h // 2
                ctx_ps = psC.tile([DH + 1, Sq], f32, tag="psC", name=f"ctx{h}")
                ex2 = None
                for st in range(NS):
                    sc_ps = psA.tile([P, Sq], f32, tag="psA", name=f"sc{h}_{st}")
                    for c in range(nq):
                        sl = slice(512 * c, 512 * (c + 1))
                        nc.tensor.matmul(
                            sc_ps[:, sl],
                            _r(Kt[hb : hb + DH, j, st * P : (st + 1) * P]),
                            _r(Qt[hb : hb + DH, j, sl]),
                            start=True,
                            stop=True,
                        )
                    if st % 2 == 0:
                        ex2 = expool.tile(
                            [P, 2, Sq], f8, tag="ex", name=f"ex{h}_{st // 2}"
                        )
                    with nc.allow_low_precision(reason="fp8 probs"):
                        nc.scalar.activation(ex2[:, st % 2, :], sc_ps, AF.Exp)
                    if st % 2 == 1:
                        sp = st // 2
                        for c in range(nq):
                            sl = slice(512 * c, 512 * (c + 1))
                            nc.tensor.matmul(
                                ctx_ps[:, sl],
                                Vt[:, 2 * sp : 2 * sp + 2,
                                   (DH + 1) * h : (DH + 1) * h + DH + 1],
                                ex2[:, :, sl],
                                start=(sp == 0),
                                stop=(sp == NS // 2 - 1),
                                perf_mode=PM.DoubleRow,
                            )
                rs = smp.tile([DH + 1, Sq], f16, tag="rs", name=f"rs{h}")
                with nc.allow_low_precision(reason="softmax recip fp16"):
                    nc.vector.reciprocal(
                        rs[DH : DH + 1, :], ctx_ps[DH : DH + 1, :]
                    )
                ctxn = ctxp.tile([DH, Sq], f8, tag="ctxh", name=f"ctxn{h}")
                for c in range(nq):
                    sl = slice(512 * c, 512 * (c + 1))
                    br_ps = psS.tile([DH, 512], f32, tag="psS", name=f"br{h}{c}")
                    nc.tensor.matmul(
                        br_ps,
                        ones_r64[DH : DH + 1, :],
                        rs[DH : DH + 1, sl],
                        start=True,
                        stop=True,
                    )
                    brc = brp.tile([DH, 512], f32, tag="brc", name=f"brc{h}{c}")
                    nc.vector.tensor_copy(brc, br_ps)
                    nc.vector.tensor_mul(ctxn[:, sl], ctx_ps[0:DH, sl], brc)
                ctx_sink(h, ctxn)

        # ================= BLOCK 0 (full S, self-attention on src) =========
        with tc.tile_pool(name="b0a", bufs=4) as act6, tc.tile_pool(
            name="b0x", bufs=1
        ) as act7, tc.tile_pool(name="b0v", bufs=1) as vp0:
            X0 = act7.tile([P, ND + 1, S], f16, tag="a7", name="X0")
            nc.sync.dma_start(
                out=X0[:, 0:ND, :],
                in_=din["srcT"].ap().rearrange("(j p) s -> p j s", p=P),
            )
            nc.vector.memset(X0[0:1, ND, :], 1.0)
            X0q = act7.tile([P, NS, S], f8, tag="x8", name="X0q")
            nc.sync.dma_start(
                out=X0q[:, 0:ND, :],
                in_=din["srcT8"].ap().rearrange("(j p) s -> p j s", p=P),
            )
            nc.vector.memset(X0q[:, ND : ND + 2, :], 0.0)
            nc.vector.memset(X0q[0:1, ND, :], 1.0)

            x_k = [X0[:, k, :] for k in range(ND)]
            V0 = vp0.tile([P, NS, VWP], f8, name="V0")

            # block1 q-projection depends only on inputs: emit first to fill
            # the startup bubble while block0 weights stream in.
            q1w8 = load_w8(din["l1_qT8"], ND, D)
            bq1 = par["l1_bq"]

            def ev_q1(m, ps):
                nc.vector.tensor_scalar_add(Q1[:, m, :], ps, bq1[:, m : m + 1])

            fm_proj8(S1Tq, q1w8, SH, ev_q1)

            def blk(li, Sq, Qsrc8, KVsrc8, resid_k, Vt, CTXa, Za, Zb,
                    ATT, OUTLN_fn):
                """One transformer block in feature-major layout."""
                pre = f"l{li}_"
                # --- q/k (feature-major, fp8 DoubleRow) ---
                qw8 = load_w8(din[pre + "qT8"], ND, D)
                Qt = CTXa["Q"]
                bq = par[pre + "bq"]

                def ev_q(m, ps):
                    nc.vector.tensor_scalar_add(Qt[:, m, 0:Sq], ps, bq[:, m : m + 1])

                fm_proj8(Qsrc8, qw8, Sq, ev_q)

                kw8 = load_w8(din[pre + "kT8"], ND, D)
                Kt = CTXa["K"]
                bk = par[pre + "bk"]

                def ev_k(m, ps):
                    nc.vector.tensor_scalar_add(Kt[:, m, :], ps, bk[:, m : m + 1])

                fm_proj8(KVsrc8, kw8, S, ev_k)

                # --- v (keys-major, fp8 DoubleRow, bias via plane 6/7) ---
                vw8 = load_w8(din[pre + "vT8"], NS, VWP)
                v_proj8(KVsrc8, vw8, Vt, li)

                # --- attention ---
                ow8 = load_w8(din[pre + "oT8"], ND, D)
                CTXh = CTXa["CTX"]

                def sink(h, ctxn):
                    hb = DH * (h % 2)
                    j = h // 2
                    nc.sync.dma_start(out=CTXh[hb : hb + DH, j, 0:Sq], in_=ctxn)

                attention(Sq, Qt, Kt, Vt, sink)

                # --- o-proj + bias + residual -> Za ---
                bo = par[pre + "bo"]

                def ev_o(m, ps):
                    for c in range(Sq // 512):
                        sl = slice(512 * c, 512 * (c + 1))
                        t = sqp.tile([P, 512], f16, tag="sq", name=f"oe{m}_{c}")
                        nc.scalar.activation(
                            t, ps[:, sl], AF.Identity, bias=bo[:, m : m + 1]
                        )
                        nc.vector.tensor_add(Za[:, m, sl], t, resid_k[m][:, sl])

                fm_proj8(CTXh, ow8, Sq, ev_o)

                # --- LN (attn) -> ATT ---
                ag, ab = par[pre + "ag"], par[pre + "ab"]

                def out_att(k, c, t1, g_t, b_t, sl):
                    nc.vector.tensor_scalar(
                        ATT[:, k, sl], t1, g_t[:, k : k + 1], b_t[:, k : k + 1],
                        OP.mult, OP.add,
                    )

                layernorm(Za, Sq, ag, ab, out_att)

                # --- ffn w1 + gelu ---
                w1 = load_w(din[pre + "w1T"], D, ND)
                H1 = CTXa["H1"]
                b1 = par[pre + "b1"]

                def ev_w1(m, ps):
                    nc.scalar.activation(
                        H1[:, m, 0:Sq], ps, AF.Gelu, bias=b1[:, m : m + 1]
                    )

                fm_proj([ATT[:, k, 0:Sq] for k in range(ND)], w1, Sq, ev_w1)

                # --- ffn w2 + bias + residual -> Zb, LN -> OUTLN_fn ---
                w28 = load_w8(din[pre + "w2T8"], ND, D)
                b2 = par[pre + "b2"]

                def ev_w2(m, ps):
                    for c in range(Sq // 512):
                        sl = slice(512 * c, 512 * (c + 1))
                        t = sqp.tile([P, 512], f16, tag="sq", name=f"w2e{m}_{c}")
                        nc.scalar.activation(
                            t, ps[:, sl], AF.Identity, bias=b2[:, m : m + 1]
                        )
                        nc.vector.tensor_add(Zb[:, m, sl], t, ATT[:, m, sl])

                fm_proj8(H1, w28, Sq, ev_w2)

                fg, fb = par[pre + "fg"], par[pre + "fb"]
                layernorm(Zb, Sq, fg, fb, OUTLN_fn)

            # block0 tensor buffers (rotating in act6)
            Q0 = act6.tile([P, ND, S], f16, tag="a6", name="Q0")
            K0 = act6.tile([P, ND, S], f16, tag="a6", name="K0")
            CTX0 = act6.tile([P, ND, S], f8, tag="a6", name="CTX0")
            Z0a = act6.tile([P, ND, S], f16, tag="a6", name="Z0a")
            ATT0 = act6.tile([P, ND, S], f16, tag="a6", name="ATT0")
            H10 = act6.tile([P, ND, S], f8, tag="a6", name="H10")
            Z0b = act6.tile([P, ND, S], f16, tag="a6", name="Z0b")

            def out_feats(k, c, t1, g_t, b_t, sl):
                nc.vector.tensor_scalar(
                    FEATS[:, k, sl], t1, g_t[:, k : k + 1], b_t[:, k : k + 1],
                    OP.mult, OP.add,
                )
                with nc.allow_low_precision(reason="fp8 feats"):
                    nc.gpsimd.tensor_copy(FEATSq[:, k, sl], FEATS[:, k, sl])

            blk(
                0, S, X0q, X0q, x_k, V0,
                {"Q": Q0, "K": K0, "CTX": CTX0, "H1": H10},
                Z0a, Z0b, ATT0, out_feats,
            )

        # ================= BLOCK 1 (half S on q-side, cross-attention) ======
        with tc.tile_pool(name="b1a", bufs=4) as a6h, tc.tile_pool(
            name="b1b", bufs=1
        ) as a6f, tc.tile_pool(name="b1v", bufs=1) as vp1:
            feats_k = [FEATS[:, k, :] for k in range(ND)]
            s1_k = [S1T[:, k, :] for k in range(ND)]
            K1 = a6f.tile([P, ND, S], f16, tag="af", name="K1")
            CTX1 = a6h.tile([P, ND, SH], f8, tag="ah", name="CTX1")
            Z1a = a6h.tile([P, ND, SH], f16, tag="ah", name="Z1a")
            ATT1 = a6h.tile([P, ND, SH], f16, tag="ah", name="ATT1")
            H11 = a6h.tile([P, ND, SH], f8, tag="ah", name="H11")
            Z1b = a6h.tile([P, ND, SH], f16, tag="ah", name="Z1b")
            V1 = vp1.tile([P, NS, VWP], f8, name="V1")

            def out_ctx1(k, c, t1, g_t, b_t, sl):
                # write straight into M_viewT layout (4 strided slices)
                for s0 in range(4):
                    u = 6 * s0 + k
                    t_, rho = u % 8, u // 8
                    nc.vector.tensor_scalar(
                        MVT[:, t_, rho : 3 * P : 3], t1[:, s0 : SH : 4],
                        g_t[:, k : k + 1], b_t[:, k : k + 1],
                        OP.mult, OP.add,
                    )

            # reuse blk via closure over act6-> but buffers differ; inline call:
            # q from s1 (Sq=SH), k/v from feats (full S), residual = s1
            def blk1():
                pre = "l1_"
                kw8 = load_w8(din[pre + "kT8"], ND, D)
                bk = par[pre + "bk"]

                def ev_k(m, ps):
                    nc.vector.tensor_scalar_add(K1[:, m, :], ps, bk[:, m : m + 1])

                fm_proj8(FEATSq, kw8, S, ev_k)

                vw8 = load_w8(din[pre + "vT8"], NS, VWP)
                v_proj8(FEATSq, vw8, V1, 1)

                ow8 = load_w8(din[pre + "oT8"], ND, D)

                # head-pair attention: two heads share one [128,1024] psum
                for jp in range(ND):
                    ctx_ps = psC.tile(
                        [DH + 1, 2 * SH], f32, tag="psC", name=f"c1ps{jp}"
                    )
                    for st in range(NS):
                        sc_ps = psA.tile(
                            [P, 2 * SH], f32, tag="psA", name=f"s1ps{jp}_{st}"
                        )
                        for half in (0, 1):
                            hb = DH * half
                            sl = slice(SH * half, SH * (half + 1))
                            nc.tensor.matmul(
                                sc_ps[:, sl],
                                K1[hb : hb + DH, jp, st * P : (st + 1) * P],
                                Q1[hb : hb + DH, jp, :],
                                start=True,
                                stop=True,
                            )
                        if st % 2 == 0:
                            ex2 = expool.tile(
                                [P, 2, 2 * SH], f8, tag="ex",
                                name=f"e1x{jp}_{st // 2}",
                            )
                        with nc.allow_low_precision(reason="fp8 probs"):
                            nc.scalar.activation(ex2[:, st % 2, :], sc_ps, AF.Exp)
                        if st % 2 == 1:
                            sp = st // 2
                            for half in (0, 1):
                                h = 2 * jp + half
                                sl = slice(SH * half, SH * (half + 1))
                                nc.tensor.matmul(
                                    ctx_ps[:, sl],
                                    V1[:, 2 * sp : 2 * sp + 2,
                                       (DH + 1) * h : (DH + 1) * h + DH + 1],
                                    ex2[:, :, sl],
                                    start=(sp == 0),
                                    stop=(sp == NS // 2 - 1),
                                    perf_mode=PM.DoubleRow,
                                )
                    rs = smp.tile([DH + 1, 2 * SH], f16, tag="rs", name=f"r1s{jp}")
                    with nc.allow_low_precision(reason="softmax recip fp16"):
                        nc.vector.reciprocal(
                            rs[DH : DH + 1, :], ctx_ps[DH : DH + 1, :]
                        )
                    for half in (0, 1):
                        hb = DH * half
                        sl = slice(SH * half, SH * (half + 1))
                        br_ps = psS.tile(
                            [DH, 512], f32, tag="psS", name=f"b1r{jp}{half}"
                        )
                        nc.tensor.matmul(
                            br_ps,
                            ones_r64[DH : DH + 1, :],
                            rs[DH : DH + 1, sl],
                            start=True,
                            stop=True,
                        )
                        brc = brp.tile([DH, 512], f32, tag="brc", name=f"b1c{jp}{half}")
                        nc.vector.tensor_copy(brc, br_ps)
                        ctxn = ctxp.tile([DH, SH], f8, tag="ctxh", name=f"c1n{jp}{half}")
                        nc.vector.tensor_mul(ctxn, ctx_ps[0:DH, sl], brc)
                        nc.sync.dma_start(
                            out=CTX1[hb : hb + DH, jp, :], in_=ctxn
                        )

                bo = par[pre + "bo"]

                def ev_o(m, ps):
                    t = sqp.tile([P, 512], f16, tag="sq", name=f"o1e{m}")
                    nc.scalar.activation(
                        t, ps, AF.Identity, bias=bo[:, m : m + 1]
                    )
                    nc.vector.tensor_add(Z1a[:, m, :], t, S1T[:, m, :])

                fm_proj8(CTX1, ow8, SH, ev_o)

                ag, ab = par[pre + "ag"], par[pre + "ab"]

                def out_att(k, c, t1, g_t, b_t, sl):
                    nc.vector.tensor_scalar(
                        ATT1[:, k, sl], t1, g_t[:, k : k + 1], b_t[:, k : k + 1],
                        OP.mult, OP.add,
                    )

                layernorm(Z1a, SH, ag, ab, out_att)

                w1 = load_w(din[pre + "w1T"], D, ND)
                b1 = par[pre + "b1"]

                def ev_w1(m, ps):
                    nc.scalar.activation(
                        H11[:, m, :], ps, AF.Gelu, bias=b1[:, m : m + 1]
                    )

                fm_proj([ATT1[:, k, :] for k in range(ND)], w1, SH, ev_w1)

                w28 = load_w8(din[pre + "w2T8"], ND, D)
                b2 = par[pre + "b2"]

                def ev_w2(m, ps):
                    t = sqp.tile([P, 512], f16, tag="sq", name=f"w21e{m}")
                    nc.scalar.activation(
                        t, ps, AF.Identity, bias=b2[:, m : m + 1]
                    )
                    nc.vector.tensor_add(Z1b[:, m, :], t, ATT1[:, m, :])

                fm_proj8(H11, w28, SH, ev_w2)

                fg, fb = par[pre + "fg"], par[pre + "fb"]
                layernorm(Z1b, SH, fg, fb, out_ctx1)

            blk1()

        # ================= POOL + FINAL =====================================
        with tc.tile_pool(name="late", bufs=2) as lp, tc.tile_pool(
            name="wbig", bufs=14
        ) as wb:
            # weight prefetch first: overlaps the whole pool chain
            pw_t = []
            for k in range(NS):
                t = wb.tile([P, S], f16, tag="wb", name=f"pw{k}")
                nc.sync.dma_start(out=t, in_=din["pwT"].ap()[k * P : (k + 1) * P, :])
                pw_t.append(t)
            pbT = wb.tile([P, NS], f32, tag="wb", name="pbT")
            nc.gpsimd.dma_start(out=pbT, in_=din["pbT"].ap())
            fin_t = []
            for k in range(2 * ND):
                t = wb.tile([P, D], f16, tag="wb", name=f"fin{k}")
                nc.sync.dma_start(
                    out=t, in_=din["finT"].ap()[k * P : (k + 1) * P, :]
                )
                fin_t.append(t)

            # pool matmul on M_viewT: poolT[128*jt+p, r] accumulated over
            # c-tiles t; evacuate straight into app^T layout via 3 strided
            # activations per jt (APPT[p, jd, sg+4k] = poolT[.., i+3k]).
            APPT = lp.tile([P, ND, SH], f16, tag="lt", name="APPT")
            for jt in range(NS):
                ps = psA.tile([P, 3 * P], f32, tag="psA", name=f"plps{jt}")
                for t in range(NS):
                    nc.tensor.matmul(
                        ps,
                        _r(pw_t[t][:, jt * P : (jt + 1) * P]),
                        _r(MVT[:, t, :]),
                        start=(t == 0),
                        stop=(t == NS - 1),
                    )
                for i in range(3):
                    u2 = 8 * i + jt
                    sg, jd = divmod(u2, 6)
                    nc.scalar.activation(
                        APPT[:, jd, sg : SH : 4],
                        ps[:, i : 3 * P : 3],
                        AF.Identity,
                        bias=pbT[:, jt : jt + 1],
                    )
            # final: out' = finT.T @ [feats_half ; app]
            OUTT = lp.tile([P, ND, SH], f32, tag="lt", name="OUTT")
            for m in range(ND):
                ps = psS.tile([P, SH], f32, tag="psS", name=f"fps{m}")
                for ki in range(2 * ND):
                    rhs = (
                        FEATS[:, ki, 0:SH]
                        if ki < ND
                        else APPT[:, ki - ND, :]
                    )
                    nc.tensor.matmul(
                        ps,
                        _r(fin_t[ki][:, m * P : (m + 1) * P]),
                        _r(rhs),
                        start=(ki == 0),
                        stop=(ki == 2 * ND - 1),
                    )
                nc.scalar.activation(
                    OUTT[:, m, :], ps, AF.Identity, bias=finb[:, m : m + 1]
                )
            nc.sync.dma_start(
                out=outT.ap().rearrange("(j p) s -> p j s", p=P), in_=OUTT
            )


def _q8(x):
    import ml_dtypes

    return np.ascontiguousarray(
        np.clip(np.asarray(x, np.float32), -240.0, 240.0)
    ).astype(ml_dtypes.float8_e4m3)


def _prep_inputs(inputs):
    e = np.ascontiguousarray(np.asarray(inputs["e"], dtype=np.float32))
    f = np.ascontiguousarray(np.asarray(inputs["f"], dtype=np.float32))
    wq = np.asarray(inputs["wq"], np.float32)
    wk = np.asarray(inputs["wk"], np.float32)
    wv = np.asarray(inputs["wv"], np.float32)
    wo = np.asarray(inputs["wo"], np.float32)
    bq = np.asarray(inputs["bq"], np.float32)
    bk = np.asarray(inputs["bk"], np.float32)
    bv = np.asarray(inputs["bv"], np.float32)
    bo = np.asarray(inputs["bo"], np.float32)
    ag = np.asarray(inputs["attn_ln_g"], np.float32)
    ab = np.asarray(inputs["attn_ln_b"], np.float32)
    w1 = np.asarray(inputs["ffn_w1"], np.float32)
    b1 = np.asarray(inputs["ffn_b1"], np.float32)
    w2 = np.asarray(inputs["ffn_w2"], np.float32)
    b2 = np.asarray(inputs["ffn_b2"], np.float32)
    fg = np.asarray(inputs["ffn_ln_g"], np.float32)
    fb = np.asarray(inputs["ffn_ln_b"], np.float32)
    pw = np.asarray(inputs["pool_w"], np.float32)
    pb = np.asarray(inputs["pool_b"], np.float32)
    fw = np.asarray(inputs["final_w"], np.float32)
    fnb = np.asarray(inputs["final_b"], np.float32)

    def vec6(v):
        return np.ascontiguousarray(v.reshape(ND, P).T)

    scale = 1.0 / math.sqrt(DH)
    in_maps = []
    for c in range(8):
        ti, b, h = c // 4, (c // 2) % 2, c % 2
        src = e if ti == 0 else f
        s1 = f if ti == 0 else e
        own = slice(SH * h, SH * (h + 1))
        oth = slice(SH * (1 - h), SH * (2 - h))
        src_b = src[:, b, :]
        src_perm = np.concatenate([src_b[own], src_b[oth]], axis=0)
        m = {
            "srcT": np.ascontiguousarray(src_perm.T).astype(np.float16),
            "srcT8": _q8(src_perm.T),
            "s1T": np.ascontiguousarray(s1[own, b, :].T).astype(np.float16),
            "s1T8": _q8(s1[own, b, :].T),
            "pwT": np.ascontiguousarray(
                np.concatenate([pw[ti].T, pb[ti][None, :]], axis=0)
            ).astype(np.float16),
            "pbT": np.ascontiguousarray(pb[ti].reshape(NS, P).T),
            "finT": np.ascontiguousarray(fw[ti].T).astype(np.float16),
            "finb": vec6(fnb[ti]),
        }
        for li in (0, 1):
            # vT8 planes: 0-5 = wv.T head-blocks, 6 = row0 bias/ones, 7 = 0
            vT8 = np.zeros((NS * P, VWP), np.float32)
            wvT = wv[ti, li].T
            for hh in range(H):
                vT8[0:D, (DH + 1) * hh : (DH + 1) * hh + DH] = wvT[
                    :, DH * hh : DH * (hh + 1)
                ]
                vT8[D, (DH + 1) * hh : (DH + 1) * hh + DH] = bv[
                    ti, li, DH * hh : DH * (hh + 1)
                ]
                vT8[D, (DH + 1) * hh + DH] = 1.0
            m.update(
                {
                    f"l{li}_qT8": _q8(wq[ti, li].T * scale),
                    f"l{li}_kT8": _q8(wk[ti, li].T),
                    f"l{li}_vT8": _q8(vT8),
                    f"l{li}_oT8": _q8(wo[ti, li].T),
                    f"l{li}_w1T": np.ascontiguousarray(w1[ti, li].T).astype(np.float16),
                    f"l{li}_w2T8": _q8(w2[ti, li].T),
                    f"l{li}_bq": vec6(bq[ti, li] * scale),
                    f"l{li}_bk": vec6(bk[ti, li]),
                    f"l{li}_bo": vec6(bo[ti, li]),
                    f"l{li}_b1": vec6(b1[ti, li]),
                    f"l{li}_b2": vec6(b2[ti, li]),
                    f"l{li}_ag": vec6(ag[ti, li]),
                    f"l{li}_ab": vec6(ab[ti, li]),
                    f"l{li}_fg": vec6(fg[ti, li]),
                    f"l{li}_fb": vec6(fb[ti, li]),
                }
            )
        in_maps.append(m)
    return in_maps


def get_program():
    if "nc" not in _BUILT:
        _BUILT["nc"] = _build_program()
    return _BUILT["nc"]


def kernel(**inputs):
    from concourse.bass_utils import run_bass_kernel_spmd

    nc = get_program()
    in_maps = _prep_inputs(inputs)
    res = run_bass_kernel_spmd(nc, in_maps, core_ids=list(range(8)))
    c_e_f = np.empty((S, B, D), np.float32)
    c_f_e = np.empty((S, B, D), np.float32)
    for c in range(8):
        ti, b, h = c // 4, (c // 2) % 2, c % 2
        dst = c_e_f if ti == 0 else c_f_e
        dst[SH * h : SH * (h + 1), b, :] = res.results[c]["outT"].T
    return c_e_f, c_f_e



# revision 28
# speedup vs baseline: 1.3287x; 1.0085x over previous
# Trainium2 Bass kernel for nn_Cross_Transformer (dense_transformer).
#
# Sharding: 8 cores = 2 towers x 2 batches x 2 sequence-halves.
# Each core computes block0 (self-attention) in full (its inputs are permuted
# so its own half leads, keeping the program SPMD-uniform), then its half of
# block1 (cross-attention), pool, and final projection. No collectives.
#
# Layout: activations are feature-major [D on partitions, S on free] so every
# projection is lhsT=W^T tiles (stationary) x X' (moving). Matmuls run as
# float32r (1 cyc/row at N>=256). Attention probabilities/V run in fp16.
# LayerNorm over D (= partitions) uses ones-column matmuls for sums and a
# K=1 ones-row matmul to broadcast per-column stats across partitions.

import math

import numpy as np

S = 1024
B = 2
D = 768
H = 12
DH = 64
EPS = 1e-6
SH = S // 2  # 512, per-core block1 rows
P = 128
ND = D // P  # 6 d-tiles
NS = S // P  # 8 s-tiles
VW = H * (DH + 1)  # 780: v row-major padded with a ones column per head
VWP = 784  # fp8 DoubleRow needs the st-pair stride 16B-aligned

F32 = None  # filled lazily (mybir.dt.float32)
_BUILT = {}


def _dt():
    from concourse import mybir

    return mybir.dt


def _r(ap):
    """View an fp32 AP as float32r for full-rate PE matmuls; fp16 passes through."""
    dt = _dt()
    return ap.bitcast(dt.float32r) if ap.dtype == dt.float32 else ap


def _build_program():
    import concourse.bacc as bacc
    import concourse.tile as tile
    from concourse import mybir
    from concourse.masks import make_identity

    dt = mybir.dt
    f32 = dt.float32
    f16 = dt.float16
    AF = mybir.ActivationFunctionType
    OP = mybir.AluOpType

    nc = bacc.Bacc("TRN2", target_bir_lowering=False, debug=False, num_devices=8)

    # ---- DRAM I/O ----
    din = {}

    def dram_in(name, shape, dty=None):
        din[name] = nc.dram_tensor(
            name, list(shape), dty or f16, kind="ExternalInput"
        )
        return din[name]

    f8 = mybir.dt.float8e4
    dram_in("srcT", (D, S))
    dram_in("srcT8", (D, S), f8)
    dram_in("s1T", (D, SH))
    dram_in("s1T8", (D, SH), f8)
    for li in (0, 1):
        dram_in(f"l{li}_qT8", (D, D), f8)
        dram_in(f"l{li}_kT8", (D, D), f8)
        dram_in(f"l{li}_vT8", (NS * P, VWP), f8)
        dram_in(f"l{li}_oT8", (D, D), f8)
        dram_in(f"l{li}_w1T", (D, D))
        dram_in(f"l{li}_w2T8", (D, D), f8)
    dram_in("pwT", (S + 1, S))
    dram_in("finT", (2 * D, D))
    # all small per-channel params in one tensor: 18 bias/LN vectors
    # (2 layers x 9) + finb + pbT
    dram_in("pars", (P, 19 * ND + NS), f32)

    outT = nc.dram_tensor("outT", [D, SH], f32, kind="ExternalOutput")

    with tile.TileContext(nc) as tc:
        _emit(nc, tc, tile, dt, AF, OP, din, outT, make_identity)

    nc.compile()
    return nc


def _emit(nc, tc, tile, dt, AF, OP, din, outT, make_identity):
    f32 = dt.float32
    f16 = dt.float16
    f8 = dt.float8e4
    from concourse import mybir as _mb

    PM = _mb.MatmulPerfMode
    import contextlib

    es = contextlib.ExitStack()
    with es:
        persist = es.enter_context(tc.tile_pool(name="persist", bufs=1))
        wp = es.enter_context(tc.tile_pool(name="wp", bufs=9))
        w8p = es.enter_context(tc.tile_pool(name="w8p", bufs=3))
        psA = es.enter_context(tc.tile_pool(name="psA", bufs=2, space="PSUM"))
        psC = es.enter_context(tc.tile_pool(name="psC", bufs=1, space="PSUM"))
        psS = es.enter_context(tc.tile_pool(name="psS", bufs=2, space="PSUM"))
        expool = es.enter_context(tc.tile_pool(name="expool", bufs=4))
        ctxp = es.enter_context(tc.tile_pool(name="ctxp", bufs=2))
        sqp = es.enter_context(tc.tile_pool(name="sqp", bufs=4))
        brp = es.enter_context(tc.tile_pool(name="brp", bufs=2))
        smp = es.enter_context(tc.tile_pool(name="smp", bufs=2))

        # --- constants ---
        ident = persist.tile([P, P], f16, name="ident")
        make_identity(nc, ident)
        ones_col = persist.tile([P, 1], f16, name="ones_col")
        nc.vector.memset(ones_col, 1.0)
        ones_r64 = persist.tile([DH + 1, DH], f16, name="ones_r64")
        nc.vector.memset(ones_r64[DH : DH + 1, :], 1.0)
        ones_r128 = persist.tile([1, P], f16, name="ones_r128")
        nc.vector.memset(ones_r128, 1.0)

        # --- small params (biases, LN): one tile, one DMA ---
        PARS = persist.tile([P, 19 * ND + NS], f32, name="PARS")
        nc.sync.dma_start(out=PARS, in_=din["pars"].ap())
        par = {}
        idx = 0
        for li in (0, 1):
            for bn in ("bq", "bk", "bo", "b1", "b2", "ag", "ab", "fg", "fb"):
                par[f"l{li}_{bn}"] = PARS[:, idx : idx + ND]
                idx += ND
        finb = PARS[:, idx : idx + ND]
        idx += ND
        pbT_par = PARS[:, idx : idx + NS]

        # persistent activations
        FEATS = persist.tile([P, ND + 1, S], f16, name="FEATS")
        nc.vector.memset(FEATS[0:1, ND, :], 1.0)
        FEATSq = persist.tile([P, NS, S], f8, name="FEATSq")
        nc.vector.memset(FEATSq[:, ND : ND + 2, :], 0.0)
        nc.vector.memset(FEATSq[0:1, ND, :], 1.0)
        S1T = persist.tile([P, ND, SH], f16, name="S1T")
        nc.sync.dma_start(
            out=S1T, in_=din["s1T"].ap().rearrange("(j p) s -> p j s", p=P)
        )
        S1Tq = persist.tile([P, ND, SH], f8, name="S1Tq")
        nc.sync.dma_start(
            out=S1Tq, in_=din["s1T8"].ap().rearrange("(j p) s -> p j s", p=P)
        )
        # M_viewT [c-tile partitions, c-tile idx, r]: M_view[r, 128t+p] with
        # r = 3k+rho -> ctx1_ln[4k + u//6, d=128*(u%6)+p], u = 8*rho + t.
        # Written directly (strided) by block1's final LN.
        MVT = persist.tile([P, NS, 3 * P], f16, name="MVT")
        Q1 = persist.tile([P, ND, SH], f16, name="Q1")
        rb_t = persist.tile([P, S], f16, name="rb_t")
        mrb_t = persist.tile([P, S], f16, name="mrb_t")
        # LN small stats rows
        lnm = persist.tile([1, S], f32, name="lnm")
        lns2 = persist.tile([1, S], f32, name="lns2")
        lnt = persist.tile([1, S], f32, name="lnt")
        lnr16 = persist.tile([1, S], f16, name="lnr16")
        lnmr16 = persist.tile([1, S], f16, name="lnmr16")

        def load_w(dram_h, width, nk, tagsuf=""):
            """DMA weight k-tiles [P, width] (+ optional trailing [1, width])."""
            ap = dram_h.ap()
            tiles = []
            for t in range(nk):
                wt = wp.tile([P, width], f16, tag="w", name=f"w_{dram_h.name}_{t}")
                nc.sync.dma_start(out=wt, in_=ap[t * P : (t + 1) * P, :])
                tiles.append(wt)
            return tiles

        def fm_proj(x_ktiles, w_tiles, Sx, evac, extra_k=None):
            """Feature-major projection: out[m] = sum_k w[k][:,m].T @ x[k].
            x_ktiles: list of APs [kp, Sx]; w_tiles: list of APs [kp, D].
            evac(m, ps): consume psum [P, Sx]."""
            nch = Sx // 512
            ks = list(zip(x_ktiles, w_tiles))
            if extra_k is not None:
                ks.append(extra_k)
            for m in range(ND):
                ps = psA.tile([P, Sx], f32, tag="psA", name=f"ps_m{m}")
                for c in range(nch):
                    sl = slice(512 * c, 512 * (c + 1))
                    for ki, (xk, wk) in enumerate(ks):
                        nc.tensor.matmul(
                            ps[:, sl],
                            _r(wk[:, m * P : (m + 1) * P]),
                            _r(xk[:, sl]),
                            start=(ki == 0),
                            stop=(ki == len(ks) - 1),
                        )
                evac(m, ps)

        def load_w8(dram_h, nk, width):
            """One-DMA fp8 weight load: [P, nk, width] (k-tile planes)."""
            wt = w8p.tile([P, nk, width], f8, tag="w8", name=f"w8_{dram_h.name}")
            nc.sync.dma_start(
                out=wt, in_=dram_h.ap().rearrange("(t p) m -> p t m", p=P)
            )
            return wt

        def fm_proj8(Xq, w8, Sx, evac, mlist=None):
            """fp8 DoubleRow projection: contraction over 3 k-tile pairs.
            Xq [P, >=6, Sfull] fp8 planes; w8 [P, 6, D] fp8."""
            nch = Sx // 512
            for m in mlist if mlist is not None else range(ND):
                ps = psA.tile([P, Sx], f32, tag="psA", name=f"ps_m{m}")
                for c in range(nch):
                    sl = slice(512 * c, 512 * (c + 1))
                    for kp in range(3):
                        nc.tensor.matmul(
                            ps[:, sl],
                            w8[:, 2 * kp : 2 * kp + 2, m * P : (m + 1) * P],
                            Xq[:, 2 * kp : 2 * kp + 2, sl],
                            start=(kp == 0),
                            stop=(kp == 2),
                            perf_mode=PM.DoubleRow,
                        )
                evac(m, ps)

        def v_proj8(Xq8, vw8, Vt, li, stlist=None):
            """fp8 DoubleRow v-projection (keys-major, 4 plane-pairs: 6 data
            + ones/bias plane + zero plane)."""
            for st in stlist if stlist is not None else range(NS):
                ps = psA.tile([P, VWP], f32, tag="psA", name=f"vps{li}_{st}")
                ssl = slice(st * P, (st + 1) * P)
                for c0, c1 in ((0, 512), (512, VWP)):
                    for kp in range(4):
                        nc.tensor.matmul(
                            ps[:, c0:c1],
                            Xq8[:, 2 * kp : 2 * kp + 2, ssl],
                            vw8[:, 2 * kp : 2 * kp + 2, c0:c1],
                            start=(kp == 0),
                            stop=(kp == 3),
                            perf_mode=PM.DoubleRow,
                        )
                with nc.allow_low_precision(reason="fp8 V"):
                    nc.vector.tensor_copy(Vt[:, st, 0:VW], ps[:, 0:VW])

        def layernorm(Zt, Sx, g_t, b_t, out_fn):
            """LN over partitions(d) of Zt [P, ND, Sx] (Bessel std + eps).
            out_fn(k, c, src_ap, sl): writes result tile."""
            nch = Sx // 512
            for c in range(nch):
                sl = slice(512 * c, 512 * (c + 1))
                sum_ps = psS.tile([1, 512], f32, tag="psS", name=f"lnsum{c}")
                for k in range(ND):
                    nc.tensor.matmul(
                        sum_ps,
                        _r(ones_col),
                        _r(Zt[:, k, sl]),
                        start=(k == 0),
                        stop=(k == ND - 1),
                    )
                nc.scalar.activation(
                    lnm[:, sl], sum_ps, AF.Identity, scale=1.0 / D
                )
                sq_ps = psS.tile([1, 512], f32, tag="psS", name=f"lnsq{c}")
                for k in range(ND):
                    sq = sqp.tile([P, 512], f16, tag="sq", name=f"sq{k}{c}")
                    nc.vector.tensor_mul(sq, Zt[:, k, sl], Zt[:, k, sl])
                    nc.tensor.matmul(
                        sq_ps,
                        _r(ones_col),
                        _r(sq),
                        start=(k == 0),
                        stop=(k == ND - 1),
                    )
                nc.scalar.activation(
                    lns2[:, sl], sq_ps, AF.Identity, scale=1.0 / (D - 1)
                )
                # per-chunk stats chain so chunk 0 applies while chunk 1 sums
                nc.scalar.activation(
                    lnt[:, sl], lnm[:, sl], AF.Square,
                    scale=math.sqrt(D / (D - 1.0)),
                )
                nc.vector.tensor_sub(lns2[:, sl], lns2[:, sl], lnt[:, sl])
                nc.scalar.sqrt(lns2[:, sl], lns2[:, sl])
                nc.vector.tensor_scalar_add(lns2[:, sl], lns2[:, sl], EPS)
                nc.vector.reciprocal(lnt[:, sl], lns2[:, sl])  # r
                nc.vector.tensor_mul(lnm[:, sl], lnm[:, sl], lnt[:, sl])  # m*r
                nc.scalar.activation(lnr16[:, sl], lnt[:, sl], AF.Identity)
                nc.scalar.activation(lnmr16[:, sl], lnm[:, sl], AF.Identity)
            for c in range(nch):
                sl = slice(512 * c, 512 * (c + 1))
                rb_ps = psS.tile([P, 512], f32, tag="psS", name=f"rbps{c}")
                nc.tensor.matmul(
                    rb_ps, ones_r128, lnr16[0:1, sl], start=True, stop=True
                )
                nc.vector.tensor_copy(rb_t[:, sl], rb_ps)
                mrb_ps = psS.tile([P, 512], f32, tag="psS", name=f"mrbps{c}")
                nc.tensor.matmul(
                    mrb_ps, ones_r128, lnmr16[0:1, sl], start=True, stop=True
                )
                nc.vector.tensor_copy(mrb_t[:, sl], mrb_ps)
                for k in range(ND):
                    t1 = sqp.tile([P, 512], f16, tag="sq", name=f"ap{k}{c}")
                    nc.vector.tensor_mul(t1, Zt[:, k, sl], rb_t[:, sl])
                    nc.vector.tensor_sub(t1, t1, mrb_t[:, sl])
                    out_fn(k, c, t1, g_t, b_t, sl)

        def attention(Sq, Qt, Kt, Vt, ctx_sink):
            """ctx_sink(h, ap[64, Sq]) receives normalized per-head context.
            Probabilities and V run in fp8e4; P@V uses DoubleRow over
            st-pairs (256-row contraction at 0.5 cyc/row)."""
            nq = Sq // 512
            for h in range(H):
                hb = DH * (h % 2)
                j = h // 2
                ctx_ps = psC.tile([DH + 1, Sq], f32, tag="psC", name=f"ctx{h}")
                ex2 = None
                for st in range(NS):
                    sc_ps = psA.tile([P, Sq], f32, tag="psA", name=f"sc{h}_{st}")
                    for c in range(nq):
                        sl = slice(512 * c, 512 * (c + 1))
                        nc.tensor.matmul(
                            sc_ps[:, sl],
                            _r(Kt[hb : hb + DH, j, st * P : (st + 1) * P]),
                            _r(Qt[hb : hb + DH, j, sl]),
                            start=True,
                            stop=True,
                        )
                    if st % 2 == 0:
                        ex2 = expool.tile(
                            [P, 2, Sq], f8, tag="ex", name=f"ex{h}_{st // 2}"
                        )
                    with nc.allow_low_precision(reason="fp8 probs"):
                        nc.scalar.activation(ex2[:, st % 2, :], sc_ps, AF.Exp)
                    if st % 2 == 1:
                        sp = st // 2
                        for c in range(nq):
                            sl = slice(512 * c, 512 * (c + 1))
                            nc.tensor.matmul(
                                ctx_ps[:, sl],
                                Vt[:, 2 * sp : 2 * sp + 2,
                                   (DH + 1) * h : (DH + 1) * h + DH + 1],
                                ex2[:, :, sl],
                                start=(sp == 0),
                                stop=(sp == NS // 2 - 1),
                                perf_mode=PM.DoubleRow,
                            )
                rs = smp.tile([DH + 1, Sq], f16, tag="rs", name=f"rs{h}")
                with nc.allow_low_precision(reason="softmax recip fp16"):
                    nc.vector.reciprocal(
                        rs[DH : DH + 1, :], ctx_ps[DH : DH + 1, :]
                    )
                ctxn = ctxp.tile([DH, Sq], f8, tag="ctxh", name=f"ctxn{h}")
                for c in range(nq):
                    sl = slice(512 * c, 512 * (c + 1))
                    br_ps = psS.tile([DH, 512], f32, tag="psS", name=f"br{h}{c}")
                    nc.tensor.matmul(
                        br_ps,
                        ones_r64[DH : DH + 1, :],
                        rs[DH : DH + 1, sl],
                        start=True,
                        stop=True,
                    )
                    brc = brp.tile([DH, 512], f32, tag="brc", name=f"brc{h}{c}")
                    nc.vector.tensor_copy(brc, br_ps)
                    nc.vector.tensor_mul(ctxn[:, sl], ctx_ps[0:DH, sl], brc)
                ctx_sink(h, ctxn)

        # ================= BLOCK 0 (full S, self-attention on src) =========
        with tc.tile_pool(name="b0a", bufs=4) as act6, tc.tile_pool(
            name="b0x", bufs=1
        ) as act7, tc.tile_pool(name="b0v", bufs=1) as vp0:
            X0 = act7.tile([P, ND + 1, S], f16, tag="a7", name="X0")
            nc.sync.dma_start(
                out=X0[:, 0:ND, :],
                in_=din["srcT"].ap().rearrange("(j p) s -> p j s", p=P),
            )
            nc.vector.memset(X0[0:1, ND, :], 1.0)
            X0q = act7.tile([P, NS, S], f8, tag="x8", name="X0q")
            nc.sync.dma_start(
                out=X0q[:, 0:ND, :],
                in_=din["srcT8"].ap().rearrange("(j p) s -> p j s", p=P),
            )
            nc.vector.memset(X0q[:, ND : ND + 2, :], 0.0)
            nc.vector.memset(X0q[0:1, ND, :], 1.0)

            x_k = [X0[:, k, :] for k in range(ND)]
            V0 = vp0.tile([P, NS, VWP], f8, name="V0")

            # block1 q-projection depends only on inputs: emit first to fill
            # the startup bubble while block0 weights stream in.
            q1w8 = load_w8(din["l1_qT8"], ND, D)
            bq1 = par["l1_bq"]

            def ev_q1(m, ps):
                nc.vector.tensor_scalar_add(Q1[:, m, :], ps, bq1[:, m : m + 1])

            fm_proj8(S1Tq, q1w8, SH, ev_q1)

            def blk(li, Sq, Qsrc8, KVsrc8, resid_k, Vt, CTXa, Za, Zb,
                    ATT, OUTLN_fn):
                """One transformer block in feature-major layout."""
                pre = f"l{li}_"
                # --- q/k (feature-major, fp8 DoubleRow) ---
                qw8 = load_w8(din[pre + "qT8"], ND, D)
                Qt = CTXa["Q"]
                bq = par[pre + "bq"]

                def ev_q(m, ps):
                    nc.vector.tensor_scalar_add(Qt[:, m, 0:Sq], ps, bq[:, m : m + 1])

                fm_proj8(Qsrc8, qw8, Sq, ev_q)

                kw8 = load_w8(din[pre + "kT8"], ND, D)
                Kt = CTXa["K"]
                bk = par[pre + "bk"]

                def ev_k(m, ps):
                    nc.vector.tensor_scalar_add(Kt[:, m, :], ps, bk[:, m : m + 1])

                fm_proj8(KVsrc8, kw8, S, ev_k)

                # --- v (keys-major, fp8 DoubleRow, bias via plane 6/7) ---
                vw8 = load_w8(din[pre + "vT8"], NS, VWP)
                v_proj8(KVsrc8, vw8, Vt, li)

                # --- attention ---
                ow8 = load_w8(din[pre + "oT8"], ND, D)
                CTXh = CTXa["CTX"]

                def sink(h, ctxn):
                    hb = DH * (h % 2)
                    j = h // 2
                    nc.sync.dma_start(out=CTXh[hb : hb + DH, j, 0:Sq], in_=ctxn)

                attention(Sq, Qt, Kt, Vt, sink)

                # --- o-proj + bias + residual -> Za ---
                bo = par[pre + "bo"]

                def ev_o(m, ps):
                    for c in range(Sq // 512):
                        sl = slice(512 * c, 512 * (c + 1))
                        t = sqp.tile([P, 512], f16, tag="sq", name=f"oe{m}_{c}")
                        nc.scalar.activation(
                            t, ps[:, sl], AF.Identity, bias=bo[:, m : m + 1]
                        )
                        nc.vector.tensor_add(Za[:, m, sl], t, resid_k[m][:, sl])

                fm_proj8(CTXh, ow8, Sq, ev_o)

                # --- LN (attn) -> ATT ---
                ag, ab = par[pre + "ag"], par[pre + "ab"]

                def out_att(k, c, t1, g_t, b_t, sl):
                    nc.vector.tensor_scalar(
                        ATT[:, k, sl], t1, g_t[:, k : k + 1], b_t[:, k : k + 1],
                        OP.mult, OP.add,
                    )

                layernorm(Za, Sq, ag, ab, out_att)

                # --- ffn w1 + gelu ---
                w1 = load_w(din[pre + "w1T"], D, ND)
                H1 = CTXa["H1"]
                b1 = par[pre + "b1"]

                def ev_w1(m, ps):
                    nc.scalar.activation(
                        H1[:, m, 0:Sq], ps, AF.Gelu, bias=b1[:, m : m + 1]
                    )

                fm_proj([ATT[:, k, 0:Sq] for k in range(ND)], w1, Sq, ev_w1)

                # --- ffn w2 + bias + residual -> Zb, LN -> OUTLN_fn ---
                w28 = load_w8(din[pre + "w2T8"], ND, D)
                b2 = par[pre + "b2"]

                def ev_w2(m, ps):
                    for c in range(Sq // 512):
                        sl = slice(512 * c, 512 * (c + 1))
                        t = sqp.tile([P, 512], f16, tag="sq", name=f"w2e{m}_{c}")
                        nc.scalar.activation(
                            t, ps[:, sl], AF.Identity, bias=b2[:, m : m + 1]
                        )
                        nc.vector.tensor_add(Zb[:, m, sl], t, ATT[:, m, sl])

                fm_proj8(H1, w28, Sq, ev_w2)

                fg, fb = par[pre + "fg"], par[pre + "fb"]
                layernorm(Zb, Sq, fg, fb, OUTLN_fn)

            # block0 tensor buffers (rotating in act6)
            Q0 = act6.tile([P, ND, S], f16, tag="a6", name="Q0")
            K0 = act6.tile([P, ND, S], f16, tag="a6", name="K0")
            CTX0 = act6.tile([P, ND, S], f8, tag="a6", name="CTX0")
            Z0a = act6.tile([P, ND, S], f16, tag="a6", name="Z0a")
            ATT0 = act6.tile([P, ND, S], f16, tag="a6", name="ATT0")
            H10 = act6.tile([P, ND, S], f8, tag="a6", name="H10")
            Z0b = act6.tile([P, ND, S], f16, tag="a6", name="Z0b")

            def out_feats(k, c, t1, g_t, b_t, sl):
                nc.vector.tensor_scalar(
                    FEATS[:, k, sl], t1, g_t[:, k : k + 1], b_t[:, k : k + 1],
                    OP.mult, OP.add,
                )
                with nc.allow_low_precision(reason="fp8 feats"):
                    nc.vector.tensor_scalar(
                        FEATSq[:, k, sl], t1, g_t[:, k : k + 1],
                        b_t[:, k : k + 1], OP.mult, OP.add,
                    )

            blk(
                0, S, X0q, X0q, x_k, V0,
                {"Q": Q0, "K": K0, "CTX": CTX0, "H1": H10},
                Z0a, Z0b, ATT0, out_feats,
            )

        # ================= BLOCK 1 (half S on q-side, cross-attention) ======
        with tc.tile_pool(name="b1a", bufs=4) as a6h, tc.tile_pool(
            name="b1b", bufs=1
        ) as a6f, tc.tile_pool(name="b1v", bufs=1) as vp1:
            feats_k = [FEATS[:, k, :] for k in range(ND)]
            s1_k = [S1T[:, k, :] for k in range(ND)]
            K1 = a6f.tile([P, ND, S], f16, tag="af", name="K1")
            CTX1 = a6h.tile([P, ND, SH], f8, tag="ah", name="CTX1")
            Z1a = a6h.tile([P, ND, SH], f16, tag="ah", name="Z1a")
            ATT1 = a6h.tile([P, ND, SH], f16, tag="ah", name="ATT1")
            H11 = a6h.tile([P, ND, SH], f8, tag="ah", name="H11")
            Z1b = a6h.tile([P, ND, SH], f16, tag="ah", name="Z1b")
            V1 = vp1.tile([P, NS, VWP], f8, name="V1")

            def out_ctx1(k, c, t1, g_t, b_t, sl):
                # write straight into M_viewT layout (4 strided slices)
                for s0 in range(4):
                    u = 6 * s0 + k
                    t_, rho = u % 8, u // 8
                    nc.vector.tensor_scalar(
                        MVT[:, t_, rho : 3 * P : 3], t1[:, s0 : SH : 4],
                        g_t[:, k : k + 1], b_t[:, k : k + 1],
                        OP.mult, OP.add,
                    )

            # reuse blk via closure over act6-> but buffers differ; inline call:
            # q from s1 (Sq=SH), k/v from feats (full S), residual = s1
            def blk1():
                pre = "l1_"
                kw8 = load_w8(din[pre + "kT8"], ND, D)
                bk = par[pre + "bk"]

                def ev_k(m, ps):
                    nc.vector.tensor_scalar_add(K1[:, m, :], ps, bk[:, m : m + 1])

                fm_proj8(FEATSq, kw8, S, ev_k)

                vw8 = load_w8(din[pre + "vT8"], NS, VWP)
                v_proj8(FEATSq, vw8, V1, 1)

                ow8 = load_w8(din[pre + "oT8"], ND, D)

                # head-pair attention: two heads share one [128,1024] psum
                for jp in range(ND):
                    ctx_ps = psC.tile(
                        [DH + 1, 2 * SH], f32, tag="psC", name=f"c1ps{jp}"
                    )
                    for st in range(NS):
                        sc_ps = psA.tile(
                            [P, 2 * SH], f32, tag="psA", name=f"s1ps{jp}_{st}"
                        )
                        for half in (0, 1):
                            hb = DH * half
                            sl = slice(SH * half, SH * (half + 1))
                            nc.tensor.matmul(
                                sc_ps[:, sl],
                                K1[hb : hb + DH, jp, st * P : (st + 1) * P],
                                Q1[hb : hb + DH, jp, :],
                                start=True,
                                stop=True,
                            )
                        if st % 2 == 0:
                            ex2 = expool.tile(
                                [P, 2, 2 * SH], f8, tag="ex",
                                name=f"e1x{jp}_{st // 2}",
                            )
                        with nc.allow_low_precision(reason="fp8 probs"):
                            nc.scalar.activation(ex2[:, st % 2, :], sc_ps, AF.Exp)
                        if st % 2 == 1:
                            sp = st // 2
                            for half in (0, 1):
                                h = 2 * jp + half
                                sl = slice(SH * half, SH * (half + 1))
                                nc.tensor.matmul(
                                    ctx_ps[:, sl],
                                    V1[:, 2 * sp : 2 * sp + 2,
                                       (DH + 1) * h : (DH + 1) * h + DH + 1],
                                    ex2[:, :, sl],
                                    start=(sp == 0),
                                    stop=(sp == NS // 2 - 1),
                                    perf_mode=PM.DoubleRow,
                                )
                    rs = smp.tile([DH + 1, 2 * SH], f16, tag="rs", name=f"r1s{jp}")
                    with nc.allow_low_precision(reason="softmax recip fp16"):
                        nc.vector.reciprocal(
                            rs[DH : DH + 1, :], ctx_ps[DH : DH + 1, :]
                        )
                    for half in (0, 1):
                        hb = DH * half
                        sl = slice(SH * half, SH * (half + 1))
                        br_ps = psS.tile(
                            [DH, 512], f32, tag="psS", name=f"b1r{jp}{half}"
                        )
                        nc.tensor.matmul(
                            br_ps,
                            ones_r64[DH : DH + 1, :],
                            rs[DH : DH + 1, sl],
                            start=True,
                            stop=True,
                        )
                        brc = brp.tile([DH, 512], f32, tag="brc", name=f"b1c{jp}{half}")
                        nc.vector.tensor_copy(brc, br_ps)
                        ctxn = ctxp.tile([DH, SH], f8, tag="ctxh", name=f"c1n{jp}{half}")
                        nc.vector.tensor_mul(ctxn, ctx_ps[0:DH, sl], brc)
                        nc.sync.dma_start(
                            out=CTX1[hb : hb + DH, jp, :], in_=ctxn
                        )

                bo = par[pre + "bo"]

                def ev_o(m, ps):
                    t = sqp.tile([P, 512], f16, tag="sq", name=f"o1e{m}")
                    nc.scalar.activation(
                        t, ps, AF.Identity, bias=bo[:, m : m + 1]
                    )
                    nc.vector.tensor_add(Z1a[:, m, :], t, S1T[:, m, :])

                fm_proj8(CTX1, ow8, SH, ev_o)

                ag, ab = par[pre + "ag"], par[pre + "ab"]

                def out_att(k, c, t1, g_t, b_t, sl):
                    nc.vector.tensor_scalar(
                        ATT1[:, k, sl], t1, g_t[:, k : k + 1], b_t[:, k : k + 1],
                        OP.mult, OP.add,
                    )

                layernorm(Z1a, SH, ag, ab, out_att)

                w1 = load_w(din[pre + "w1T"], D, ND)
                b1 = par[pre + "b1"]

                def ev_w1(m, ps):
                    nc.scalar.activation(
                        H11[:, m, :], ps, AF.Gelu, bias=b1[:, m : m + 1]
                    )

                fm_proj([ATT1[:, k, :] for k in range(ND)], w1, SH, ev_w1)

                w28 = load_w8(din[pre + "w2T8"], ND, D)
                b2 = par[pre + "b2"]

                def ev_w2(m, ps):
                    t = sqp.tile([P, 512], f16, tag="sq", name=f"w21e{m}")
                    nc.scalar.activation(
                        t, ps, AF.Identity, bias=b2[:, m : m + 1]
                    )
                    nc.vector.tensor_add(Z1b[:, m, :], t, ATT1[:, m, :])

                fm_proj8(H11, w28, SH, ev_w2)

                fg, fb = par[pre + "fg"], par[pre + "fb"]
                layernorm(Z1b, SH, fg, fb, out_ctx1)

            blk1()

        # ================= POOL + FINAL =====================================
        with tc.tile_pool(name="late", bufs=2) as lp, tc.tile_pool(
            name="wbig", bufs=14
        ) as wb:
            # weight prefetch first: overlaps the whole pool chain
            pw_t = []
            for k in range(NS):
                t = wb.tile([P, S], f16, tag="wb", name=f"pw{k}")
                nc.sync.dma_start(out=t, in_=din["pwT"].ap()[k * P : (k + 1) * P, :])
                pw_t.append(t)
            fin_t = []
            for k in range(2 * ND):
                t = wb.tile([P, D], f16, tag="wb", name=f"fin{k}")
                nc.sync.dma_start(
                    out=t, in_=din["finT"].ap()[k * P : (k + 1) * P, :]
                )
                fin_t.append(t)

            # pool matmul on M_viewT: poolT[128*jt+p, r] accumulated over
            # c-tiles t; evacuate straight into app^T layout via 3 strided
            # activations per jt (APPT[p, jd, sg+4k] = poolT[.., i+3k]).
            APPT = lp.tile([P, ND, SH], f16, tag="lt", name="APPT")
            for jt in range(NS):
                ps = psA.tile([P, 3 * P], f32, tag="psA", name=f"plps{jt}")
                for t in range(NS):
                    nc.tensor.matmul(
                        ps,
                        _r(pw_t[t][:, jt * P : (jt + 1) * P]),
                        _r(MVT[:, t, :]),
                        start=(t == 0),
                        stop=(t == NS - 1),
                    )
                for i in range(3):
                    u2 = 8 * i + jt
                    sg, jd = divmod(u2, 6)
                    nc.scalar.activation(
                        APPT[:, jd, sg : SH : 4],
                        ps[:, i : 3 * P : 3],
                        AF.Identity,
                        bias=pbT_par[:, jt : jt + 1],
                    )
            # final: out' = finT.T @ [feats_half ; app]
            OUTT = lp.tile([P, ND, SH], f32, tag="lt", name="OUTT")
            for m in range(ND):
                ps = psS.tile([P, SH], f32, tag="psS", name=f"fps{m}")
                for ki in range(2 * ND):
                    rhs = (
                        FEATS[:, ki, 0:SH]
                        if ki < ND
                        else APPT[:, ki - ND, :]
                    )
                    nc.tensor.matmul(
                        ps,
                        _r(fin_t[ki][:, m * P : (m + 1) * P]),
                        _r(rhs),
                        start=(ki == 0),
                        stop=(ki == 2 * ND - 1),
                    )
                nc.scalar.activation(
                    OUTT[:, m, :], ps, AF.Identity, bias=finb[:, m : m + 1]
                )
            nc.sync.dma_start(
                out=outT.ap().rearrange("(j p) s -> p j s", p=P), in_=OUTT
            )


def _q8(x):
    import ml_dtypes

    return np.ascontiguousarray(
        np.clip(np.asarray(x, np.float32), -240.0, 240.0)
    ).astype(ml_dtypes.float8_e4m3)


def _prep_inputs(inputs):
    e = np.ascontiguousarray(np.asarray(inputs["e"], dtype=np.float32))
    f = np.ascontiguousarray(np.asarray(inputs["f"], dtype=np.float32))
    wq = np.asarray(inputs["wq"], np.float32)
    wk = np.asarray(inputs["wk"], np.float32)
    wv = np.asarray(inputs["wv"], np.float32)
    wo = np.asarray(inputs["wo"], np.float32)
    bq = np.asarray(inputs["bq"], np.float32)
    bk = np.asarray(inputs["bk"], np.float32)
    bv = np.asarray(inputs["bv"], np.float32)
    bo = np.asarray(inputs["bo"], np.float32)
    ag = np.asarray(inputs["attn_ln_g"], np.float32)
    ab = np.asarray(inputs["attn_ln_b"], np.float32)
    w1 = np.asarray(inputs["ffn_w1"], np.float32)
    b1 = np.asarray(inputs["ffn_b1"], np.float32)
    w2 = np.asarray(inputs["ffn_w2"], np.float32)
    b2 = np.asarray(inputs["ffn_b2"], np.float32)
    fg = np.asarray(inputs["ffn_ln_g"], np.float32)
    fb = np.asarray(inputs["ffn_ln_b"], np.float32)
    pw = np.asarray(inputs["pool_w"], np.float32)
    pb = np.asarray(inputs["pool_b"], np.float32)
    fw = np.asarray(inputs["final_w"], np.float32)
    fnb = np.asarray(inputs["final_b"], np.float32)

    def vec6(v):
        return np.ascontiguousarray(v.reshape(ND, P).T)

    scale = 1.0 / math.sqrt(DH)
    in_maps = []
    for c in range(8):
        ti, b, h = c // 4, (c // 2) % 2, c % 2
        src = e if ti == 0 else f
        s1 = f if ti == 0 else e
        own = slice(SH * h, SH * (h + 1))
        oth = slice(SH * (1 - h), SH * (2 - h))
        src_b = src[:, b, :]
        src_perm = np.concatenate([src_b[own], src_b[oth]], axis=0)
        m = {
            "srcT": np.ascontiguousarray(src_perm.T).astype(np.float16),
            "srcT8": _q8(src_perm.T),
            "s1T": np.ascontiguousarray(s1[own, b, :].T).astype(np.float16),
            "s1T8": _q8(s1[own, b, :].T),
            "pwT": np.ascontiguousarray(
                np.concatenate([pw[ti].T, pb[ti][None, :]], axis=0)
            ).astype(np.float16),
            "finT": np.ascontiguousarray(fw[ti].T).astype(np.float16),
        }
        pars_cols = []
        for li in (0, 1):
            pars_cols += [
                vec6(bq[ti, li] * scale), vec6(bk[ti, li]), vec6(bo[ti, li]),
                vec6(b1[ti, li]), vec6(b2[ti, li]), vec6(ag[ti, li]),
                vec6(ab[ti, li]), vec6(fg[ti, li]), vec6(fb[ti, li]),
            ]
        pars_cols.append(vec6(fnb[ti]))
        pars_cols.append(np.ascontiguousarray(pb[ti].reshape(NS, P).T))
        m["pars"] = np.ascontiguousarray(np.concatenate(pars_cols, axis=1))
        for li in (0, 1):
            # vT8 planes: 0-5 = wv.T head-blocks, 6 = row0 bias/ones, 7 = 0
            vT8 = np.zeros((NS * P, VWP), np.float32)
            wvT = wv[ti, li].T
            for hh in range(H):
                vT8[0:D, (DH + 1) * hh : (DH + 1) * hh + DH] = wvT[
                    :, DH * hh : DH * (hh + 1)
                ]
                vT8[D, (DH + 1) * hh : (DH + 1) * hh + DH] = bv[
                    ti, li, DH * hh : DH * (hh + 1)
                ]
                vT8[D, (DH + 1) * hh + DH] = 1.0
            m.update(
                {
                    f"l{li}_qT8": _q8(wq[ti, li].T * scale),
                    f"l{li}_kT8": _q8(wk[ti, li].T),
                    f"l{li}_vT8": _q8(vT8),
                    f"l{li}_oT8": _q8(wo[ti, li].T),
                    f"l{li}_w1T": np.ascontiguousarray(w1[ti, li].T).astype(np.float16),
                    f"l{li}_w2T8": _q8(w2[ti, li].T),
                }
            )
        in_maps.append(m)
    return in_maps


def get_program():
    if "nc" not in _BUILT:
        _BUILT["nc"] = _build_program()
    return _BUILT["nc"]


def kernel(**inputs):
    from concourse.bass_utils import run_bass_kernel_spmd

    nc = get_program()
    in_maps = _prep_inputs(inputs)
    res = run_bass_kernel_spmd(nc, in_maps, core_ids=list(range(8)))
    c_e_f = np.empty((S, B, D), np.float32)
    c_f_e = np.empty((S, B, D), np.float32)
    for c in range(8):
        ti, b, h = c // 4, (c // 2) % 2, c % 2
        dst = c_e_f if ti == 0 else c_f_e
        dst[SH * h : SH * (h + 1), b, :] = res.results[c]["outT"].T
    return c_e_f, c_f_e

